# revision 9
# baseline (speedup 1.0000x reference)
"""Trainium2 Bass kernel for a fused multi-head attention layer.

Math (per batch b):
    xh = x.reshape(S, H, d); q/k/v = xh @ W{q,k,v}[h] + b
    scores = q @ k^T  (per head);  scores[-1, -1024:] = -inf
    attn = softmax(scores, -1) / sqrt(D)
    o = concat_h(attn @ v);  proj = o @ Wo + bo
    out = LayerNorm(x + proj) * g + beta

Sharding: 8 cores = 2 batches x 4 query-blocks of 512 rows. Each core
computes K/V for its full batch (duplicated across the 4 cores of a
batch) and Q/attention/projection/LN for its own 512 query rows. No
collectives.

v2 design notes (vs the all-bf16 v1):
  * The exp stream is split between ScalarE (activation Exp -> fp8e4)
    and the DVE (Schraudolph fast-exp: round(a*s + 56) to int8 IS the
    fp8e4 bit pattern of exp(s); verified round-to-nearest+saturate on
    HW).  Both engines also share the PSUM->SBUF cast pool.
  * V and the attention weights are fp8e4; the PV matmuls run in
    DoubleRow perf mode contracting two 128-key chunks at once
    (lhsT [128,2,65] incl the ones-column, rhs [128,2,256]); the
    ones-column still yields the softmax denominator for free.
  * The seq-mask costs no per-chunk work: score chunks >= 8 use a
    second qT whose column 511 is zeroed on the masked core (GpSimd),
    making the masked scores 0 -> exp = 1 exactly; per-pair DoubleRow
    fixup matmuls with rhs = -mask subtract the spurious sum_v/count
    from the PV output and denominator.
  * v-casts are batched 4 chunks per DVE op; oT normalization mult
    runs on GpSimd.
"""

import numpy as np
import ml_dtypes

import concourse.bass as bass
import concourse.mybir as mybir
import concourse.tile as tile
from concourse import bacc
from concourse.bass import ds, ts
from concourse.bass_utils import run_bass_kernel_spmd

BF16 = mybir.dt.bfloat16
F32 = mybir.dt.float32
FP8 = mybir.dt.float8e4
I8 = mybir.dt.int8
AF = mybir.ActivationFunctionType
OP = mybir.AluOpType
DR = mybir.MatmulPerfMode.DoubleRow

B, S, D, H = 2, 2048, 1024, 16
d = 64            # head dim
NP = H // 2       # 8 head pairs
SQ = S // 4       # 512 query rows per core
TCK = S // 128    # 16 key chunks of 128
NCP = TCK // 2    # 8 chunk-pairs
SEQ_LEN = 1024
SCALE = float(np.sqrt(D))
LN_EPS = 1e-5
N_CORES = 8
EXPA = 8.0 / float(np.log(2.0))   # Schraudolph slope for e4m3 bits
EXPB = 56.0                        # 8 * bias(7)
# chunks handled by the DVE fast-exp (rest on ScalarE)
DVE_CHUNKS = (3, 7, 11, 14)


def _bcast(ap, p=128):
    """AP replicating `ap` across p partitions (partition step 0)."""
    return bass.AP(tensor=ap.tensor, offset=ap.offset, ap=[[0, p]] + list(ap.ap))


def build_nc(apply_gb=True, apply_qkvb=True):
    nc = bacc.Bacc("TRN2")

    xT = nc.dram_tensor("xT", [D, S], BF16, kind="ExternalInput")       # x[b].T
    xqT = nc.dram_tensor("xqT", [D, SQ], BF16, kind="ExternalInput")    # x[b,rows].T
    xq = nc.dram_tensor("xq", [128, 4, D], F32, kind="ExternalInput")   # x[b,rows]+bo
    wq = nc.dram_tensor("wq", [128, NP, 128], BF16, kind="ExternalInput")
    wk = nc.dram_tensor("wk", [128, NP, 128], BF16, kind="ExternalInput")
    wv = nc.dram_tensor("wv", [128, NP, 128], BF16, kind="ExternalInput")
    bqk = nc.dram_tensor("bqk", [128, 2 * NP], F32, kind="ExternalInput")
    bvt = nc.dram_tensor("bvt", [NP, 128], F32, kind="ExternalInput")
    wo = nc.dram_tensor("wo", [128, NP, D], BF16, kind="ExternalInput")
    gg = nc.dram_tensor("gg", [D], F32, kind="ExternalInput")
    bb = nc.dram_tensor("bb", [D], F32, kind="ExternalInput")
    msk = nc.dram_tensor("msk", [1, 1], F32, kind="ExternalInput")      # 0 if masked
    negm = nc.dram_tensor("negm", [128, 2, 16], FP8, kind="ExternalInput")
    out = nc.dram_tensor("out", [SQ, D], F32, kind="ExternalOutput")

    with tile.TileContext(nc) as tc:
        with (
            tc.tile_pool(name="singles", bufs=1) as singles,
            tc.tile_pool(name="xpool", bufs=2) as xpool,
            tc.tile_pool(name="kpool", bufs=2) as kpool,
            tc.tile_pool(name="qpool", bufs=2) as qpool,
            tc.tile_pool(name="qxpool", bufs=2) as qxpool,
            tc.tile_pool(name="vpool", bufs=2) as vpool,
            tc.tile_pool(name="epool", bufs=4) as epool,
            tc.tile_pool(name="rpool", bufs=2) as rpool,
            tc.tile_pool(name="ypool", bufs=4) as ypool,
            tc.tile_pool(name="stpool", bufs=4) as stpool,
            tc.tile_pool(name="psA", bufs=2, space="PSUM") as psA,
            tc.tile_pool(name="psB", bufs=2, space="PSUM") as psB,
            tc.tile_pool(name="psD", bufs=2, space="PSUM") as psD,
        ):
            # ---- warm-up: bridge the PE HAM clock gate until real MMs
            wu = singles.tile([128, 512], BF16)
            nc.vector.memset(wu, 0.0)
            for _ in range(8):
                wps = psD.tile([128, 512], F32, tag="qkv", name="wps")
                nc.tensor.matmul(wps, lhsT=wu[:, 0:128], rhs=wu,
                                 start=True, stop=True)

            # ---- constants / weights (contiguous host-prearranged DMAs).
            wk0_sb = singles.tile([128, 2, 128], BF16)
            wq0_sb = singles.tile([128, 1, 128], BF16)
            wv0_sb = singles.tile([128, 1, 128], BF16)
            wkr_sb = singles.tile([128, NP - 2, 128], BF16)
            wqr_sb = singles.tile([128, NP - 1, 128], BF16)
            wvr_sb = singles.tile([128, NP - 1, 128], BF16)
            msk_sb = singles.tile([128, 1], F32)
            negm_sb = singles.tile([128, 2, 16], FP8)
            nc.gpsimd.dma_start(out=wk0_sb, in_=wk[:, 0:2, :])
            nc.gpsimd.dma_start(out=wq0_sb, in_=wq[:, 0:1, :])
            nc.gpsimd.dma_start(out=wv0_sb, in_=wv[:, 0:1, :])
            nc.gpsimd.dma_start(out=msk_sb, in_=_bcast(msk[:].rearrange("a b -> (a b)")))
            nc.gpsimd.dma_start(out=negm_sb, in_=negm[:])
            nc.gpsimd.dma_start(out=wkr_sb, in_=wk[:, 2:NP, :])
            nc.gpsimd.dma_start(out=wqr_sb, in_=wq[:, 1:NP, :])
            nc.gpsimd.dma_start(out=wvr_sb, in_=wv[:, 1:NP, :])

            def wk_ap(p):
                return wk0_sb[:, p, :] if p < 2 else wkr_sb[:, p - 2, :]

            def wq_ap(p):
                return wq0_sb[:, 0, :] if p < 1 else wqr_sb[:, p - 1, :]

            def wv_ap(p):
                return wv0_sb[:, 0, :] if p < 1 else wvr_sb[:, p - 1, :]

            if apply_qkvb:
                bqk_sb = singles.tile([128, 2 * NP], F32)
                nc.gpsimd.dma_start(out=bqk_sb, in_=bqk[:])
                bq_sb = bqk_sb[:, 0:NP]
                bk_sb = bqk_sb[:, NP:2 * NP]
                bv_bc = singles.tile([128, NP, 128], F32)
                nc.gpsimd.dma_start(out=bv_bc, in_=_bcast(bvt[:]))
            # bulk tensors go LAST on the gpsimd ring
            wo_sb = singles.tile([128, NP, D], BF16)
            nc.gpsimd.dma_start(out=wo_sb, in_=wo[:])
            xq_sb = singles.tile([128, 4, D], F32)
            nc.gpsimd.dma_start(out=xq_sb, in_=xq[:])
            if apply_gb:
                g_bc = singles.tile([128, D], F32)
                b_bc = singles.tile([128, D], F32)
                nc.gpsimd.dma_start(out=g_bc, in_=_bcast(gg[:]))
                nc.gpsimd.dma_start(out=b_bc, in_=_bcast(bb[:]))
            eps_sb = singles.tile([128, 1], F32)
            nc.vector.memset(eps_sb, LN_EPS)
            ones_sc = singles.tile([1, d], BF16)
            nc.vector.memset(ones_sc, SCALE)
            oT_sb = singles.tile([128, NP, SQ], BF16)

            # Touch DMA-loaded constants once on VectorE / GpSimd so later
            # consumers need no DMA waits.
            scr = singles.tile([128, 8], F32)
            touches = [msk_sb[:, 0:1]]
            if apply_qkvb:
                touches += [bqk_sb[:, 0:1], bv_bc[:, 0, 0:1]]
            for i, t in enumerate(touches):
                nc.vector.tensor_copy(out=scr[:, i:i + 1], in_=t)
            scr8 = singles.tile([128, 16], FP8)
            nc.gpsimd.tensor_copy(out=scr8, in_=negm_sb[:, 0, :])

            # ---- per-pair qkv emission pieces -------------------------
            built = {}

            def qkv_pieces(p):
                xT_t = xpool.tile([128, S], BF16, name="xT_t")
                xqT_t = qxpool.tile([128, SQ], BF16, name="xqT_t")
                kT_t = kpool.tile([128, S], BF16, name="kT_t")
                qT_t = qpool.tile([128, 2, SQ], BF16, name="qT_t")
                v_t = vpool.tile([128, TCK, 2, 80], FP8, name="v_t")
                built[p] = (kT_t, qT_t, v_t)
                head = []

                def dma_piece():
                    # xqT on the scalar ring (parallel to sync); xT split in
                    # 4 so k_piece(c) waits only on its own 512-col chunk
                    nc.scalar.dma_start(out=xqT_t, in_=xqT[ds(128 * p, 128), :])
                    for c in range(4):
                        nc.sync.dma_start(out=xT_t[:, ts(c, 512)],
                                          in_=xT[ds(128 * p, 128), ts(c, 512)])
                head.append(dma_piece)

                def k_piece(c):
                    def f():
                        ps = psD.tile([128, 512], F32, tag="qkv", name="ps")
                        nc.tensor.matmul(ps, lhsT=wk_ap(p),
                                         rhs=xT_t[:, ts(c, 512)],
                                         start=True, stop=True)
                        if apply_qkvb:
                            nc.vector.tensor_scalar(
                                out=kT_t[:, ts(c, 512)], in0=ps,
                                scalar1=bk_sb[:, p:p + 1],
                                scalar2=None, op0=OP.add)
                        else:
                            nc.vector.tensor_copy(out=kT_t[:, ts(c, 512)],
                                                  in_=ps)
                    return f
                for c in range(4):
                    head.append(k_piece(c))

                def q_piece():
                    ps = psD.tile([128, 512], F32, tag="qkv", name="ps")
                    nc.tensor.matmul(ps, lhsT=wq_ap(p), rhs=xqT_t,
                                     start=True, stop=True)
                    if apply_qkvb:
                        nc.vector.tensor_scalar(out=qT_t[:, 0, :], in0=ps,
                                                scalar1=bq_sb[:, p:p + 1],
                                                scalar2=None, op0=OP.add)
                    else:
                        nc.vector.tensor_copy(out=qT_t[:, 0, :], in_=ps)
                    # masked-query variant for key chunks >= 8: col 511
                    # scaled by msk (0 on the masked core -> score 0)
                    if apply_qkvb:
                        nc.scalar.add(out=qT_t[:, 1, :], in_=ps,
                                      add=bq_sb[:, p:p + 1])
                    else:
                        nc.scalar.copy(out=qT_t[:, 1, :], in_=ps)
                    nc.gpsimd.tensor_scalar(out=qT_t[:, 1, 511:512],
                                            in0=qT_t[:, 1, 511:512],
                                            scalar1=msk_sb[:, 0:1],
                                            scalar2=None, op0=OP.mult)
                head.append(q_piece)

                def ones_piece():
                    nc.gpsimd.memset(v_t[:, :, :, 64:65], 1.0)
                head.append(ones_piece)

                tail = []

                def v_piece(tc0):
                    def f():
                        ps = psD.tile([128, 512], F32, tag="qkv", name="ps")
                        for j in range(4):
                            nc.tensor.matmul(ps[:, ts(j, 128)],
                                             lhsT=xT_t[:, ds(128 * (tc0 + j), 128)],
                                             rhs=wv_ap(p),
                                             start=True, stop=True)
                        if apply_qkvb:
                            for j in range(4):
                                nc.vector.tensor_tensor(
                                    out=v_t[:, tc0 + j, :, 0:64],
                                    in0=ps[:, ts(j, 128)].rearrange(
                                        "a (h e) -> a h e", h=2),
                                    in1=bv_bc[:, p, :].rearrange(
                                        "a (h e) -> a h e", h=2),
                                    op=OP.add)
                        else:
                            nc.vector.tensor_copy(
                                out=v_t[:, tc0:tc0 + 4, :, 0:64],
                                in_=ps.rearrange("a (c h e) -> a c h e",
                                                 c=4, h=2))
                    return f
                for tc0 in range(0, TCK, 4):
                    tail.append(v_piece(tc0))
                return head, tail

            # ---- normalization tail: scale oT[:, pp, :] by 1/(SCALE*den)
            dens = {}

            def emit_norm_tail(pp, bc):
                den = dens.pop(pp)
                nc.tensor.matmul(bc[0:64, :], lhsT=ones_sc[0:1, :],
                                 rhs=den[0:1, 0, :], start=True, stop=True)
                nc.tensor.matmul(bc[64:128, :], lhsT=ones_sc[0:1, :],
                                 rhs=den[0:1, 1, :], start=True, stop=True)
                scale_t = rpool.tile([128, 512], F32, tag="rs", name="scale_t")
                nc.vector.reciprocal_approx_fast(out=scale_t, in_=bc)
                nc.gpsimd.tensor_tensor(out=oT_sb[:, pp, :],
                                        in0=oT_sb[:, pp, :], in1=scale_t,
                                        op=OP.mult)

            # psD pre-accumulated projection groups for m=3 (built during
            # the last pair's attention stream; pairs 0..6 only)
            prd = {}

            def prd_pieces():
                pieces = [None] * 6  # chunks 0..5: wait for norm_tail(6)
                t30 = psD.tile([128, 512], F32, tag="qkv", name="t30")
                t31 = psD.tile([128, 512], F32, tag="qkv", name="t31")
                prd[(3, 0)] = t30
                prd[(3, 1)] = t31

                def acc_piece(p7):
                    def f():
                        for fc0, t in ((0, t30), (1, t31)):
                            nc.tensor.matmul(t, lhsT=oT_sb[:, p7, ts(3, 128)],
                                             rhs=wo_sb[:, p7, ts(fc0, 512)],
                                             start=(p7 == 0), stop=False)
                    return f
                for p7 in range(NP - 1):
                    pieces.append(acc_piece(p7))
                return [], pieces

            # ---- attention: score chunks -> exp (Scalar/DVE split) ->
            # DoubleRow PV per chunk-pair, lagged
            pvq = []

            def pop_pv():
                fn = pvq.pop(0)
                fn()

            head0, tail0 = qkv_pieces(0)
            for piece in head0:
                piece()

            for p in range(NP):
                if p == 0:
                    head, tail = qkv_pieces(1)
                    pieces = tail0 + head + tail
                elif p + 1 < NP:
                    head, tail = qkv_pieces(p + 1)
                    pieces = head + tail
                else:
                    head, tail = prd_pieces()
                    pieces = tail
                kT_t, qT_t, v_t = built.pop(p)
                oA = psB.tile([65, 512], F32, tag="ov", name="oA")
                oB = psB.tile([65, 512], F32, tag="ov", name="oB")

                def mk_pv(oA, oB, v_t, p, cp, ex2):
                    def f():
                        for h, o in ((0, oA), (1, oB)):
                            nc.tensor.matmul(
                                o[:, :],
                                lhsT=v_t[:, 2 * cp:2 * cp + 2, h, 0:65],
                                rhs=ex2[:, h, :, :],
                                start=(cp == 0), stop=False,
                                perf_mode=DR)
                        if cp >= NCP // 2:
                            # subtract the spurious exp=1 contribution of
                            # the zeroed masked-query column (all-zero
                            # rhs on unmasked cores)
                            for h, o in ((0, oA), (1, oB)):
                                nc.tensor.matmul(
                                    o[:, 511:512],
                                    lhsT=v_t[:, 2 * cp:2 * cp + 2, h, 0:65],
                                    rhs=negm_sb[:, :, 0:1],
                                    start=False, stop=(cp == NCP - 1),
                                    perf_mode=DR)
                        if cp == NCP - 1:
                            # Drain oA/oB; denominators from row 64.
                            nc.vector.tensor_copy(out=oT_sb[0:64, p, :],
                                                  in_=oA[0:64, :])
                            nc.vector.tensor_copy(out=oT_sb[64:128, p, :],
                                                  in_=oB[0:64, :])
                            den = rpool.tile([1, 2, 512], BF16, tag="den",
                                             name="den")
                            deng = nc.scalar if p == NP - 1 else nc.vector
                            if p == NP - 1:
                                deng.copy(out=den[0:1, 0, :], in_=oA[64:65, :])
                                deng.copy(out=den[0:1, 1, :], in_=oB[64:65, :])
                            else:
                                deng.tensor_copy(out=den[0:1, 0, :],
                                                 in_=oA[64:65, :])
                                deng.tensor_copy(out=den[0:1, 1, :],
                                                 in_=oB[64:65, :])
                            dens[p] = den
                    return f

                ex2 = None
                for c in range(TCK):
                    cp, ci = divmod(c, 2)
                    if ci == 0:
                        ex2 = epool.tile([128, 2, 2, 512], FP8, name="ex2")
                    sc = psA.tile([128, 2, 512], F32, tag="sc", name="sc")
                    qv = 1 if c >= TCK // 2 else 0
                    with tc.high_priority():
                        nc.tensor.matmul(sc[:, 0, :],
                                         lhsT=kT_t[0:64, ds(128 * c, 128)],
                                         rhs=qT_t[0:64, qv, :],
                                         start=True, stop=True)
                        nc.tensor.matmul(sc[:, 1, :],
                                         lhsT=kT_t[64:128, ds(128 * c, 128)],
                                         rhs=qT_t[64:128, qv, :],
                                         start=True, stop=True)
                    if c in DVE_CHUNKS:
                        with tc.high_priority():
                            nc.vector.tensor_scalar(
                                out=ex2[:, :, ci, :].bitcast(I8), in0=sc,
                                scalar1=EXPA, scalar2=EXPB,
                                op0=OP.mult, op1=OP.add)
                    else:
                        with tc.high_priority():
                            nc.scalar.activation(out=ex2[:, :, ci, :], in_=sc,
                                                 func=AF.Exp)
                    if ci == 1:
                        pvq.append(mk_pv(oA, oB, v_t, p, cp, ex2))
                    if len(pvq) >= 2 and (ci != 1 or cp != NCP - 1
                                          or p == NP - 1):
                        # defer the last chunk-pair's pop across the pair
                        # seam (except the final pair, whose drain gates
                        # the tail)
                        pop_pv()
                    if c == 6 and p > 0:
                        bc = psD.tile([128, 512], F32, tag="qkv", name="bc")
                        emit_norm_tail(p - 1, bc)
                    npiece = 2 if (p == 0 and c < 8) else 1
                    for _ in range(npiece):
                        if pieces:
                            piece = pieces.pop(0)
                            if piece:
                                piece()

            while pvq:
                pop_pv()

            # sqrt table preload: fills ScalarE's idle window right after
            # the last exp so the LN sqrts don't pay the table switch
            dum = stpool.tile([128, 1], F32, tag="dum", name="dum")
            nc.scalar.activation(out=dum, in_=eps_sb, func=AF.Sqrt)

            # last pair's normalization first (bc takes the psA slot freed
            # at the last exp)
            prs = {}
            bc7 = psA.tile([128, 2, 512], F32, tag="sc", name="bc7")
            with tc.high_priority():
                emit_norm_tail(NP - 1, bc7[:, 0, :])

            # (0,*) groups: pre-accumulate pairs 0..6 in the other psA slot
            pr2a = psA.tile([128, 2, 512], F32, tag="sc", name="pr2a")
            for gi in range(2):
                for p7 in range(NP - 1):
                    nc.tensor.matmul(pr2a[:, gi, :],
                                     lhsT=oT_sb[:, p7, ts(0, 128)],
                                     rhs=wo_sb[:, p7, ts(gi, 512)],
                                     start=(p7 == 0), stop=False)
                prs[(0, gi)] = pr2a[:, gi, :]

            # (1,*) groups: pre-accumulate in the psB slots freed by the
            # pair-7 drains
            for fc0 in range(2):
                prb = psB.tile([128, 512], F32, tag="ov", name="prb")
                for p7 in range(NP - 1):
                    nc.tensor.matmul(prb, lhsT=oT_sb[:, p7, ts(1, 128)],
                                     rhs=wo_sb[:, p7, ts(fc0, 512)],
                                     start=(p7 == 0), stop=False)
                prs[(1, fc0)] = prb
            # (2,0): the unused half of the bc7 tile is a free psum bank
            for p7 in range(NP - 1):
                nc.tensor.matmul(bc7[:, 1, :], lhsT=oT_sb[:, p7, ts(2, 128)],
                                 rhs=wo_sb[:, p7, ts(0, 512)],
                                 start=(p7 == 0), stop=False)
            prs[(2, 0)] = bc7[:, 1, :]
            # (2,1): the broadcast half of bc7 frees once the reciprocal
            # has read it; start=True reclaims the bank
            for p7 in range(NP - 1):
                nc.tensor.matmul(bc7[:, 0, :], lhsT=oT_sb[:, p7, ts(2, 128)],
                                 rhs=wo_sb[:, p7, ts(1, 512)],
                                 start=(p7 == 0), stop=False)
            prs[(2, 1)] = bc7[:, 0, :]
            prs.update(prd)
            prd.clear()

            if apply_gb:
                for i, t in enumerate([g_bc[:, 0:1], b_bc[:, 0:1]]):
                    nc.vector.tensor_copy(out=scr[:, 6 + i:7 + i], in_=t)

            # ---- finish projections + residual + fused-stats LayerNorm
            out_queues = [nc.sync, nc.scalar]
            ys = {}
            for m in range(4):
                y_t = ypool.tile([128, D], F32, tag="y", name="y_t")
                sums = stpool.tile([128, 3], F32, tag="sums", name="sums")
                for fc in range(2):
                    pr = prs.pop((m, fc))
                    with tc.high_priority():
                        nc.tensor.matmul(pr,
                                         lhsT=oT_sb[:, NP - 1, ts(m, 128)],
                                         rhs=wo_sb[:, NP - 1, ts(fc, 512)],
                                         start=False, stop=True)
                    nc.vector.scalar_tensor_tensor(
                        out=y_t[:, ts(fc, 512)], in0=pr, scalar=1.0,
                        in1=xq_sb[:, m, ts(fc, 512)],
                        op0=OP.mult, op1=OP.add,
                        accum_out=sums[:, fc:fc + 1])
                ysq = ypool.tile([128, D], BF16, tag="ysq", name="ysq")
                nc.scalar.activation(out=ysq, in_=y_t, func=AF.Square,
                                     accum_out=sums[:, 2:3])
                ys[m] = (y_t, sums)

            # phase 2: stats combine, normalize, store
            for m in range(4):
                y_t, sums = ys.pop(m)
                mv = stpool.tile([128, 2], F32, tag="mv", name="mv")
                nc.vector.scalar_tensor_tensor(
                    out=mv[:, 0:1], in0=sums[:, 0:1], scalar=1.0,
                    in1=sums[:, 1:2], op0=OP.mult, op1=OP.add)
                nc.vector.tensor_scalar(out=mv[:, 0:1], in0=mv[:, 0:1],
                                        scalar1=1.0 / D, scalar2=None,
                                        op0=OP.mult)
                nc.vector.tensor_tensor(out=mv[:, 1:2], in0=mv[:, 0:1],
                                        in1=mv[:, 0:1], op=OP.mult)
                var = stpool.tile([128, 1], F32, tag="var", name="var")
                nc.vector.scalar_tensor_tensor(
                    out=var, in0=sums[:, 2:3], scalar=1.0 / D,
                    in1=mv[:, 1:2], op0=OP.mult, op1=OP.subtract)
                sd = stpool.tile([128, 1], F32, tag="sd", name="sd")
                nc.scalar.activation(out=sd, in_=var, func=AF.Sqrt,
                                     bias=eps_sb[:, 0:1], scale=1.0)
                rstd = stpool.tile([128, 1], F32, tag="rsd", name="rstd")
                nc.vector.reciprocal(out=rstd, in_=sd)
                yn = ypool.tile([128, D], F32, tag="yn", name="yn")
                nc.vector.tensor_scalar(out=yn, in0=y_t, scalar1=mv[:, 0:1],
                                        scalar2=rstd, op0=OP.subtract,
                                        op1=OP.mult)
                if apply_gb:
                    ot = ypool.tile([128, D], F32, tag="ot", name="ot")
                    nc.vector.tensor_tensor(out=ot[:, 0:512], in0=yn[:, 0:512],
                                            in1=g_bc[:, 0:512], op=OP.mult)
                    nc.vector.tensor_tensor(out=ot[:, 512:1024],
                                            in0=yn[:, 512:1024],
                                            in1=g_bc[:, 512:1024], op=OP.mult)
                    nc.vector.tensor_tensor(out=ot[:, 0:512], in0=ot[:, 0:512],
                                            in1=b_bc[:, 0:512], op=OP.add)
                    nc.vector.tensor_tensor(out=ot[:, 512:1024],
                                            in0=ot[:, 512:1024],
                                            in1=b_bc[:, 512:1024], op=OP.add)
                    for fc in range(2):
                        out_queues[fc].dma_start(
                            out=out[ds(128 * m, 128), ts(fc, 512)],
                            in_=ot[:, ts(fc, 512)])
                else:
                    for fc in range(2):
                        out_queues[fc].dma_start(
                            out=out[ds(128 * m, 128), ts(fc, 512)],
                            in_=yn[:, ts(fc, 512)])
    nc.compile()
    return nc


def prep_inputs(x, Wq, bq, Wk, bk, Wv, bv, Wo, bo, ln_g, ln_b):
    """Host-side sharding/layout prep -> list of 8 per-core input maps."""
    bf = ml_dtypes.bfloat16
    x = np.asarray(x, np.float32)
    Wq, Wk, Wv = (np.asarray(w, np.float32) for w in (Wq, Wk, Wv))
    Wo = np.asarray(Wo, np.float32)
    bq, bk, bv, bo = (np.asarray(v_, np.float32) for v_ in (bq, bk, bv, bo))
    ln_g, ln_b = np.asarray(ln_g, np.float32), np.asarray(ln_b, np.float32)

    def pairs(W):  # [H,d,d] -> [128,NP,128]: block-diag per pair, part-major
        out = np.zeros((NP, 128, 128), np.float32)
        for p in range(NP):
            out[p, :d, :d] = W[2 * p]
            out[p, d:, d:] = W[2 * p + 1]
        return np.ascontiguousarray(out.transpose(1, 0, 2)).astype(bf)

    wq_b, wk_b, wv_b = pairs(Wq), pairs(Wk), pairs(Wv)
    bqk = np.concatenate([bq.reshape(NP, 128).T, bk.reshape(NP, 128).T],
                         1).copy()             # [128, 2*NP]
    bvt = bv.reshape(NP, 128).copy()            # [NP, 128]
    wo_b = np.ascontiguousarray(
        Wo.reshape(NP, 128, D).transpose(1, 0, 2)).astype(bf)  # [128,NP,D]
    xT_all = [np.ascontiguousarray(x[b_].T).astype(bf) for b_ in range(B)]

    e4 = ml_dtypes.float8_e4m3fn
    in_maps = []
    for c in range(N_CORES):
        b_, j = divmod(c, 4)
        rows = slice(j * SQ, (j + 1) * SQ)
        xq_pre = np.ascontiguousarray(
            (x[b_, rows] + bo).reshape(4, 128, D).transpose(1, 0, 2)
        ).astype(np.float32)                    # [128, 4, D]
        masked = (j == 3)
        negm = np.zeros((128, 2, 16), e4)
        if masked:
            negm[:, :, 0] = -1.0
        in_maps.append({
            "xT": xT_all[b_],
            "xqT": np.ascontiguousarray(xT_all[b_][:, rows]),
            "xq": xq_pre,
            "wq": wq_b, "wk": wk_b, "wv": wv_b,
            "bqk": bqk, "bvt": bvt,
            "wo": wo_b,
            "gg": ln_g, "bb": ln_b,
            "msk": np.array([[0.0 if masked else 1.0]], np.float32),
            "negm": negm.view(np.uint8),
        })
    return in_maps


_NC = {}


def _get_nc(apply_gb, apply_qkvb):
    key = (apply_gb, apply_qkvb)
    if key not in _NC:
        _NC[key] = build_nc(apply_gb=apply_gb, apply_qkvb=apply_qkvb)
    return _NC[key]


def _gather(results):
    y = np.empty((B, S, D), np.float32)
    for c, r in enumerate(results):
        b_, j = divmod(c, 4)
        y[b_, j * SQ:(j + 1) * SQ] = r["out"]
    return y


def _needs_gb(ln_g, ln_b):
    return not (np.all(np.asarray(ln_g) == 1.0)
                and np.all(np.asarray(ln_b) == 0.0))


def _needs_qkvb(bq, bk, bv):
    return not all(np.all(np.asarray(b) == 0.0) for b in (bq, bk, bv))


def kernel(**inputs):
    apply_gb = _needs_gb(inputs["ln_g"], inputs["ln_b"])
    apply_qkvb = _needs_qkvb(inputs["bq"], inputs["bk"], inputs["bv"])
    nc = _get_nc(apply_gb, apply_qkvb)
    in_maps = prep_inputs(**inputs)
    res = run_bass_kernel_spmd(nc, in_maps, core_ids=list(range(N_CORES)))
    return _gather(res.results)


def kernel_timed(**inputs):
    """Returns (output, exec_time_ns or None). Used by test.py."""
    apply_gb = _needs_gb(inputs["ln_g"], inputs["ln_b"])
    apply_qkvb = _needs_qkvb(inputs["bq"], inputs["bk"], inputs["bv"])
    nc = _get_nc(apply_gb, apply_qkvb)
    in_maps = prep_inputs(**inputs)
    res = run_bass_kernel_spmd(nc, in_maps, core_ids=list(range(N_CORES)),
                               trace=True)
    return _gather(res.results), res.exec_time_ns


# revision 24
# speedup vs baseline: 1.0064x; 1.0064x over previous
"""Trainium2 Bass kernel for a fused multi-head attention layer.

Math (per batch b):
    xh = x.reshape(S, H, d); q/k/v = xh @ W{q,k,v}[h] + b
    scores = q @ k^T  (per head);  scores[-1, -1024:] = -inf
    attn = softmax(scores, -1) / sqrt(D)
    o = concat_h(attn @ v);  proj = o @ Wo + bo
    out = LayerNorm(x + proj) * g + beta

Sharding: 8 cores = 2 batches x 4 query-blocks of 512 rows. Each core
computes K/V for its full batch (duplicated across the 4 cores of a
batch) and Q/attention/projection/LN for its own 512 query rows. No
collectives.

v2 design notes (vs the all-bf16 v1):
  * The exp stream is split between ScalarE (activation Exp -> fp8e4)
    and the DVE (Schraudolph fast-exp: round(a*s + 56) to int8 IS the
    fp8e4 bit pattern of exp(s); verified round-to-nearest+saturate on
    HW).  Both engines also share the PSUM->SBUF cast pool.
  * V and the attention weights are fp8e4; the PV matmuls run in
    DoubleRow perf mode contracting two 128-key chunks at once
    (lhsT [128,2,65] incl the ones-column, rhs [128,2,256]); the
    ones-column still yields the softmax denominator for free.
  * The seq-mask costs no per-chunk work: score chunks >= 8 use a
    second qT whose column 511 is zeroed on the masked core (GpSimd),
    making the masked scores 0 -> exp = 1 exactly; per-pair DoubleRow
    fixup matmuls with rhs = -mask subtract the spurious sum_v/count
    from the PV output and denominator.
  * v-casts are batched 4 chunks per DVE op; oT normalization mult
    runs on GpSimd.
"""

import numpy as np
import ml_dtypes

import concourse.bass as bass
import concourse.mybir as mybir
import concourse.tile as tile
from concourse import bacc
from concourse.bass import ds, ts
from concourse.bass_utils import run_bass_kernel_spmd

BF16 = mybir.dt.bfloat16
F32 = mybir.dt.float32
FP8 = mybir.dt.float8e4
I8 = mybir.dt.int8
AF = mybir.ActivationFunctionType
OP = mybir.AluOpType
DR = mybir.MatmulPerfMode.DoubleRow

B, S, D, H = 2, 2048, 1024, 16
d = 64            # head dim
NP = H // 2       # 8 head pairs
SQ = S // 4       # 512 query rows per core
TCK = S // 128    # 16 key chunks of 128
NCP = TCK // 2    # 8 chunk-pairs
SEQ_LEN = 1024
SCALE = float(np.sqrt(D))
LN_EPS = 1e-5
N_CORES = 8
EXPA = 8.0 / float(np.log(2.0))   # Schraudolph slope for e4m3 bits
EXPB = 56.0                        # 8 * bias(7)
# chunks handled by the DVE fast-exp (rest on ScalarE)
DVE_CHUNKS = (3, 7, 11, 14)


def _bcast(ap, p=128):
    """AP replicating `ap` across p partitions (partition step 0)."""
    return bass.AP(tensor=ap.tensor, offset=ap.offset, ap=[[0, p]] + list(ap.ap))


def build_nc(apply_gb=True, apply_qkvb=True):
    nc = bacc.Bacc("TRN2")

    xT = nc.dram_tensor("xT", [D, S], BF16, kind="ExternalInput")       # x[b].T
    xqT = nc.dram_tensor("xqT", [D, SQ], BF16, kind="ExternalInput")    # x[b,rows].T
    xq = nc.dram_tensor("xq", [128, 4, D], F32, kind="ExternalInput")   # x[b,rows]+bo
    # combined weight loads: one DMA descriptor each (descriptor gen on the
    # gpsimd ring is ~640ns apiece and serializes startup)
    wfirst = nc.dram_tensor("wfirst", [128, 4, 128], BF16, kind="ExternalInput")
    wrest = nc.dram_tensor("wrest", [128, 20, 128], BF16, kind="ExternalInput")
    bqk = nc.dram_tensor("bqk", [128, 2 * NP], F32, kind="ExternalInput")
    bvt = nc.dram_tensor("bvt", [NP, 128], F32, kind="ExternalInput")
    wo8 = nc.dram_tensor("wo8", [128, NP, D], FP8, kind="ExternalInput")
    gg = nc.dram_tensor("gg", [D], F32, kind="ExternalInput")
    bb = nc.dram_tensor("bb", [D], F32, kind="ExternalInput")
    msk = nc.dram_tensor("msk", [1, 1], F32, kind="ExternalInput")      # 0 if masked
    negm = nc.dram_tensor("negm", [128, 2, 16], FP8, kind="ExternalInput")
    out = nc.dram_tensor("out", [SQ, D], F32, kind="ExternalOutput")

    with tile.TileContext(nc) as tc:
        with (
            tc.tile_pool(name="singles", bufs=1) as singles,
            tc.tile_pool(name="xpool", bufs=2) as xpool,
            tc.tile_pool(name="kpool", bufs=2) as kpool,
            tc.tile_pool(name="qpool", bufs=2) as qpool,
            tc.tile_pool(name="qxpool", bufs=2) as qxpool,
            tc.tile_pool(name="vpool", bufs=2) as vpool,
            tc.tile_pool(name="epool", bufs=4) as epool,
            tc.tile_pool(name="rpool", bufs=2) as rpool,
            tc.tile_pool(name="orpool", bufs=2) as orpool,
            tc.tile_pool(name="ypool", bufs=4) as ypool,
            tc.tile_pool(name="stpool", bufs=4) as stpool,
            tc.tile_pool(name="psA", bufs=2, space="PSUM") as psA,
            tc.tile_pool(name="psB", bufs=2, space="PSUM") as psB,
            tc.tile_pool(name="psD", bufs=2, space="PSUM") as psD,
        ):
            # ---- warm-up: bridge the PE HAM clock gate until real MMs
            wu = singles.tile([128, 512], BF16)
            nc.vector.memset(wu, 0.0)
            for _ in range(8):
                wps = psD.tile([128, 512], F32, tag="qkv", name="wps")
                nc.tensor.matmul(wps, lhsT=wu[:, 0:128], rhs=wu,
                                 start=True, stop=True)

            # ---- constants / weights (contiguous host-prearranged DMAs).
            # wfirst = [wk p0, wk p1, wq p0, wv p0]; wrest = wk p2..7 +
            # wq p1..7 + wv p1..7 (one descriptor each on the gpsimd ring)
            wf_sb = singles.tile([128, 4, 128], BF16)
            wr_sb = singles.tile([128, 20, 128], BF16)
            msk_sb = singles.tile([128, 1], F32)
            negm_sb = singles.tile([128, 2, 16], FP8)
            nc.gpsimd.dma_start(out=wf_sb, in_=wfirst[:])
            nc.scalar.dma_start(out=msk_sb, in_=_bcast(msk[:].rearrange("a b -> (a b)")))
            nc.scalar.dma_start(out=negm_sb, in_=negm[:])
            nc.gpsimd.dma_start(out=wr_sb, in_=wrest[:])

            def wk_ap(p):
                return wf_sb[:, p, :] if p < 2 else wr_sb[:, p - 2, :]

            def wq_ap(p):
                return wf_sb[:, 2, :] if p < 1 else wr_sb[:, 5 + p, :]

            def wv_ap(p):
                return wf_sb[:, 3, :] if p < 1 else wr_sb[:, 12 + p, :]

            if apply_qkvb:
                bqk_sb = singles.tile([128, 2 * NP], F32)
                nc.gpsimd.dma_start(out=bqk_sb, in_=bqk[:])
                bq_sb = bqk_sb[:, 0:NP]
                bk_sb = bqk_sb[:, NP:2 * NP]
                bv_bc = singles.tile([128, NP, 128], F32)
                nc.gpsimd.dma_start(out=bv_bc, in_=_bcast(bvt[:]))
            # bulk tensors go LAST on the gpsimd ring
            wo8_sb = singles.tile([128, NP, D], FP8)
            nc.gpsimd.dma_start(out=wo8_sb, in_=wo8[:])
            xq_sb = singles.tile([128, 4, D], F32)
            nc.gpsimd.dma_start(out=xq_sb, in_=xq[:])
            if apply_gb:
                g_bc = singles.tile([128, D], F32)
                b_bc = singles.tile([128, D], F32)
                nc.gpsimd.dma_start(out=g_bc, in_=_bcast(gg[:]))
                nc.gpsimd.dma_start(out=b_bc, in_=_bcast(bb[:]))
            eps_sb = singles.tile([128, 1], F32)
            nc.vector.memset(eps_sb, LN_EPS)
            # oT is stored fp8 scaled by 64 (wo is prescaled by 8 on host;
            # the residual add divides by 512): bcast = (SCALE/64)*den, so
            # 1/bcast = 64/(SCALE*den)
            ones_sc = singles.tile([1, d], BF16)
            nc.vector.memset(ones_sc, SCALE / 64.0)
            oT8_sb = singles.tile([128, NP, SQ], FP8)

            # Touch DMA-loaded constants once on VectorE / GpSimd so later
            # consumers need no DMA waits.
            scr = singles.tile([128, 8], F32)
            touches = [msk_sb[:, 0:1]]
            if apply_qkvb:
                touches += [bqk_sb[:, 0:1], bv_bc[:, 0, 0:1]]
            for i, t in enumerate(touches):
                nc.vector.tensor_copy(out=scr[:, i:i + 1], in_=t)
            scr8 = singles.tile([128, 16], FP8)
            nc.gpsimd.tensor_copy(out=scr8, in_=negm_sb[:, 0, :])

            # ---- per-pair qkv emission pieces -------------------------
            built = {}

            def qkv_pieces(p):
                xT_t = xpool.tile([128, S], BF16, name="xT_t")
                xqT_t = qxpool.tile([128, SQ], BF16, name="xqT_t")
                kT_t = kpool.tile([128, S], BF16, name="kT_t")
                qT_t = qpool.tile([128, 2, SQ], BF16, name="qT_t")
                v_t = vpool.tile([128, TCK, 2, 80], FP8, name="v_t")
                built[p] = (kT_t, qT_t, v_t)
                head = []

                def dma_piece():
                    # xqT on the scalar ring (parallel to sync); xT split in
                    # 4 so k_piece(c) waits only on its own 512-col chunk
                    nc.scalar.dma_start(out=xqT_t, in_=xqT[ds(128 * p, 128), :])
                    for c in range(4):
                        nc.sync.dma_start(out=xT_t[:, ts(c, 512)],
                                          in_=xT[ds(128 * p, 128), ts(c, 512)])
                head.append(dma_piece)

                def k_piece(c):
                    def f():
                        ps = psD.tile([128, 512], F32, tag="qkv", name="ps")
                        nc.tensor.matmul(ps, lhsT=wk_ap(p),
                                         rhs=xT_t[:, ts(c, 512)],
                                         start=True, stop=True)
                        if apply_qkvb:
                            nc.vector.tensor_scalar(
                                out=kT_t[:, ts(c, 512)], in0=ps,
                                scalar1=bk_sb[:, p:p + 1],
                                scalar2=None, op0=OP.add)
                        else:
                            nc.vector.tensor_copy(out=kT_t[:, ts(c, 512)],
                                                  in_=ps)
                    return f
                for c in range(4):
                    head.append(k_piece(c))

                def q_piece():
                    ps = psD.tile([128, 512], F32, tag="qkv", name="ps")
                    nc.tensor.matmul(ps, lhsT=wq_ap(p), rhs=xqT_t,
                                     start=True, stop=True)
                    if apply_qkvb:
                        nc.vector.tensor_scalar(out=qT_t[:, 0, :], in0=ps,
                                                scalar1=bq_sb[:, p:p + 1],
                                                scalar2=None, op0=OP.add)
                    else:
                        nc.vector.tensor_copy(out=qT_t[:, 0, :], in_=ps)
                    # masked-query variant for key chunks >= 8: col 511
                    # scaled by msk (0 on the masked core -> score 0)
                    if apply_qkvb:
                        nc.scalar.add(out=qT_t[:, 1, :], in_=ps,
                                      add=bq_sb[:, p:p + 1])
                    else:
                        nc.scalar.copy(out=qT_t[:, 1, :], in_=ps)
                    nc.gpsimd.tensor_scalar(out=qT_t[:, 1, 511:512],
                                            in0=qT_t[:, 1, 511:512],
                                            scalar1=msk_sb[:, 0:1],
                                            scalar2=None, op0=OP.mult)
                head.append(q_piece)

                def ones_piece():
                    nc.gpsimd.memset(v_t[:, :, :, 64:65], 1.0)
                head.append(ones_piece)

                tail = []

                def v_piece(tc0):
                    def f():
                        ps = psD.tile([128, 512], F32, tag="qkv", name="ps")
                        for j in range(4):
                            nc.tensor.matmul(ps[:, ts(j, 128)],
                                             lhsT=xT_t[:, ds(128 * (tc0 + j), 128)],
                                             rhs=wv_ap(p),
                                             start=True, stop=True)
                        if apply_qkvb:
                            for j in range(4):
                                nc.vector.tensor_tensor(
                                    out=v_t[:, tc0 + j, :, 0:64],
                                    in0=ps[:, ts(j, 128)].rearrange(
                                        "a (h e) -> a h e", h=2),
                                    in1=bv_bc[:, p, :].rearrange(
                                        "a (h e) -> a h e", h=2),
                                    op=OP.add)
                        else:
                            nc.vector.tensor_copy(
                                out=v_t[:, tc0:tc0 + 4, :, 0:64],
                                in_=ps.rearrange("a (c h e) -> a c h e",
                                                 c=4, h=2))
                    return f
                for tc0 in range(0, TCK, 4):
                    tail.append(v_piece(tc0))
                return head, tail

            # ---- normalization tail: oT8[:, pp, :] = oTr * 64/(SCALE*den)
            dens = {}

            def emit_norm_tail(pp, bc):
                den, oTr = dens.pop(pp)
                nc.tensor.matmul(bc[0:64, :], lhsT=ones_sc[0:1, :],
                                 rhs=den[0:1, 0, :], start=True, stop=True)
                nc.tensor.matmul(bc[64:128, :], lhsT=ones_sc[0:1, :],
                                 rhs=den[0:1, 1, :], start=True, stop=True)
                scale_t = rpool.tile([128, 512], F32, tag="rs", name="scale_t")
                nc.vector.reciprocal_approx_fast(out=scale_t, in_=bc)
                nc.gpsimd.tensor_tensor(out=oT8_sb[:, pp, :],
                                        in0=oTr, in1=scale_t,
                                        op=OP.mult)

            def proj_mm(t, pp, m, fc, start, stop):
                nc.tensor.matmul(t,
                                 lhsT=oT8_sb[:, 2 * pp:2 * pp + 2, ts(m, 128)],
                                 rhs=wo8_sb[:, 2 * pp:2 * pp + 2, ts(fc, 512)],
                                 start=start, stop=stop, perf_mode=DR)

            # psD pre-accumulated projection groups for m=3 (built during
            # the last pair's attention stream; pair-pairs 0..2 = pairs 0..5)
            prd = {}

            def prd_pieces():
                # t30/t31 allocation is deferred past chunk 6 so the
                # norm_tail(6) bc tile grabs a psD slot first (the t3x
                # slots are only released in the LN tail -> cycle)
                def acc_piece(pp):
                    def f():
                        if pp == 0:
                            prd[(3, 0)] = psD.tile([128, 512], F32,
                                                   tag="qkv", name="t30")
                            prd[(3, 1)] = psD.tile([128, 512], F32,
                                                   tag="qkv", name="t31")
                        for fc0 in range(2):
                            proj_mm(prd[(3, fc0)], pp, 3, fc0,
                                    start=(pp == 0), stop=False)
                    return f
                return [], [None] * 7 + [acc_piece(pp) for pp in range(3)]

            # ---- attention: score chunks -> exp (Scalar/DVE split) ->
            # DoubleRow PV per chunk-pair, lagged
            pvq = []

            def pop_pv():
                fn = pvq.pop(0)
                fn()

            head0, tail0 = qkv_pieces(0)
            for piece in head0:
                piece()

            for p in range(NP):
                if p == 0:
                    head, tail = qkv_pieces(1)
                    pieces = tail0 + head + tail
                elif p + 1 < NP:
                    head, tail = qkv_pieces(p + 1)
                    pieces = head + tail
                else:
                    head, tail = prd_pieces()
                    pieces = tail
                kT_t, qT_t, v_t = built.pop(p)
                oA = psB.tile([65, 512], F32, tag="ov", name="oA")
                oB = psB.tile([65, 512], F32, tag="ov", name="oB")

                def mk_pv(oA, oB, v_t, p, cp, ex2):
                    def f():
                        for h, o in ((0, oA), (1, oB)):
                            nc.tensor.matmul(
                                o[:, :],
                                lhsT=v_t[:, 2 * cp:2 * cp + 2, h, 0:65],
                                rhs=ex2[:, h, :, :],
                                start=(cp == 0), stop=False,
                                perf_mode=DR)
                        if cp >= NCP // 2:
                            # subtract the spurious exp=1 contribution of
                            # the zeroed masked-query column (all-zero
                            # rhs on unmasked cores)
                            for h, o in ((0, oA), (1, oB)):
                                nc.tensor.matmul(
                                    o[:, 511:512],
                                    lhsT=v_t[:, 2 * cp:2 * cp + 2, h, 0:65],
                                    rhs=negm_sb[:, :, 0:1],
                                    start=False, stop=(cp == NCP - 1),
                                    perf_mode=DR)
                        if cp == NCP - 1:
                            # Drain oA/oB (raw bf16); denominators from row 64.
                            oTr = orpool.tile([128, 512], BF16, tag="or",
                                              name="oTr")
                            nc.vector.tensor_copy(out=oTr[0:64, :],
                                                  in_=oA[0:64, :])
                            nc.vector.tensor_copy(out=oTr[64:128, :],
                                                  in_=oB[0:64, :])
                            den = rpool.tile([1, 2, 512], BF16, tag="den",
                                             name="den")
                            deng = nc.scalar if p == NP - 1 else nc.vector
                            if p == NP - 1:
                                deng.copy(out=den[0:1, 0, :], in_=oA[64:65, :])
                                deng.copy(out=den[0:1, 1, :], in_=oB[64:65, :])
                            else:
                                deng.tensor_copy(out=den[0:1, 0, :],
                                                 in_=oA[64:65, :])
                                deng.tensor_copy(out=den[0:1, 1, :],
                                                 in_=oB[64:65, :])
                            dens[p] = (den, oTr)
                    return f

                ex2 = None
                for c in range(TCK):
                    cp, ci = divmod(c, 2)
                    if ci == 0:
                        ex2 = epool.tile([128, 2, 2, 512], FP8, name="ex2")
                    sc = psA.tile([128, 2, 512], F32, tag="sc", name="sc")
                    qv = 1 if c >= TCK // 2 else 0
                    with tc.high_priority():
                        nc.tensor.matmul(sc[:, 0, :],
                                         lhsT=kT_t[0:64, ds(128 * c, 128)],
                                         rhs=qT_t[0:64, qv, :],
                                         start=True, stop=True)
                        nc.tensor.matmul(sc[:, 1, :],
                                         lhsT=kT_t[64:128, ds(128 * c, 128)],
                                         rhs=qT_t[64:128, qv, :],
                                         start=True, stop=True)
                    if c in DVE_CHUNKS:
                        with tc.high_priority():
                            nc.vector.tensor_scalar(
                                out=ex2[:, :, ci, :].bitcast(I8), in0=sc,
                                scalar1=EXPA, scalar2=EXPB,
                                op0=OP.mult, op1=OP.add)
                    else:
                        with tc.high_priority():
                            nc.scalar.activation(out=ex2[:, :, ci, :], in_=sc,
                                                 func=AF.Exp)
                    if ci == 1:
                        pvq.append(mk_pv(oA, oB, v_t, p, cp, ex2))
                    if len(pvq) >= 2 and (ci != 1 or cp != NCP - 1
                                          or p == NP - 1):
                        # defer the last chunk-pair's pop across the pair
                        # seam (except the final pair, whose drain gates
                        # the tail)
                        pop_pv()
                    if c == 6 and p > 0:
                        bc = psD.tile([128, 512], F32, tag="qkv", name="bc")
                        emit_norm_tail(p - 1, bc)
                    npiece = 2 if (p == 0 and c < 8) else 1
                    for _ in range(npiece):
                        if pieces:
                            piece = pieces.pop(0)
                            if piece:
                                piece()

            while pvq:
                pop_pv()

            # sqrt table preload: fills ScalarE's idle window right after
            # the last exp so the LN sqrts don't pay the table switch.
            # Reads the last sc tile so the scheduler cannot hoist it early
            # (which would evict the exp table set before the exps run).
            dum = stpool.tile([128, 1], F32, tag="dum", name="dum")
            nc.scalar.activation(out=dum, in_=sc[:, 0, 0:1], func=AF.Sqrt)

            # last pair's normalization first (bc takes the psA slot freed
            # at the last exp)
            prs = {}
            bc7 = psA.tile([128, 2, 512], F32, tag="sc", name="bc7")
            with tc.high_priority():
                emit_norm_tail(NP - 1, bc7[:, 0, :])

            # (0,*) groups: pre-accumulate pair-pairs 0..2 in the other psA
            # slot
            pr2a = psA.tile([128, 2, 512], F32, tag="sc", name="pr2a")
            for gi in range(2):
                for pp in range(3):
                    proj_mm(pr2a[:, gi, :], pp, 0, gi,
                            start=(pp == 0), stop=False)
                prs[(0, gi)] = pr2a[:, gi, :]

            # (1,*) groups: pre-accumulate in the psB slots freed by the
            # pair-7 drains
            for fc0 in range(2):
                prb = psB.tile([128, 512], F32, tag="ov", name="prb")
                for pp in range(3):
                    proj_mm(prb, pp, 1, fc0, start=(pp == 0), stop=False)
                prs[(1, fc0)] = prb
            # (2,0): the unused half of the bc7 tile is a free psum bank
            for pp in range(3):
                proj_mm(bc7[:, 1, :], pp, 2, 0, start=(pp == 0), stop=False)
            prs[(2, 0)] = bc7[:, 1, :]
            # (2,1): the broadcast half of bc7 frees once the reciprocal
            # has read it; start=True reclaims the bank
            for pp in range(3):
                proj_mm(bc7[:, 0, :], pp, 2, 1, start=(pp == 0), stop=False)
            prs[(2, 1)] = bc7[:, 0, :]
            prs.update(prd)
            prd.clear()

            if apply_gb:
                for i, t in enumerate([g_bc[:, 0:1], b_bc[:, 0:1]]):
                    nc.vector.tensor_copy(out=scr[:, 6 + i:7 + i], in_=t)

            # ---- finish projections + residual + fused-stats LayerNorm
            out_queues = [nc.sync, nc.scalar]
            ys = {}
            for m in range(4):
                y_t = ypool.tile([128, D], F32, tag="y", name="y_t")
                sums = stpool.tile([128, 3], F32, tag="sums", name="sums")
                for fc in range(2):
                    pr = prs.pop((m, fc))
                    with tc.high_priority():
                        proj_mm(pr, 3, m, fc, start=False, stop=True)
                    nc.vector.scalar_tensor_tensor(
                        out=y_t[:, ts(fc, 512)], in0=pr, scalar=1.0 / 512.0,
                        in1=xq_sb[:, m, ts(fc, 512)],
                        op0=OP.mult, op1=OP.add,
                        accum_out=sums[:, fc:fc + 1])
                ysq = ypool.tile([128, D], BF16, tag="ysq", name="ysq")
                nc.scalar.activation(out=ysq, in_=y_t, func=AF.Square,
                                     accum_out=sums[:, 2:3])
                ys[m] = (y_t, sums)

            # phase 2: stats combine, normalize, store
            for m in range(4):
                y_t, sums = ys.pop(m)
                mv = stpool.tile([128, 2], F32, tag="mv", name="mv")
                nc.vector.scalar_tensor_tensor(
                    out=mv[:, 0:1], in0=sums[:, 0:1], scalar=1.0,
                    in1=sums[:, 1:2], op0=OP.mult, op1=OP.add)
                nc.vector.tensor_scalar(out=mv[:, 0:1], in0=mv[:, 0:1],
                                        scalar1=1.0 / D, scalar2=None,
                                        op0=OP.mult)
                nc.vector.tensor_tensor(out=mv[:, 1:2], in0=mv[:, 0:1],
                                        in1=mv[:, 0:1], op=OP.mult)
                var = stpool.tile([128, 1], F32, tag="var", name="var")
                nc.vector.scalar_tensor_tensor(
                    out=var, in0=sums[:, 2:3], scalar=1.0 / D,
                    in1=mv[:, 1:2], op0=OP.mult, op1=OP.subtract)
                sd = stpool.tile([128, 1], F32, tag="sd", name="sd")
                nc.scalar.activation(out=sd, in_=var, func=AF.Sqrt,
                                     bias=eps_sb[:, 0:1], scale=1.0)
                rstd = stpool.tile([128, 1], F32, tag="rsd", name="rstd")
                nc.vector.reciprocal(out=rstd, in_=sd)
                yn = ypool.tile([128, D], F32, tag="yn", name="yn")
                nc.vector.tensor_scalar(out=yn, in0=y_t, scalar1=mv[:, 0:1],
                                        scalar2=rstd, op0=OP.subtract,
                                        op1=OP.mult)
                if apply_gb:
                    ot = ypool.tile([128, D], F32, tag="ot", name="ot")
                    nc.vector.tensor_tensor(out=ot[:, 0:512], in0=yn[:, 0:512],
                                            in1=g_bc[:, 0:512], op=OP.mult)
                    nc.vector.tensor_tensor(out=ot[:, 512:1024],
                                            in0=yn[:, 512:1024],
                                            in1=g_bc[:, 512:1024], op=OP.mult)
                    nc.vector.tensor_tensor(out=ot[:, 0:512], in0=ot[:, 0:512],
                                            in1=b_bc[:, 0:512], op=OP.add)
                    nc.vector.tensor_tensor(out=ot[:, 512:1024],
                                            in0=ot[:, 512:1024],
                                            in1=b_bc[:, 512:1024], op=OP.add)
                    for fc in range(2):
                        out_queues[fc].dma_start(
                            out=out[ds(128 * m, 128), ts(fc, 512)],
                            in_=ot[:, ts(fc, 512)])
                else:
                    for fc in range(2):
                        out_queues[fc].dma_start(
                            out=out[ds(128 * m, 128), ts(fc, 512)],
                            in_=yn[:, ts(fc, 512)])
    nc.compile()
    return nc


def prep_inputs(x, Wq, bq, Wk, bk, Wv, bv, Wo, bo, ln_g, ln_b):
    """Host-side sharding/layout prep -> list of 8 per-core input maps."""
    bf = ml_dtypes.bfloat16
    x = np.asarray(x, np.float32)
    Wq, Wk, Wv = (np.asarray(w, np.float32) for w in (Wq, Wk, Wv))
    Wo = np.asarray(Wo, np.float32)
    bq, bk, bv, bo = (np.asarray(v_, np.float32) for v_ in (bq, bk, bv, bo))
    ln_g, ln_b = np.asarray(ln_g, np.float32), np.asarray(ln_b, np.float32)

    def pairs(W):  # [H,d,d] -> [128,NP,128]: block-diag per pair, part-major
        out = np.zeros((NP, 128, 128), np.float32)
        for p in range(NP):
            out[p, :d, :d] = W[2 * p]
            out[p, d:, d:] = W[2 * p + 1]
        return np.ascontiguousarray(out.transpose(1, 0, 2)).astype(bf)

    wq_b, wk_b, wv_b = pairs(Wq), pairs(Wk), pairs(Wv)
    wfirst = np.ascontiguousarray(np.stack(
        [wk_b[:, 0], wk_b[:, 1], wq_b[:, 0], wv_b[:, 0]], axis=1))
    wrest = np.ascontiguousarray(np.concatenate(
        [wk_b[:, 2:NP], wq_b[:, 1:NP], wv_b[:, 1:NP]], axis=1))
    bqk = np.concatenate([bq.reshape(NP, 128).T, bk.reshape(NP, 128).T],
                         1).copy()             # [128, 2*NP]
    bvt = bv.reshape(NP, 128).copy()            # [NP, 128]
    e4 = ml_dtypes.float8_e4m3fn
    wo8_b = np.ascontiguousarray(
        (Wo * 8.0).reshape(NP, 128, D).transpose(1, 0, 2)).astype(e4)
    xT_all = [np.ascontiguousarray(x[b_].T).astype(bf) for b_ in range(B)]

    in_maps = []
    for c in range(N_CORES):
        b_, j = divmod(c, 4)
        rows = slice(j * SQ, (j + 1) * SQ)
        xq_pre = np.ascontiguousarray(
            (x[b_, rows] + bo).reshape(4, 128, D).transpose(1, 0, 2)
        ).astype(np.float32)                    # [128, 4, D]
        masked = (j == 3)
        negm = np.zeros((128, 2, 16), e4)
        if masked:
            negm[:, :, 0] = -1.0
        in_maps.append({
            "xT": xT_all[b_],
            "xqT": np.ascontiguousarray(xT_all[b_][:, rows]),
            "xq": xq_pre,
            "wfirst": wfirst, "wrest": wrest,
            "bqk": bqk, "bvt": bvt,
            "wo8": wo8_b.view(np.uint8),
            "gg": ln_g, "bb": ln_b,
            "msk": np.array([[0.0 if masked else 1.0]], np.float32),
            "negm": negm.view(np.uint8),
        })
    return in_maps


_NC = {}


def _get_nc(apply_gb, apply_qkvb):
    key = (apply_gb, apply_qkvb)
    if key not in _NC:
        _NC[key] = build_nc(apply_gb=apply_gb, apply_qkvb=apply_qkvb)
    return _NC[key]


def _gather(results):
    y = np.empty((B, S, D), np.float32)
    for c, r in enumerate(results):
        b_, j = divmod(c, 4)
        y[b_, j * SQ:(j + 1) * SQ] = r["out"]
    return y


def _needs_gb(ln_g, ln_b):
    return not (np.all(np.asarray(ln_g) == 1.0)
                and np.all(np.asarray(ln_b) == 0.0))


def _needs_qkvb(bq, bk, bv):
    return not all(np.all(np.asarray(b) == 0.0) for b in (bq, bk, bv))


def kernel(**inputs):
    apply_gb = _needs_gb(inputs["ln_g"], inputs["ln_b"])
    apply_qkvb = _needs_qkvb(inputs["bq"], inputs["bk"], inputs["bv"])
    nc = _get_nc(apply_gb, apply_qkvb)
    in_maps = prep_inputs(**inputs)
    res = run_bass_kernel_spmd(nc, in_maps, core_ids=list(range(N_CORES)))
    return _gather(res.results)


def kernel_timed(**inputs):
    """Returns (output, exec_time_ns or None). Used by test.py."""
    apply_gb = _needs_gb(inputs["ln_g"], inputs["ln_b"])
    apply_qkvb = _needs_qkvb(inputs["bq"], inputs["bk"], inputs["bv"])
    nc = _get_nc(apply_gb, apply_qkvb)
    in_maps = prep_inputs(**inputs)
    res = run_bass_kernel_spmd(nc, in_maps, core_ids=list(range(N_CORES)),
                               trace=True)
    return _gather(res.results), res.exec_time_ns


# revision 27
# speedup vs baseline: 1.0266x; 1.0201x over previous
"""Trainium2 Bass kernel for a fused multi-head attention layer.

Math (per batch b):
    xh = x.reshape(S, H, d); q/k/v = xh @ W{q,k,v}[h] + b
    scores = q @ k^T  (per head);  scores[-1, -1024:] = -inf
    attn = softmax(scores, -1) / sqrt(D)
    o = concat_h(attn @ v);  proj = o @ Wo + bo
    out = LayerNorm(x + proj) * g + beta

Sharding: 8 cores = 2 batches x 4 query-blocks of 512 rows. Each core
computes K/V for its full batch (duplicated across the 4 cores of a
batch) and Q/attention/projection/LN for its own 512 query rows. No
collectives.

v2 design notes (vs the all-bf16 v1):
  * The exp stream is split between ScalarE (activation Exp -> fp8e4)
    and the DVE (Schraudolph fast-exp: round(a*s + 56) to int8 IS the
    fp8e4 bit pattern of exp(s); verified round-to-nearest+saturate on
    HW).  Both engines also share the PSUM->SBUF cast pool.
  * V and the attention weights are fp8e4; the PV matmuls run in
    DoubleRow perf mode contracting two 128-key chunks at once
    (lhsT [128,2,65] incl the ones-column, rhs [128,2,256]); the
    ones-column still yields the softmax denominator for free.
  * The seq-mask costs no per-chunk work: score chunks >= 8 use a
    second qT whose column 511 is zeroed on the masked core (GpSimd),
    making the masked scores 0 -> exp = 1 exactly; per-pair DoubleRow
    fixup matmuls with rhs = -mask subtract the spurious sum_v/count
    from the PV output and denominator.
  * v-casts are batched 4 chunks per DVE op; oT normalization mult
    runs on GpSimd.
"""

import numpy as np
import ml_dtypes

import concourse.bass as bass
import concourse.mybir as mybir
import concourse.tile as tile
from concourse import bacc
from concourse.bass import ds, ts
from concourse.bass_utils import run_bass_kernel_spmd

BF16 = mybir.dt.bfloat16
F32 = mybir.dt.float32
FP8 = mybir.dt.float8e4
I8 = mybir.dt.int8
AF = mybir.ActivationFunctionType
OP = mybir.AluOpType
DR = mybir.MatmulPerfMode.DoubleRow

B, S, D, H = 2, 2048, 1024, 16
d = 64            # head dim
NP = H // 2       # 8 head pairs
SQ = S // 4       # 512 query rows per core
TCK = S // 128    # 16 key chunks of 128
NCP = TCK // 2    # 8 chunk-pairs
SEQ_LEN = 1024
SCALE = float(np.sqrt(D))
LN_EPS = 1e-5
N_CORES = 8
EXPA = 8.0 / float(np.log(2.0))   # Schraudolph slope for e4m3 bits
EXPB = 56.0                        # 8 * bias(7)
# chunks handled by the DVE fast-exp (rest on ScalarE)
DVE_CHUNKS = (5, 9, 13)


def _bcast(ap, p=128):
    """AP replicating `ap` across p partitions (partition step 0)."""
    return bass.AP(tensor=ap.tensor, offset=ap.offset, ap=[[0, p]] + list(ap.ap))


def build_nc(apply_gb=True, apply_qkvb=True):
    nc = bacc.Bacc("TRN2")

    xT = nc.dram_tensor("xT", [D, S], BF16, kind="ExternalInput")       # x[b].T
    xqT = nc.dram_tensor("xqT", [D, SQ], BF16, kind="ExternalInput")    # x[b,rows].T
    xq = nc.dram_tensor("xq", [128, 4, D], F32, kind="ExternalInput")   # x[b,rows]+bo
    # combined weight loads: one DMA descriptor each (descriptor gen on the
    # gpsimd ring is ~640ns apiece and serializes startup)
    wfirst = nc.dram_tensor("wfirst", [128, 4, 128], BF16, kind="ExternalInput")
    wrest = nc.dram_tensor("wrest", [128, 20, 128], BF16, kind="ExternalInput")
    bqk = nc.dram_tensor("bqk", [128, 2 * NP], F32, kind="ExternalInput")
    bvt = nc.dram_tensor("bvt", [NP, 128], F32, kind="ExternalInput")
    wo8 = nc.dram_tensor("wo8", [128, NP, D], FP8, kind="ExternalInput")
    gg = nc.dram_tensor("gg", [D], F32, kind="ExternalInput")
    bb = nc.dram_tensor("bb", [D], F32, kind="ExternalInput")
    msk = nc.dram_tensor("msk", [1, 1], F32, kind="ExternalInput")      # 0 if masked
    negm = nc.dram_tensor("negm", [128, 2, 16], FP8, kind="ExternalInput")
    out = nc.dram_tensor("out", [SQ, D], F32, kind="ExternalOutput")

    with tile.TileContext(nc) as tc:
        with (
            tc.tile_pool(name="singles", bufs=1) as singles,
            tc.tile_pool(name="xpool", bufs=2) as xpool,
            tc.tile_pool(name="kpool", bufs=2) as kpool,
            tc.tile_pool(name="qpool", bufs=2) as qpool,
            tc.tile_pool(name="qxpool", bufs=2) as qxpool,
            tc.tile_pool(name="vpool", bufs=2) as vpool,
            tc.tile_pool(name="epool", bufs=4) as epool,
            tc.tile_pool(name="rpool", bufs=2) as rpool,
            tc.tile_pool(name="orpool", bufs=2) as orpool,
            tc.tile_pool(name="ypool", bufs=4) as ypool,
            tc.tile_pool(name="stpool", bufs=4) as stpool,
            tc.tile_pool(name="psA", bufs=2, space="PSUM") as psA,
            tc.tile_pool(name="psB", bufs=2, space="PSUM") as psB,
            tc.tile_pool(name="psD", bufs=2, space="PSUM") as psD,
        ):
            # ---- warm-up: bridge the PE HAM clock gate until real MMs
            wu = singles.tile([128, 512], BF16)
            nc.vector.memset(wu, 0.0)
            for _ in range(8):
                wps = psD.tile([128, 512], F32, tag="qkv", name="wps")
                nc.tensor.matmul(wps, lhsT=wu[:, 0:128], rhs=wu,
                                 start=True, stop=True)

            # ---- constants / weights (contiguous host-prearranged DMAs).
            # wfirst = [wk p0, wk p1, wq p0, wv p0]; wrest = wk p2..7 +
            # wq p1..7 + wv p1..7 (one descriptor each on the gpsimd ring)
            wf_sb = singles.tile([128, 4, 128], BF16)
            wr_sb = singles.tile([128, 20, 128], BF16)
            msk_sb = singles.tile([128, 1], F32)
            negm_sb = singles.tile([128, 2, 16], FP8)
            nc.gpsimd.dma_start(out=wf_sb, in_=wfirst[:])
            nc.scalar.dma_start(out=msk_sb, in_=_bcast(msk[:].rearrange("a b -> (a b)")))
            nc.scalar.dma_start(out=negm_sb, in_=negm[:])
            nc.gpsimd.dma_start(out=wr_sb, in_=wrest[:])

            def wk_ap(p):
                return wf_sb[:, p, :] if p < 2 else wr_sb[:, p - 2, :]

            def wq_ap(p):
                return wf_sb[:, 2, :] if p < 1 else wr_sb[:, 5 + p, :]

            def wv_ap(p):
                return wf_sb[:, 3, :] if p < 1 else wr_sb[:, 12 + p, :]

            if apply_qkvb:
                bqk_sb = singles.tile([128, 2 * NP], F32)
                nc.gpsimd.dma_start(out=bqk_sb, in_=bqk[:])
                bq_sb = bqk_sb[:, 0:NP]
                bk_sb = bqk_sb[:, NP:2 * NP]
                bv_bc = singles.tile([128, NP, 128], F32)
                nc.gpsimd.dma_start(out=bv_bc, in_=_bcast(bvt[:]))
            # bulk tensors go LAST on the gpsimd ring
            wo8_sb = singles.tile([128, NP, D], FP8)
            nc.gpsimd.dma_start(out=wo8_sb, in_=wo8[:])
            xq_sb = singles.tile([128, 4, D], F32)
            nc.gpsimd.dma_start(out=xq_sb, in_=xq[:])
            if apply_gb:
                g_bc = singles.tile([128, D], F32)
                b_bc = singles.tile([128, D], F32)
                nc.gpsimd.dma_start(out=g_bc, in_=_bcast(gg[:]))
                nc.gpsimd.dma_start(out=b_bc, in_=_bcast(bb[:]))
            eps_sb = singles.tile([128, 1], F32)
            nc.vector.memset(eps_sb, LN_EPS)
            # oT is stored fp8 scaled by 64 (wo is prescaled by 8 on host;
            # the residual add divides by 512): bcast = (SCALE/64)*den, so
            # 1/bcast = 64/(SCALE*den)
            ones_sc = singles.tile([1, d], BF16)
            nc.vector.memset(ones_sc, SCALE / 64.0)
            oT8_sb = singles.tile([128, NP, SQ], FP8)

            # Touch DMA-loaded constants once on VectorE / GpSimd so later
            # consumers need no DMA waits.
            scr = singles.tile([128, 8], F32)
            touches = [msk_sb[:, 0:1]]
            if apply_qkvb:
                touches += [bqk_sb[:, 0:1], bv_bc[:, 0, 0:1]]
            for i, t in enumerate(touches):
                nc.vector.tensor_copy(out=scr[:, i:i + 1], in_=t)
            scr8 = singles.tile([128, 16], FP8)
            nc.gpsimd.tensor_copy(out=scr8, in_=negm_sb[:, 0, :])

            # ---- per-pair qkv emission pieces -------------------------
            built = {}

            def qkv_pieces(p):
                xT_t = xpool.tile([128, S], BF16, name="xT_t")
                xqT_t = qxpool.tile([128, SQ], BF16, name="xqT_t")
                kT_t = kpool.tile([128, S], BF16, name="kT_t")
                qT_t = qpool.tile([128, 2, SQ], BF16, name="qT_t")
                v_t = vpool.tile([128, TCK, 2, 80], FP8, name="v_t")
                built[p] = (kT_t, qT_t, v_t)
                head = []

                def dma_piece():
                    # xqT on the scalar ring (parallel to sync); xT split in
                    # 4 so k_piece(c) waits only on its own 512-col chunk
                    nc.scalar.dma_start(out=xqT_t, in_=xqT[ds(128 * p, 128), :])
                    for c in range(4):
                        nc.sync.dma_start(out=xT_t[:, ts(c, 512)],
                                          in_=xT[ds(128 * p, 128), ts(c, 512)])
                head.append(dma_piece)

                def k_piece(c):
                    def f():
                        ps = psD.tile([128, 512], F32, tag="qkv", name="ps")
                        nc.tensor.matmul(ps, lhsT=wk_ap(p),
                                         rhs=xT_t[:, ts(c, 512)],
                                         start=True, stop=True)
                        if apply_qkvb:
                            nc.vector.tensor_scalar(
                                out=kT_t[:, ts(c, 512)], in0=ps,
                                scalar1=bk_sb[:, p:p + 1],
                                scalar2=None, op0=OP.add)
                        else:
                            nc.vector.tensor_copy(out=kT_t[:, ts(c, 512)],
                                                  in_=ps)
                    return f
                for c in range(4):
                    head.append(k_piece(c))

                def q_piece():
                    ps = psD.tile([128, 512], F32, tag="qkv", name="ps")
                    nc.tensor.matmul(ps, lhsT=wq_ap(p), rhs=xqT_t,
                                     start=True, stop=True)
                    if apply_qkvb:
                        nc.vector.tensor_scalar(out=qT_t[:, 0, :], in0=ps,
                                                scalar1=bq_sb[:, p:p + 1],
                                                scalar2=None, op0=OP.add)
                    else:
                        nc.vector.tensor_copy(out=qT_t[:, 0, :], in_=ps)
                    # masked-query variant for key chunks >= 8: col 511
                    # scaled by msk (0 on the masked core -> score 0)
                    if apply_qkvb:
                        nc.vector.tensor_scalar(out=qT_t[:, 1, :], in0=ps,
                                                scalar1=bq_sb[:, p:p + 1],
                                                scalar2=None, op0=OP.add)
                    else:
                        nc.vector.tensor_copy(out=qT_t[:, 1, :], in_=ps)
                    nc.gpsimd.tensor_scalar(out=qT_t[:, 1, 511:512],
                                            in0=qT_t[:, 1, 511:512],
                                            scalar1=msk_sb[:, 0:1],
                                            scalar2=None, op0=OP.mult)
                head.append(q_piece)

                def ones_piece():
                    nc.gpsimd.memset(v_t[:, :, :, 64:65], 1.0)
                head.append(ones_piece)

                tail = []

                def v_piece(tc0):
                    def f():
                        ps = psD.tile([128, 512], F32, tag="qkv", name="ps")
                        for j in range(4):
                            nc.tensor.matmul(ps[:, ts(j, 128)],
                                             lhsT=xT_t[:, ds(128 * (tc0 + j), 128)],
                                             rhs=wv_ap(p),
                                             start=True, stop=True)
                        if apply_qkvb:
                            for j in range(4):
                                nc.vector.tensor_tensor(
                                    out=v_t[:, tc0 + j, :, 0:64],
                                    in0=ps[:, ts(j, 128)].rearrange(
                                        "a (h e) -> a h e", h=2),
                                    in1=bv_bc[:, p, :].rearrange(
                                        "a (h e) -> a h e", h=2),
                                    op=OP.add)
                        else:
                            nc.vector.tensor_copy(
                                out=v_t[:, tc0:tc0 + 4, :, 0:64],
                                in_=ps.rearrange("a (c h e) -> a c h e",
                                                 c=4, h=2))
                    return f
                for tc0 in range(0, TCK, 4):
                    tail.append(v_piece(tc0))
                return head, tail

            # ---- normalization tail: oT8[:, pp, :] = oTr * 64/(SCALE*den)
            dens = {}

            def emit_norm_tail(pp, bc):
                den, oTr = dens.pop(pp)
                nc.tensor.matmul(bc[0:64, :], lhsT=ones_sc[0:1, :],
                                 rhs=den[0:1, 0, :], start=True, stop=True)
                nc.tensor.matmul(bc[64:128, :], lhsT=ones_sc[0:1, :],
                                 rhs=den[0:1, 1, :], start=True, stop=True)
                scale_t = rpool.tile([128, 512], F32, tag="rs", name="scale_t")
                nc.vector.reciprocal_approx_fast(out=scale_t, in_=bc)
                nc.gpsimd.tensor_tensor(out=oT8_sb[:, pp, :],
                                        in0=oTr, in1=scale_t,
                                        op=OP.mult)

            def proj_mm(t, pp, m, fc, start, stop):
                nc.tensor.matmul(t,
                                 lhsT=oT8_sb[:, 2 * pp:2 * pp + 2, ts(m, 128)],
                                 rhs=wo8_sb[:, 2 * pp:2 * pp + 2, ts(fc, 512)],
                                 start=start, stop=stop, perf_mode=DR)

            # psD pre-accumulated projection groups for m=3 (built during
            # the last pair's attention stream; pair-pairs 0..2 = pairs 0..5)
            prd = {}

            def prd_pieces():
                # t30/t31 allocation is deferred past chunk 6 so the
                # norm_tail(6) bc tile grabs a psD slot first (the t3x
                # slots are only released in the LN tail -> cycle)
                def acc_piece(pp):
                    def f():
                        if pp == 0:
                            prd[(3, 0)] = psD.tile([128, 512], F32,
                                                   tag="qkv", name="t30")
                            prd[(3, 1)] = psD.tile([128, 512], F32,
                                                   tag="qkv", name="t31")
                        for fc0 in range(2):
                            proj_mm(prd[(3, fc0)], pp, 3, fc0,
                                    start=(pp == 0), stop=False)
                    return f
                return [], [None] * 7 + [acc_piece(pp) for pp in range(3)]

            # ---- attention: score chunks -> exp (Scalar/DVE split) ->
            # DoubleRow PV per chunk-pair, lagged
            pvq = []

            def pop_pv():
                fn = pvq.pop(0)
                fn()

            head0, tail0 = qkv_pieces(0)
            for piece in head0:
                piece()

            for p in range(NP):
                if p == 0:
                    head, tail = qkv_pieces(1)
                    pieces = tail0 + head + tail
                elif p + 1 < NP:
                    head, tail = qkv_pieces(p + 1)
                    pieces = head + tail
                else:
                    head, tail = prd_pieces()
                    pieces = tail
                kT_t, qT_t, v_t = built.pop(p)
                oA = psB.tile([65, 512], F32, tag="ov", name="oA")
                oB = psB.tile([65, 512], F32, tag="ov", name="oB")

                def mk_pv(oA, oB, v_t, p, cp, ex2):
                    def f():
                        for h, o in ((0, oA), (1, oB)):
                            nc.tensor.matmul(
                                o[:, :],
                                lhsT=v_t[:, 2 * cp:2 * cp + 2, h, 0:65],
                                rhs=ex2[:, h, :, :],
                                start=(cp == 0), stop=False,
                                perf_mode=DR)
                        if cp >= NCP // 2:
                            # subtract the spurious exp=1 contribution of
                            # the zeroed masked-query column (all-zero
                            # rhs on unmasked cores)
                            for h, o in ((0, oA), (1, oB)):
                                nc.tensor.matmul(
                                    o[:, 511:512],
                                    lhsT=v_t[:, 2 * cp:2 * cp + 2, h, 0:65],
                                    rhs=negm_sb[:, :, 0:1],
                                    start=False, stop=(cp == NCP - 1),
                                    perf_mode=DR)
                        if cp == NCP - 1:
                            # Drain oA/oB (raw bf16); denominators from row 64.
                            oTr = orpool.tile([128, 512], BF16, tag="or",
                                              name="oTr")
                            nc.vector.tensor_copy(out=oTr[0:64, :],
                                                  in_=oA[0:64, :])
                            nc.vector.tensor_copy(out=oTr[64:128, :],
                                                  in_=oB[0:64, :])
                            den = rpool.tile([1, 2, 512], BF16, tag="den",
                                             name="den")
                            deng = nc.scalar if p == NP - 1 else nc.vector
                            if p == NP - 1:
                                deng.copy(out=den[0:1, 0, :], in_=oA[64:65, :])
                                deng.copy(out=den[0:1, 1, :], in_=oB[64:65, :])
                            else:
                                deng.tensor_copy(out=den[0:1, 0, :],
                                                 in_=oA[64:65, :])
                                deng.tensor_copy(out=den[0:1, 1, :],
                                                 in_=oB[64:65, :])
                            dens[p] = (den, oTr)
                    return f

                ex2 = None
                for c in range(TCK):
                    cp, ci = divmod(c, 2)
                    if ci == 0:
                        ex2 = epool.tile([128, 2, 2, 512], FP8, name="ex2")
                    qv = 1 if c >= TCK // 2 else 0
                    if c == 0:
                        # chunk 0 scores go to psD banks (free since this
                        # pair's v-casts) so they can be computed during the
                        # PREVIOUS pair's tail; the first exp of this pair
                        # then starts the moment the last exp of the
                        # previous pair retires (no psA-bank seam stall)
                        sc0h = [psD.tile([128, 512], F32, tag="qkv",
                                         name="sc0h") for _ in range(2)]
                        with tc.high_priority():
                            for h in range(2):
                                nc.tensor.matmul(
                                    sc0h[h],
                                    lhsT=kT_t[ds(64 * h, 64), 0:128],
                                    rhs=qT_t[ds(64 * h, 64), qv, :],
                                    start=True, stop=True)
                            for h in range(2):
                                nc.scalar.activation(out=ex2[:, h, 0, :],
                                                     in_=sc0h[h],
                                                     func=AF.Exp)
                        if pieces:
                            piece = pieces.pop(0)
                            if piece:
                                piece()
                        continue
                    sc = psA.tile([128, 2, 512], F32, tag="sc", name="sc")
                    with tc.high_priority():
                        nc.tensor.matmul(sc[:, 0, :],
                                         lhsT=kT_t[0:64, ds(128 * c, 128)],
                                         rhs=qT_t[0:64, qv, :],
                                         start=True, stop=True)
                        nc.tensor.matmul(sc[:, 1, :],
                                         lhsT=kT_t[64:128, ds(128 * c, 128)],
                                         rhs=qT_t[64:128, qv, :],
                                         start=True, stop=True)
                    if c in DVE_CHUNKS:
                        with tc.high_priority():
                            nc.vector.tensor_scalar(
                                out=ex2[:, :, ci, :].bitcast(I8), in0=sc,
                                scalar1=EXPA, scalar2=EXPB,
                                op0=OP.mult, op1=OP.add)
                    else:
                        with tc.high_priority():
                            nc.scalar.activation(out=ex2[:, :, ci, :], in_=sc,
                                                 func=AF.Exp)
                    if ci == 1:
                        pvq.append(mk_pv(oA, oB, v_t, p, cp, ex2))
                    if len(pvq) >= 2 and (ci != 1 or cp != NCP - 1
                                          or p == NP - 1):
                        # defer the last chunk-pair's pop across the pair
                        # seam (except the final pair, whose drain gates
                        # the tail)
                        pop_pv()
                    if c == 6 and p > 0:
                        bc = psD.tile([128, 512], F32, tag="qkv", name="bc")
                        emit_norm_tail(p - 1, bc)
                    npiece = 2 if (p == 0 and c < 8) else 1
                    for _ in range(npiece):
                        if pieces:
                            piece = pieces.pop(0)
                            if piece:
                                piece()

            while pvq:
                pop_pv()

            # sqrt table preload: fills ScalarE's idle window right after
            # the last exp so the LN sqrts don't pay the table switch.
            # Reads the last sc tile so the scheduler cannot hoist it early
            # (which would evict the exp table set before the exps run).
            dum = stpool.tile([128, 1], F32, tag="dum", name="dum")
            nc.scalar.activation(out=dum, in_=sc[:, 0, 0:1], func=AF.Sqrt)

            # last pair's normalization first (bc takes the psA slot freed
            # at the last exp)
            prs = {}
            bc7 = psA.tile([128, 2, 512], F32, tag="sc", name="bc7")
            with tc.high_priority():
                emit_norm_tail(NP - 1, bc7[:, 0, :])

            # (0,*) groups: pre-accumulate pair-pairs 0..2 in the other psA
            # slot
            pr2a = psA.tile([128, 2, 512], F32, tag="sc", name="pr2a")
            for gi in range(2):
                for pp in range(3):
                    proj_mm(pr2a[:, gi, :], pp, 0, gi,
                            start=(pp == 0), stop=False)
                prs[(0, gi)] = pr2a[:, gi, :]

            # (1,*) groups: pre-accumulate in the psB slots freed by the
            # pair-7 drains
            for fc0 in range(2):
                prb = psB.tile([128, 512], F32, tag="ov", name="prb")
                for pp in range(3):
                    proj_mm(prb, pp, 1, fc0, start=(pp == 0), stop=False)
                prs[(1, fc0)] = prb
            # (2,0): the unused half of the bc7 tile is a free psum bank
            for pp in range(3):
                proj_mm(bc7[:, 1, :], pp, 2, 0, start=(pp == 0), stop=False)
            prs[(2, 0)] = bc7[:, 1, :]
            # (2,1): the broadcast half of bc7 frees once the reciprocal
            # has read it; start=True reclaims the bank
            for pp in range(3):
                proj_mm(bc7[:, 0, :], pp, 2, 1, start=(pp == 0), stop=False)
            prs[(2, 1)] = bc7[:, 0, :]
            prs.update(prd)
            prd.clear()

            if apply_gb:
                for i, t in enumerate([g_bc[:, 0:1], b_bc[:, 0:1]]):
                    nc.vector.tensor_copy(out=scr[:, 6 + i:7 + i], in_=t)

            # ---- finish projections + residual + fused-stats LayerNorm
            out_queues = [nc.sync, nc.scalar]
            ys = {}
            for m in range(4):
                y_t = ypool.tile([128, D], F32, tag="y", name="y_t")
                sums = stpool.tile([128, 3], F32, tag="sums", name="sums")
                for fc in range(2):
                    pr = prs.pop((m, fc))
                    with tc.high_priority():
                        proj_mm(pr, 3, m, fc, start=False, stop=True)
                    nc.vector.scalar_tensor_tensor(
                        out=y_t[:, ts(fc, 512)], in0=pr, scalar=1.0 / 512.0,
                        in1=xq_sb[:, m, ts(fc, 512)],
                        op0=OP.mult, op1=OP.add,
                        accum_out=sums[:, fc:fc + 1])
                ysq = ypool.tile([128, D], BF16, tag="ysq", name="ysq")
                nc.scalar.activation(out=ysq, in_=y_t, func=AF.Square,
                                     accum_out=sums[:, 2:3])
                ys[m] = (y_t, sums)

            # phase 2: stats combine, normalize, store
            for m in range(4):
                y_t, sums = ys.pop(m)
                mv = stpool.tile([128, 2], F32, tag="mv", name="mv")
                nc.vector.scalar_tensor_tensor(
                    out=mv[:, 0:1], in0=sums[:, 0:1], scalar=1.0,
                    in1=sums[:, 1:2], op0=OP.mult, op1=OP.add)
                nc.vector.tensor_scalar(out=mv[:, 0:1], in0=mv[:, 0:1],
                                        scalar1=1.0 / D, scalar2=None,
                                        op0=OP.mult)
                nc.vector.tensor_tensor(out=mv[:, 1:2], in0=mv[:, 0:1],
                                        in1=mv[:, 0:1], op=OP.mult)
                var = stpool.tile([128, 1], F32, tag="var", name="var")
                nc.vector.scalar_tensor_tensor(
                    out=var, in0=sums[:, 2:3], scalar=1.0 / D,
                    in1=mv[:, 1:2], op0=OP.mult, op1=OP.subtract)
                sd = stpool.tile([128, 1], F32, tag="sd", name="sd")
                nc.scalar.activation(out=sd, in_=var, func=AF.Sqrt,
                                     bias=eps_sb[:, 0:1], scale=1.0)
                rstd = stpool.tile([128, 1], F32, tag="rsd", name="rstd")
                nc.vector.reciprocal(out=rstd, in_=sd)
                yn = ypool.tile([128, D], F32, tag="yn", name="yn")
                nc.vector.tensor_scalar(out=yn, in0=y_t, scalar1=mv[:, 0:1],
                                        scalar2=rstd, op0=OP.subtract,
                                        op1=OP.mult)
                if apply_gb:
                    ot = ypool.tile([128, D], F32, tag="ot", name="ot")
                    nc.vector.tensor_tensor(out=ot[:, 0:512], in0=yn[:, 0:512],
                                            in1=g_bc[:, 0:512], op=OP.mult)
                    nc.vector.tensor_tensor(out=ot[:, 512:1024],
                                            in0=yn[:, 512:1024],
                                            in1=g_bc[:, 512:1024], op=OP.mult)
                    nc.vector.tensor_tensor(out=ot[:, 0:512], in0=ot[:, 0:512],
                                            in1=b_bc[:, 0:512], op=OP.add)
                    nc.vector.tensor_tensor(out=ot[:, 512:1024],
                                            in0=ot[:, 512:1024],
                                            in1=b_bc[:, 512:1024], op=OP.add)
                    for fc in range(2):
                        out_queues[fc].dma_start(
                            out=out[ds(128 * m, 128), ts(fc, 512)],
                            in_=ot[:, ts(fc, 512)])
                else:
                    for fc in range(2):
                        out_queues[fc].dma_start(
                            out=out[ds(128 * m, 128), ts(fc, 512)],
                            in_=yn[:, ts(fc, 512)])
    nc.compile()
    return nc


def prep_inputs(x, Wq, bq, Wk, bk, Wv, bv, Wo, bo, ln_g, ln_b):
    """Host-side sharding/layout prep -> list of 8 per-core input maps."""
    bf = ml_dtypes.bfloat16
    x = np.asarray(x, np.float32)
    Wq, Wk, Wv = (np.asarray(w, np.float32) for w in (Wq, Wk, Wv))
    Wo = np.asarray(Wo, np.float32)
    bq, bk, bv, bo = (np.asarray(v_, np.float32) for v_ in (bq, bk, bv, bo))
    ln_g, ln_b = np.asarray(ln_g, np.float32), np.asarray(ln_b, np.float32)

    def pairs(W):  # [H,d,d] -> [128,NP,128]: block-diag per pair, part-major
        out = np.zeros((NP, 128, 128), np.float32)
        for p in range(NP):
            out[p, :d, :d] = W[2 * p]
            out[p, d:, d:] = W[2 * p + 1]
        return np.ascontiguousarray(out.transpose(1, 0, 2)).astype(bf)

    wq_b, wk_b, wv_b = pairs(Wq), pairs(Wk), pairs(Wv)
    wfirst = np.ascontiguousarray(np.stack(
        [wk_b[:, 0], wk_b[:, 1], wq_b[:, 0], wv_b[:, 0]], axis=1))
    wrest = np.ascontiguousarray(np.concatenate(
        [wk_b[:, 2:NP], wq_b[:, 1:NP], wv_b[:, 1:NP]], axis=1))
    bqk = np.concatenate([bq.reshape(NP, 128).T, bk.reshape(NP, 128).T],
                         1).copy()             # [128, 2*NP]
    bvt = bv.reshape(NP, 128).copy()            # [NP, 128]
    e4 = ml_dtypes.float8_e4m3fn
    wo8_b = np.ascontiguousarray(
        (Wo * 8.0).reshape(NP, 128, D).transpose(1, 0, 2)).astype(e4)
    xT_all = [np.ascontiguousarray(x[b_].T).astype(bf) for b_ in range(B)]

    in_maps = []
    for c in range(N_CORES):
        b_, j = divmod(c, 4)
        rows = slice(j * SQ, (j + 1) * SQ)
        xq_pre = np.ascontiguousarray(
            (x[b_, rows] + bo).reshape(4, 128, D).transpose(1, 0, 2)
        ).astype(np.float32)                    # [128, 4, D]
        masked = (j == 3)
        negm = np.zeros((128, 2, 16), e4)
        if masked:
            negm[:, :, 0] = -1.0
        in_maps.append({
            "xT": xT_all[b_],
            "xqT": np.ascontiguousarray(xT_all[b_][:, rows]),
            "xq": xq_pre,
            "wfirst": wfirst, "wrest": wrest,
            "bqk": bqk, "bvt": bvt,
            "wo8": wo8_b.view(np.uint8),
            "gg": ln_g, "bb": ln_b,
            "msk": np.array([[0.0 if masked else 1.0]], np.float32),
            "negm": negm.view(np.uint8),
        })
    return in_maps


_NC = {}


def _get_nc(apply_gb, apply_qkvb):
    key = (apply_gb, apply_qkvb)
    if key not in _NC:
        _NC[key] = build_nc(apply_gb=apply_gb, apply_qkvb=apply_qkvb)
    return _NC[key]


def _gather(results):
    y = np.empty((B, S, D), np.float32)
    for c, r in enumerate(results):
        b_, j = divmod(c, 4)
        y[b_, j * SQ:(j + 1) * SQ] = r["out"]
    return y


def _needs_gb(ln_g, ln_b):
    return not (np.all(np.asarray(ln_g) == 1.0)
                and np.all(np.asarray(ln_b) == 0.0))


def _needs_qkvb(bq, bk, bv):
    return not all(np.all(np.asarray(b) == 0.0) for b in (bq, bk, bv))


def kernel(**inputs):
    apply_gb = _needs_gb(inputs["ln_g"], inputs["ln_b"])
    apply_qkvb = _needs_qkvb(inputs["bq"], inputs["bk"], inputs["bv"])
    nc = _get_nc(apply_gb, apply_qkvb)
    in_maps = prep_inputs(**inputs)
    res = run_bass_kernel_spmd(nc, in_maps, core_ids=list(range(N_CORES)))
    return _gather(res.results)


def kernel_timed(**inputs):
    """Returns (output, exec_time_ns or None). Used by test.py."""
    apply_gb = _needs_gb(inputs["ln_g"], inputs["ln_b"])
    apply_qkvb = _needs_qkvb(inputs["bq"], inputs["bk"], inputs["bv"])
    nc = _get_nc(apply_gb, apply_qkvb)
    in_maps = prep_inputs(**inputs)
    res = run_bass_kernel_spmd(nc, in_maps, core_ids=list(range(N_CORES)),
                               trace=True)
    return _gather(res.results), res.exec_time_ns


# revision 31
# speedup vs baseline: 1.0463x; 1.0192x over previous
"""Trainium2 Bass kernel for a fused multi-head attention layer.

Math (per batch b):
    xh = x.reshape(S, H, d); q/k/v = xh @ W{q,k,v}[h] + b
    scores = q @ k^T  (per head);  scores[-1, -1024:] = -inf
    attn = softmax(scores, -1) / sqrt(D)
    o = concat_h(attn @ v);  proj = o @ Wo + bo
    out = LayerNorm(x + proj) * g + beta

Sharding: 8 cores = 2 batches x 4 query-blocks of 512 rows. Each core
computes K/V for its full batch (duplicated across the 4 cores of a
batch) and Q/attention/projection/LN for its own 512 query rows. No
collectives.

v2 design notes (vs the all-bf16 v1):
  * The exp stream is split between ScalarE (activation Exp -> fp8e4)
    and the DVE (Schraudolph fast-exp: round(a*s + 56) to int8 IS the
    fp8e4 bit pattern of exp(s); verified round-to-nearest+saturate on
    HW).  Both engines also share the PSUM->SBUF cast pool.
  * V and the attention weights are fp8e4; the PV matmuls run in
    DoubleRow perf mode contracting two 128-key chunks at once
    (lhsT [128,2,65] incl the ones-column, rhs [128,2,256]); the
    ones-column still yields the softmax denominator for free.
  * The seq-mask costs no per-chunk work: score chunks >= 8 use a
    second qT whose column 511 is zeroed on the masked core (GpSimd),
    making the masked scores 0 -> exp = 1 exactly; per-pair DoubleRow
    fixup matmuls with rhs = -mask subtract the spurious sum_v/count
    from the PV output and denominator.
  * v-casts are batched 4 chunks per DVE op; oT normalization mult
    runs on GpSimd.
"""

import numpy as np
import ml_dtypes

import concourse.bass as bass
import concourse.mybir as mybir
import concourse.tile as tile
from concourse import bacc
from concourse.bass import ds, ts
from concourse.bass_utils import run_bass_kernel_spmd

BF16 = mybir.dt.bfloat16
F32 = mybir.dt.float32
FP8 = mybir.dt.float8e4
I8 = mybir.dt.int8
AF = mybir.ActivationFunctionType
OP = mybir.AluOpType
DR = mybir.MatmulPerfMode.DoubleRow

B, S, D, H = 2, 2048, 1024, 16
d = 64            # head dim
NP = H // 2       # 8 head pairs
SQ = S // 4       # 512 query rows per core
TCK = S // 128    # 16 key chunks of 128
NCP = TCK // 2    # 8 chunk-pairs
SEQ_LEN = 1024
SCALE = float(np.sqrt(D))
LN_EPS = 1e-5
N_CORES = 8
EXPA = 8.0 / float(np.log(2.0))   # Schraudolph slope for e4m3 bits
EXPB = 56.0                        # 8 * bias(7)
# chunks handled by the DVE fast-exp (rest on ScalarE); 15 lets the
# ScalarE run ahead into the next pair's psD chunk-0 exps at the seam
DVE_CHUNKS = (5, 9, 13, 15)


def _bcast(ap, p=128):
    """AP replicating `ap` across p partitions (partition step 0)."""
    return bass.AP(tensor=ap.tensor, offset=ap.offset, ap=[[0, p]] + list(ap.ap))


def build_nc(apply_gb=True, apply_qkvb=True):
    nc = bacc.Bacc("TRN2")

    xT = nc.dram_tensor("xT", [D, S], BF16, kind="ExternalInput")       # x[b].T
    xqT = nc.dram_tensor("xqT", [D, SQ], BF16, kind="ExternalInput")    # x[b,rows].T
    xq = nc.dram_tensor("xq", [128, 4, D], F32, kind="ExternalInput")   # x[b,rows]+bo
    # combined weight loads: one DMA descriptor each (descriptor gen on the
    # gpsimd ring is ~640ns apiece and serializes startup)
    wfirst = nc.dram_tensor("wfirst", [128, 4, 128], BF16, kind="ExternalInput")
    wrest = nc.dram_tensor("wrest", [128, 20, 128], BF16, kind="ExternalInput")
    bqk = nc.dram_tensor("bqk", [128, 2 * NP], F32, kind="ExternalInput")
    bvt = nc.dram_tensor("bvt", [NP, 128], F32, kind="ExternalInput")
    wo8 = nc.dram_tensor("wo8", [128, NP, D], FP8, kind="ExternalInput")
    gg = nc.dram_tensor("gg", [D], F32, kind="ExternalInput")
    bb = nc.dram_tensor("bb", [D], F32, kind="ExternalInput")
    msk = nc.dram_tensor("msk", [1, 1], F32, kind="ExternalInput")      # 0 if masked
    negm = nc.dram_tensor("negm", [128, 2, 16], FP8, kind="ExternalInput")
    out = nc.dram_tensor("out", [SQ, D], F32, kind="ExternalOutput")

    with tile.TileContext(nc) as tc:
        with (
            tc.tile_pool(name="singles", bufs=1) as singles,
            tc.tile_pool(name="xpool", bufs=2) as xpool,
            tc.tile_pool(name="kpool", bufs=2) as kpool,
            tc.tile_pool(name="qpool", bufs=2) as qpool,
            tc.tile_pool(name="qxpool", bufs=2) as qxpool,
            tc.tile_pool(name="vpool", bufs=2) as vpool,
            tc.tile_pool(name="epool", bufs=4) as epool,
            tc.tile_pool(name="rpool", bufs=2) as rpool,
            tc.tile_pool(name="orpool", bufs=2) as orpool,
            tc.tile_pool(name="ypool", bufs=4) as ypool,
            tc.tile_pool(name="stpool", bufs=4) as stpool,
            tc.tile_pool(name="psA", bufs=2, space="PSUM") as psA,
            tc.tile_pool(name="psB", bufs=2, space="PSUM") as psB,
            tc.tile_pool(name="psD", bufs=2, space="PSUM") as psD,
        ):
            # ---- warm-up: bridge the PE HAM clock gate until real MMs
            wu = singles.tile([128, 512], BF16)
            nc.vector.memset(wu, 0.0)
            for _ in range(8):
                wps = psD.tile([128, 512], F32, tag="qkv", name="wps")
                nc.tensor.matmul(wps, lhsT=wu[:, 0:128], rhs=wu,
                                 start=True, stop=True)

            # ---- constants / weights (contiguous host-prearranged DMAs).
            # wfirst = [wk p0, wk p1, wq p0, wv p0]; wrest = wk p2..7 +
            # wq p1..7 + wv p1..7 (one descriptor each on the gpsimd ring)
            wf_sb = singles.tile([128, 4, 128], BF16)
            wr_sb = singles.tile([128, 20, 128], BF16)
            msk_sb = singles.tile([128, 1], F32)
            negm_sb = singles.tile([128, 2, 16], FP8)
            nc.gpsimd.dma_start(out=wf_sb, in_=wfirst[:])
            nc.scalar.dma_start(out=msk_sb, in_=_bcast(msk[:].rearrange("a b -> (a b)")))
            nc.scalar.dma_start(out=negm_sb, in_=negm[:])
            nc.gpsimd.dma_start(out=wr_sb, in_=wrest[:])

            def wk_ap(p):
                return wf_sb[:, p, :] if p < 2 else wr_sb[:, p - 2, :]

            def wq_ap(p):
                return wf_sb[:, 2, :] if p < 1 else wr_sb[:, 5 + p, :]

            def wv_ap(p):
                return wf_sb[:, 3, :] if p < 1 else wr_sb[:, 12 + p, :]

            if apply_qkvb:
                bqk_sb = singles.tile([128, 2 * NP], F32)
                nc.gpsimd.dma_start(out=bqk_sb, in_=bqk[:])
                bq_sb = bqk_sb[:, 0:NP]
                bk_sb = bqk_sb[:, NP:2 * NP]
                bv_bc = singles.tile([128, NP, 128], F32)
                nc.gpsimd.dma_start(out=bv_bc, in_=_bcast(bvt[:]))
            # bulk tail-only tensors (wo8/xq/ln): the DMA issue is DEFERRED
            # into pair 2's piece stream -- issuing them at t=0 saturates
            # HBM and starves the startup-critical xT/weight loads
            wo8_sb = singles.tile([128, NP, D], FP8)
            xq_sb = singles.tile([128, 4, D], F32)
            if apply_gb:
                g_bc = singles.tile([128, D], F32)
                b_bc = singles.tile([128, D], F32)

            def bulk_dma_piece():
                nc.gpsimd.dma_start(out=wo8_sb, in_=wo8[:])
                nc.gpsimd.dma_start(out=xq_sb, in_=xq[:])
                if apply_gb:
                    nc.gpsimd.dma_start(out=g_bc, in_=_bcast(gg[:]))
                    nc.gpsimd.dma_start(out=b_bc, in_=_bcast(bb[:]))
            eps_sb = singles.tile([128, 1], F32)
            nc.vector.memset(eps_sb, LN_EPS)
            # oT is stored fp8 scaled by 64 (wo is prescaled by 8 on host;
            # the residual add divides by 512): bcast = (SCALE/64)*den, so
            # 1/bcast = 64/(SCALE*den)
            ones_sc = singles.tile([1, d], BF16)
            nc.vector.memset(ones_sc, SCALE / 64.0)
            oT8_sb = singles.tile([128, NP, SQ], FP8)

            # Touch DMA-loaded constants once on VectorE / GpSimd so later
            # consumers need no DMA waits.
            scr = singles.tile([128, 8], F32)
            touches = [msk_sb[:, 0:1]]
            if apply_qkvb:
                touches += [bqk_sb[:, 0:1], bv_bc[:, 0, 0:1]]
            for i, t in enumerate(touches):
                nc.vector.tensor_copy(out=scr[:, i:i + 1], in_=t)
            scr8 = singles.tile([128, 16], FP8)
            nc.gpsimd.tensor_copy(out=scr8, in_=negm_sb[:, 0, :])

            # ---- per-pair qkv emission pieces -------------------------
            built = {}

            def qkv_pieces(p):
                xT_t = xpool.tile([128, S], BF16, name="xT_t")
                xqT_t = qxpool.tile([128, SQ], BF16, name="xqT_t")
                kT_t = kpool.tile([128, S], BF16, name="kT_t")
                qT_t = qpool.tile([128, 2, SQ], BF16, name="qT_t")
                v_t = vpool.tile([128, TCK, 2, 80], FP8, name="v_t")
                built[p] = (kT_t, qT_t, v_t)
                head = []

                def dma_piece():
                    # xqT on the scalar ring (parallel to sync); xT split in
                    # 4 so k_piece(c) waits only on its own 512-col chunk
                    nc.scalar.dma_start(out=xqT_t, in_=xqT[ds(128 * p, 128), :])
                    for c in range(4):
                        nc.sync.dma_start(out=xT_t[:, ts(c, 512)],
                                          in_=xT[ds(128 * p, 128), ts(c, 512)])
                head.append(dma_piece)

                def k_piece(c):
                    def f():
                        ps = psD.tile([128, 512], F32, tag="qkv", name="ps")
                        nc.tensor.matmul(ps, lhsT=wk_ap(p),
                                         rhs=xT_t[:, ts(c, 512)],
                                         start=True, stop=True)
                        if apply_qkvb:
                            nc.vector.tensor_scalar(
                                out=kT_t[:, ts(c, 512)], in0=ps,
                                scalar1=bk_sb[:, p:p + 1],
                                scalar2=None, op0=OP.add)
                        else:
                            nc.vector.tensor_copy(out=kT_t[:, ts(c, 512)],
                                                  in_=ps)
                    return f
                for c in range(4):
                    head.append(k_piece(c))

                def q_piece():
                    ps = psD.tile([128, 512], F32, tag="qkv", name="ps")
                    nc.tensor.matmul(ps, lhsT=wq_ap(p), rhs=xqT_t,
                                     start=True, stop=True)
                    if apply_qkvb:
                        nc.vector.tensor_scalar(out=qT_t[:, 0, :], in0=ps,
                                                scalar1=bq_sb[:, p:p + 1],
                                                scalar2=None, op0=OP.add)
                    else:
                        nc.vector.tensor_copy(out=qT_t[:, 0, :], in_=ps)
                    # masked-query variant for key chunks >= 8: col 511
                    # scaled by msk (0 on the masked core -> score 0)
                    if apply_qkvb:
                        nc.vector.tensor_scalar(out=qT_t[:, 1, :], in0=ps,
                                                scalar1=bq_sb[:, p:p + 1],
                                                scalar2=None, op0=OP.add)
                    else:
                        nc.vector.tensor_copy(out=qT_t[:, 1, :], in_=ps)
                    nc.gpsimd.tensor_scalar(out=qT_t[:, 1, 511:512],
                                            in0=qT_t[:, 1, 511:512],
                                            scalar1=msk_sb[:, 0:1],
                                            scalar2=None, op0=OP.mult)
                head.append(q_piece)

                def ones_piece():
                    nc.gpsimd.memset(v_t[:, :, :, 64:65], 1.0)
                head.append(ones_piece)

                tail = []

                def v_piece(tc0):
                    def f():
                        ps = psD.tile([128, 512], F32, tag="qkv", name="ps")
                        for j in range(4):
                            nc.tensor.matmul(ps[:, ts(j, 128)],
                                             lhsT=xT_t[:, ds(128 * (tc0 + j), 128)],
                                             rhs=wv_ap(p),
                                             start=True, stop=True)
                        if apply_qkvb:
                            for j in range(4):
                                nc.vector.tensor_tensor(
                                    out=v_t[:, tc0 + j, :, 0:64],
                                    in0=ps[:, ts(j, 128)].rearrange(
                                        "a (h e) -> a h e", h=2),
                                    in1=bv_bc[:, p, :].rearrange(
                                        "a (h e) -> a h e", h=2),
                                    op=OP.add)
                        else:
                            nc.vector.tensor_copy(
                                out=v_t[:, tc0:tc0 + 4, :, 0:64],
                                in_=ps.rearrange("a (c h e) -> a c h e",
                                                 c=4, h=2))
                    return f
                for tc0 in range(0, TCK, 4):
                    tail.append(v_piece(tc0))
                return head, tail

            # ---- normalization tail: oT8[:, pp, :] = oTr * 64/(SCALE*den)
            dens = {}

            def emit_norm_tail(pp, bc):
                den, oTr = dens.pop(pp)
                nc.tensor.matmul(bc[0:64, :], lhsT=ones_sc[0:1, :],
                                 rhs=den[0:1, 0, :], start=True, stop=True)
                nc.tensor.matmul(bc[64:128, :], lhsT=ones_sc[0:1, :],
                                 rhs=den[0:1, 1, :], start=True, stop=True)
                scale_t = rpool.tile([128, 512], F32, tag="rs", name="scale_t")
                nc.vector.reciprocal_approx_fast(out=scale_t, in_=bc)
                # last pair's norm gates the whole projection tail: the DVE
                # is faster than GpSimd and free at that point
                eng = nc.vector if pp == NP - 1 else nc.gpsimd
                eng.tensor_tensor(out=oT8_sb[:, pp, :],
                                  in0=oTr, in1=scale_t,
                                  op=OP.mult)

            def proj_mm(t, pp, m, fc, start, stop):
                nc.tensor.matmul(t,
                                 lhsT=oT8_sb[:, 2 * pp:2 * pp + 2, ts(m, 128)],
                                 rhs=wo8_sb[:, 2 * pp:2 * pp + 2, ts(fc, 512)],
                                 start=start, stop=stop, perf_mode=DR)

            # psD pre-accumulated projection groups for m=3 (built during
            # the last pair's attention stream; pair-pairs 0..2 = pairs 0..5)
            prd = {}

            def prd_pieces():
                # t30/t31 allocation is deferred past chunk 6 so the
                # norm_tail(6) bc tile grabs a psD slot first (the t3x
                # slots are only released in the LN tail -> cycle)
                def acc_piece(pp):
                    def f():
                        if pp == 0:
                            prd[(3, 0)] = psD.tile([128, 512], F32,
                                                   tag="qkv", name="t30")
                            prd[(3, 1)] = psD.tile([128, 512], F32,
                                                   tag="qkv", name="t31")
                        for fc0 in range(2):
                            proj_mm(prd[(3, fc0)], pp, 3, fc0,
                                    start=(pp == 0), stop=False)
                    return f
                return [], [None] * 7 + [acc_piece(pp) for pp in range(3)]

            # ---- attention: score chunks -> exp (Scalar/DVE split) ->
            # DoubleRow PV per chunk-pair, lagged
            pvq = []

            def pop_pv():
                fn = pvq.pop(0)
                fn()

            head0, tail0 = qkv_pieces(0)
            for piece in head0:
                piece()

            for p in range(NP):
                if p == 0:
                    head, tail = qkv_pieces(1)
                    pieces = tail0 + head + tail
                elif p + 1 < NP:
                    head, tail = qkv_pieces(p + 1)
                    pieces = head + tail
                    if p == 1:
                        pieces = head + [bulk_dma_piece] + tail
                else:
                    head, tail = prd_pieces()
                    pieces = tail
                kT_t, qT_t, v_t = built.pop(p)
                oA = psB.tile([65, 512], F32, tag="ov", name="oA")
                oB = psB.tile([65, 512], F32, tag="ov", name="oB")

                def mk_pv(oA, oB, v_t, p, cp, ex2):
                    def f():
                        for h, o in ((0, oA), (1, oB)):
                            nc.tensor.matmul(
                                o[:, :],
                                lhsT=v_t[:, 2 * cp:2 * cp + 2, h, 0:65],
                                rhs=ex2[:, h, :, :],
                                start=(cp == 0), stop=False,
                                perf_mode=DR)
                        if cp >= NCP // 2:
                            # subtract the spurious exp=1 contribution of
                            # the zeroed masked-query column (all-zero
                            # rhs on unmasked cores)
                            for h, o in ((0, oA), (1, oB)):
                                nc.tensor.matmul(
                                    o[:, 511:512],
                                    lhsT=v_t[:, 2 * cp:2 * cp + 2, h, 0:65],
                                    rhs=negm_sb[:, :, 0:1],
                                    start=False, stop=(cp == NCP - 1),
                                    perf_mode=DR)
                        if cp == NCP - 1:
                            # Drain oA/oB (raw bf16); denominators from row 64.
                            oTr = orpool.tile([128, 512], BF16, tag="or",
                                              name="oTr")
                            nc.vector.tensor_copy(out=oTr[0:64, :],
                                                  in_=oA[0:64, :])
                            nc.vector.tensor_copy(out=oTr[64:128, :],
                                                  in_=oB[0:64, :])
                            den = rpool.tile([1, 2, 512], BF16, tag="den",
                                             name="den")
                            deng = nc.scalar if p == NP - 1 else nc.vector
                            if p == NP - 1:
                                deng.copy(out=den[0:1, 0, :], in_=oA[64:65, :])
                                deng.copy(out=den[0:1, 1, :], in_=oB[64:65, :])
                            else:
                                deng.tensor_copy(out=den[0:1, 0, :],
                                                 in_=oA[64:65, :])
                                deng.tensor_copy(out=den[0:1, 1, :],
                                                 in_=oB[64:65, :])
                            dens[p] = (den, oTr)
                    return f

                ex2 = None
                for c in range(TCK):
                    cp, ci = divmod(c, 2)
                    if ci == 0:
                        ex2 = epool.tile([128, 2, 2, 512], FP8, name="ex2")
                    qv = 1 if c >= TCK // 2 else 0
                    if c == 0:
                        # chunk 0 scores go to psD banks (free since this
                        # pair's v-casts) so they can be computed during the
                        # PREVIOUS pair's tail; the first exp of this pair
                        # then starts the moment the last exp of the
                        # previous pair retires (no psA-bank seam stall)
                        sc0h = [psD.tile([128, 512], F32, tag="qkv",
                                         name="sc0h") for _ in range(2)]
                        with tc.high_priority():
                            for h in range(2):
                                nc.tensor.matmul(
                                    sc0h[h],
                                    lhsT=kT_t[ds(64 * h, 64), 0:128],
                                    rhs=qT_t[ds(64 * h, 64), qv, :],
                                    start=True, stop=True)
                            for h in range(2):
                                nc.scalar.activation(out=ex2[:, h, 0, :],
                                                     in_=sc0h[h],
                                                     func=AF.Exp)
                        if pieces:
                            piece = pieces.pop(0)
                            if piece:
                                piece()
                        continue
                    sc = psA.tile([128, 2, 512], F32, tag="sc", name="sc")
                    with tc.high_priority():
                        nc.tensor.matmul(sc[:, 0, :],
                                         lhsT=kT_t[0:64, ds(128 * c, 128)],
                                         rhs=qT_t[0:64, qv, :],
                                         start=True, stop=True)
                        nc.tensor.matmul(sc[:, 1, :],
                                         lhsT=kT_t[64:128, ds(128 * c, 128)],
                                         rhs=qT_t[64:128, qv, :],
                                         start=True, stop=True)
                    if c in DVE_CHUNKS:
                        with tc.high_priority():
                            nc.vector.tensor_scalar(
                                out=ex2[:, :, ci, :].bitcast(I8), in0=sc,
                                scalar1=EXPA, scalar2=EXPB,
                                op0=OP.mult, op1=OP.add)
                    else:
                        with tc.high_priority():
                            nc.scalar.activation(out=ex2[:, :, ci, :], in_=sc,
                                                 func=AF.Exp)
                    if ci == 1:
                        pvq.append(mk_pv(oA, oB, v_t, p, cp, ex2))
                    if len(pvq) >= 2 and (ci != 1 or cp != NCP - 1
                                          or p == NP - 1):
                        # defer the last chunk-pair's pop across the pair
                        # seam (except the final pair, whose drain gates
                        # the tail)
                        pop_pv()
                    if c == 6 and p > 0:
                        bc = psD.tile([128, 512], F32, tag="qkv", name="bc")
                        emit_norm_tail(p - 1, bc)
                    npiece = 2 if (p == 0 and c < 8) else 1
                    for _ in range(npiece):
                        if pieces:
                            piece = pieces.pop(0)
                            if piece:
                                piece()

            while pvq:
                pop_pv()

            # sqrt table preload: fills ScalarE's idle window right after
            # the last exp so the LN sqrts don't pay the table switch.
            # Reads the last sc tile so the scheduler cannot hoist it early
            # (which would evict the exp table set before the exps run).
            dum = stpool.tile([128, 1], F32, tag="dum", name="dum")
            nc.scalar.activation(out=dum, in_=sc[:, 0, 0:1], func=AF.Sqrt)

            # last pair's normalization first (bc takes the psA slot freed
            # at the last exp)
            prs = {}
            bc7 = psA.tile([128, 2, 512], F32, tag="sc", name="bc7")
            with tc.high_priority():
                emit_norm_tail(NP - 1, bc7[:, 0, :])

            # (0,*) groups: pre-accumulate pair-pairs 0..2 in the other psA
            # slot
            pr2a = psA.tile([128, 2, 512], F32, tag="sc", name="pr2a")
            for gi in range(2):
                for pp in range(3):
                    proj_mm(pr2a[:, gi, :], pp, 0, gi,
                            start=(pp == 0), stop=False)
                prs[(0, gi)] = pr2a[:, gi, :]

            # (1,*) groups: pre-accumulate in the psB slots freed by the
            # pair-7 drains
            for fc0 in range(2):
                prb = psB.tile([128, 512], F32, tag="ov", name="prb")
                for pp in range(3):
                    proj_mm(prb, pp, 1, fc0, start=(pp == 0), stop=False)
                prs[(1, fc0)] = prb
            # (2,0): the unused half of the bc7 tile is a free psum bank
            for pp in range(3):
                proj_mm(bc7[:, 1, :], pp, 2, 0, start=(pp == 0), stop=False)
            prs[(2, 0)] = bc7[:, 1, :]
            # (2,1): the broadcast half of bc7 frees once the reciprocal
            # has read it; start=True reclaims the bank
            for pp in range(3):
                proj_mm(bc7[:, 0, :], pp, 2, 1, start=(pp == 0), stop=False)
            prs[(2, 1)] = bc7[:, 0, :]
            prs.update(prd)
            prd.clear()

            if apply_gb:
                for i, t in enumerate([g_bc[:, 0:1], b_bc[:, 0:1]]):
                    nc.vector.tensor_copy(out=scr[:, 6 + i:7 + i], in_=t)

            # ---- finish projections + residual + fused-stats LayerNorm
            out_queues = [nc.sync, nc.scalar]
            ys = {}
            for m in range(4):
                y_t = ypool.tile([128, D], F32, tag="y", name="y_t")
                sums = stpool.tile([128, 3], F32, tag="sums", name="sums")
                for fc in range(2):
                    pr = prs.pop((m, fc))
                    with tc.high_priority():
                        proj_mm(pr, 3, m, fc, start=False, stop=True)
                    nc.vector.scalar_tensor_tensor(
                        out=y_t[:, ts(fc, 512)], in0=pr, scalar=1.0 / 512.0,
                        in1=xq_sb[:, m, ts(fc, 512)],
                        op0=OP.mult, op1=OP.add,
                        accum_out=sums[:, fc:fc + 1])
                ysq = ypool.tile([128, D], BF16, tag="ysq", name="ysq")
                nc.scalar.activation(out=ysq, in_=y_t, func=AF.Square,
                                     accum_out=sums[:, 2:3])
                ys[m] = (y_t, sums)

            # phase 2: stats combine, normalize, store
            for m in range(4):
                y_t, sums = ys.pop(m)
                mv = stpool.tile([128, 2], F32, tag="mv", name="mv")
                nc.vector.scalar_tensor_tensor(
                    out=mv[:, 0:1], in0=sums[:, 0:1], scalar=1.0,
                    in1=sums[:, 1:2], op0=OP.mult, op1=OP.add)
                nc.vector.tensor_scalar(out=mv[:, 0:1], in0=mv[:, 0:1],
                                        scalar1=1.0 / D, scalar2=None,
                                        op0=OP.mult)
                nc.vector.tensor_tensor(out=mv[:, 1:2], in0=mv[:, 0:1],
                                        in1=mv[:, 0:1], op=OP.mult)
                var = stpool.tile([128, 1], F32, tag="var", name="var")
                nc.vector.scalar_tensor_tensor(
                    out=var, in0=sums[:, 2:3], scalar=1.0 / D,
                    in1=mv[:, 1:2], op0=OP.mult, op1=OP.subtract)
                sd = stpool.tile([128, 1], F32, tag="sd", name="sd")
                nc.scalar.activation(out=sd, in_=var, func=AF.Sqrt,
                                     bias=eps_sb[:, 0:1], scale=1.0)
                rstd = stpool.tile([128, 1], F32, tag="rsd", name="rstd")
                nc.vector.reciprocal(out=rstd, in_=sd)
                yn = ypool.tile([128, D], F32, tag="yn", name="yn")
                nc.vector.tensor_scalar(out=yn, in0=y_t, scalar1=mv[:, 0:1],
                                        scalar2=rstd, op0=OP.subtract,
                                        op1=OP.mult)
                if apply_gb:
                    ot = ypool.tile([128, D], F32, tag="ot", name="ot")
                    nc.vector.tensor_tensor(out=ot[:, 0:512], in0=yn[:, 0:512],
                                            in1=g_bc[:, 0:512], op=OP.mult)
                    nc.vector.tensor_tensor(out=ot[:, 512:1024],
                                            in0=yn[:, 512:1024],
                                            in1=g_bc[:, 512:1024], op=OP.mult)
                    nc.vector.tensor_tensor(out=ot[:, 0:512], in0=ot[:, 0:512],
                                            in1=b_bc[:, 0:512], op=OP.add)
                    nc.vector.tensor_tensor(out=ot[:, 512:1024],
                                            in0=ot[:, 512:1024],
                                            in1=b_bc[:, 512:1024], op=OP.add)
                    for fc in range(2):
                        out_queues[fc].dma_start(
                            out=out[ds(128 * m, 128), ts(fc, 512)],
                            in_=ot[:, ts(fc, 512)])
                else:
                    for fc in range(2):
                        out_queues[fc].dma_start(
                            out=out[ds(128 * m, 128), ts(fc, 512)],
                            in_=yn[:, ts(fc, 512)])
    nc.compile()
    return nc


def prep_inputs(x, Wq, bq, Wk, bk, Wv, bv, Wo, bo, ln_g, ln_b):
    """Host-side sharding/layout prep -> list of 8 per-core input maps."""
    bf = ml_dtypes.bfloat16
    x = np.asarray(x, np.float32)
    Wq, Wk, Wv = (np.asarray(w, np.float32) for w in (Wq, Wk, Wv))
    Wo = np.asarray(Wo, np.float32)
    bq, bk, bv, bo = (np.asarray(v_, np.float32) for v_ in (bq, bk, bv, bo))
    ln_g, ln_b = np.asarray(ln_g, np.float32), np.asarray(ln_b, np.float32)

    def pairs(W):  # [H,d,d] -> [128,NP,128]: block-diag per pair, part-major
        out = np.zeros((NP, 128, 128), np.float32)
        for p in range(NP):
            out[p, :d, :d] = W[2 * p]
            out[p, d:, d:] = W[2 * p + 1]
        return np.ascontiguousarray(out.transpose(1, 0, 2)).astype(bf)

    wq_b, wk_b, wv_b = pairs(Wq), pairs(Wk), pairs(Wv)
    wfirst = np.ascontiguousarray(np.stack(
        [wk_b[:, 0], wk_b[:, 1], wq_b[:, 0], wv_b[:, 0]], axis=1))
    wrest = np.ascontiguousarray(np.concatenate(
        [wk_b[:, 2:NP], wq_b[:, 1:NP], wv_b[:, 1:NP]], axis=1))
    bqk = np.concatenate([bq.reshape(NP, 128).T, bk.reshape(NP, 128).T],
                         1).copy()             # [128, 2*NP]
    bvt = bv.reshape(NP, 128).copy()            # [NP, 128]
    e4 = ml_dtypes.float8_e4m3fn
    wo8_b = np.ascontiguousarray(
        (Wo * 8.0).reshape(NP, 128, D).transpose(1, 0, 2)).astype(e4)
    xT_all = [np.ascontiguousarray(x[b_].T).astype(bf) for b_ in range(B)]

    in_maps = []
    for c in range(N_CORES):
        b_, j = divmod(c, 4)
        rows = slice(j * SQ, (j + 1) * SQ)
        xq_pre = np.ascontiguousarray(
            (x[b_, rows] + bo).reshape(4, 128, D).transpose(1, 0, 2)
        ).astype(np.float32)                    # [128, 4, D]
        masked = (j == 3)
        negm = np.zeros((128, 2, 16), e4)
        if masked:
            negm[:, :, 0] = -1.0
        in_maps.append({
            "xT": xT_all[b_],
            "xqT": np.ascontiguousarray(xT_all[b_][:, rows]),
            "xq": xq_pre,
            "wfirst": wfirst, "wrest": wrest,
            "bqk": bqk, "bvt": bvt,
            "wo8": wo8_b.view(np.uint8),
            "gg": ln_g, "bb": ln_b,
            "msk": np.array([[0.0 if masked else 1.0]], np.float32),
            "negm": negm.view(np.uint8),
        })
    return in_maps


_NC = {}


def _get_nc(apply_gb, apply_qkvb):
    key = (apply_gb, apply_qkvb)
    if key not in _NC:
        _NC[key] = build_nc(apply_gb=apply_gb, apply_qkvb=apply_qkvb)
    return _NC[key]


def _gather(results):
    y = np.empty((B, S, D), np.float32)
    for c, r in enumerate(results):
        b_, j = divmod(c, 4)
        y[b_, j * SQ:(j + 1) * SQ] = r["out"]
    return y


def _needs_gb(ln_g, ln_b):
    return not (np.all(np.asarray(ln_g) == 1.0)
                and np.all(np.asarray(ln_b) == 0.0))


def _needs_qkvb(bq, bk, bv):
    return not all(np.all(np.asarray(b) == 0.0) for b in (bq, bk, bv))


def kernel(**inputs):
    apply_gb = _needs_gb(inputs["ln_g"], inputs["ln_b"])
    apply_qkvb = _needs_qkvb(inputs["bq"], inputs["bk"], inputs["bv"])
    nc = _get_nc(apply_gb, apply_qkvb)
    in_maps = prep_inputs(**inputs)
    res = run_bass_kernel_spmd(nc, in_maps, core_ids=list(range(N_CORES)))
    return _gather(res.results)


def kernel_timed(**inputs):
    """Returns (output, exec_time_ns or None). Used by test.py."""
    apply_gb = _needs_gb(inputs["ln_g"], inputs["ln_b"])
    apply_qkvb = _needs_qkvb(inputs["bq"], inputs["bk"], inputs["bv"])
    nc = _get_nc(apply_gb, apply_qkvb)
    in_maps = prep_inputs(**inputs)
    res = run_bass_kernel_spmd(nc, in_maps, core_ids=list(range(N_CORES)),
                               trace=True)
    return _gather(res.results), res.exec_time_ns


# revision 32
# speedup vs baseline: 1.0744x; 1.0268x over previous
"""Trainium2 Bass kernel for a fused multi-head attention layer.

Math (per batch b):
    xh = x.reshape(S, H, d); q/k/v = xh @ W{q,k,v}[h] + b
    scores = q @ k^T  (per head);  scores[-1, -1024:] = -inf
    attn = softmax(scores, -1) / sqrt(D)
    o = concat_h(attn @ v);  proj = o @ Wo + bo
    out = LayerNorm(x + proj) * g + beta

Sharding: 8 cores = 2 batches x 4 query-blocks of 512 rows. Each core
computes K/V for its full batch (duplicated across the 4 cores of a
batch) and Q/attention/projection/LN for its own 512 query rows. No
collectives.

v2 design notes (vs the all-bf16 v1):
  * The exp stream is split between ScalarE (activation Exp -> fp8e4)
    and the DVE (Schraudolph fast-exp: round(a*s + 56) to int8 IS the
    fp8e4 bit pattern of exp(s); verified round-to-nearest+saturate on
    HW).  Both engines also share the PSUM->SBUF cast pool.
  * V and the attention weights are fp8e4; the PV matmuls run in
    DoubleRow perf mode contracting two 128-key chunks at once
    (lhsT [128,2,65] incl the ones-column, rhs [128,2,256]); the
    ones-column still yields the softmax denominator for free.
  * The seq-mask costs no per-chunk work: score chunks >= 8 use a
    second qT whose column 511 is zeroed on the masked core (GpSimd),
    making the masked scores 0 -> exp = 1 exactly; per-pair DoubleRow
    fixup matmuls with rhs = -mask subtract the spurious sum_v/count
    from the PV output and denominator.
  * v-casts are batched 4 chunks per DVE op; oT normalization mult
    runs on GpSimd.
"""

import numpy as np
import ml_dtypes

import concourse.bass as bass
import concourse.mybir as mybir
import concourse.tile as tile
from concourse import bacc
from concourse.bass import ds, ts
from concourse.bass_utils import run_bass_kernel_spmd

BF16 = mybir.dt.bfloat16
F32 = mybir.dt.float32
FP8 = mybir.dt.float8e4
I8 = mybir.dt.int8
AF = mybir.ActivationFunctionType
OP = mybir.AluOpType
DR = mybir.MatmulPerfMode.DoubleRow

B, S, D, H = 2, 2048, 1024, 16
d = 64            # head dim
NP = H // 2       # 8 head pairs
SQ = S // 4       # 512 query rows per core
TCK = S // 128    # 16 key chunks of 128
NCP = TCK // 2    # 8 chunk-pairs
SEQ_LEN = 1024
SCALE = float(np.sqrt(D))
LN_EPS = 1e-5
N_CORES = 8
EXPA = 8.0 / float(np.log(2.0))   # Schraudolph slope for e4m3 bits
EXPB = 56.0                        # 8 * bias(7)
# chunks handled by the DVE fast-exp (rest on ScalarE); 15 lets the
# ScalarE run ahead into the next pair's psD chunk-0 exps at the seam
DVE_CHUNKS = (5, 9, 13, 15)


def _bcast(ap, p=128):
    """AP replicating `ap` across p partitions (partition step 0)."""
    return bass.AP(tensor=ap.tensor, offset=ap.offset, ap=[[0, p]] + list(ap.ap))


def build_nc(apply_gb=True, apply_qkvb=True):
    nc = bacc.Bacc("TRN2")

    xT = nc.dram_tensor("xT", [D, S], BF16, kind="ExternalInput")       # x[b].T
    xqT = nc.dram_tensor("xqT", [D, SQ], BF16, kind="ExternalInput")    # x[b,rows].T
    xq = nc.dram_tensor("xq", [128, 4, D], F32, kind="ExternalInput")   # x[b,rows]+bo
    # combined weight loads: one DMA descriptor each (descriptor gen on the
    # gpsimd ring is ~640ns apiece and serializes startup)
    wfirst = nc.dram_tensor("wfirst", [128, 4, 128], BF16, kind="ExternalInput")
    wrest = nc.dram_tensor("wrest", [128, 20, 128], BF16, kind="ExternalInput")
    bqk = nc.dram_tensor("bqk", [128, 2 * NP], F32, kind="ExternalInput")
    bvt = nc.dram_tensor("bvt", [NP, 128], F32, kind="ExternalInput")
    wo8 = nc.dram_tensor("wo8", [128, NP, D], FP8, kind="ExternalInput")
    gg = nc.dram_tensor("gg", [D], F32, kind="ExternalInput")
    bb = nc.dram_tensor("bb", [D], F32, kind="ExternalInput")
    msk = nc.dram_tensor("msk", [1, 1], F32, kind="ExternalInput")      # 0 if masked
    negm = nc.dram_tensor("negm", [128, 2, 16], FP8, kind="ExternalInput")
    out = nc.dram_tensor("out", [SQ, D], F32, kind="ExternalOutput")

    with tile.TileContext(nc) as tc:
        with (
            tc.tile_pool(name="singles", bufs=1) as singles,
            tc.tile_pool(name="xpool", bufs=2) as xpool,
            tc.tile_pool(name="kpool", bufs=2) as kpool,
            tc.tile_pool(name="qpool", bufs=2) as qpool,
            tc.tile_pool(name="qxpool", bufs=2) as qxpool,
            tc.tile_pool(name="vpool", bufs=2) as vpool,
            tc.tile_pool(name="epool", bufs=4) as epool,
            tc.tile_pool(name="rpool", bufs=2) as rpool,
            tc.tile_pool(name="orpool", bufs=2) as orpool,
            tc.tile_pool(name="ypool", bufs=4) as ypool,
            tc.tile_pool(name="stpool", bufs=4) as stpool,
            tc.tile_pool(name="psA", bufs=2, space="PSUM") as psA,
            tc.tile_pool(name="psB", bufs=2, space="PSUM") as psB,
            tc.tile_pool(name="psD", bufs=2, space="PSUM") as psD,
        ):
            # ---- warm-up: bridge the PE HAM clock gate until real MMs
            wu = singles.tile([128, 512], BF16)
            nc.vector.memset(wu, 0.0)
            for _ in range(8):
                wps = psD.tile([128, 512], F32, tag="qkv", name="wps")
                nc.tensor.matmul(wps, lhsT=wu[:, 0:128], rhs=wu,
                                 start=True, stop=True)

            # ---- constants / weights (contiguous host-prearranged DMAs).
            # wfirst = [wk p0, wk p1, wq p0, wv p0]; wrest = wk p2..7 +
            # wq p1..7 + wv p1..7 (one descriptor each on the gpsimd ring)
            wf_sb = singles.tile([128, 4, 128], BF16)
            wr_sb = singles.tile([128, 20, 128], BF16)
            msk_sb = singles.tile([128, 1], F32)
            negm_sb = singles.tile([128, 2, 16], FP8)
            nc.gpsimd.dma_start(out=wf_sb, in_=wfirst[:])
            nc.scalar.dma_start(out=msk_sb, in_=_bcast(msk[:].rearrange("a b -> (a b)")))
            nc.scalar.dma_start(out=negm_sb, in_=negm[:])
            nc.gpsimd.dma_start(out=wr_sb, in_=wrest[:])

            def wk_ap(p):
                return wf_sb[:, p, :] if p < 2 else wr_sb[:, p - 2, :]

            def wq_ap(p):
                return wf_sb[:, 2, :] if p < 1 else wr_sb[:, 5 + p, :]

            def wv_ap(p):
                return wf_sb[:, 3, :] if p < 1 else wr_sb[:, 12 + p, :]

            if apply_qkvb:
                bqk_sb = singles.tile([128, 2 * NP], F32)
                nc.gpsimd.dma_start(out=bqk_sb, in_=bqk[:])
                bq_sb = bqk_sb[:, 0:NP]
                bk_sb = bqk_sb[:, NP:2 * NP]
                bv_bc = singles.tile([128, NP, 128], F32)
                nc.gpsimd.dma_start(out=bv_bc, in_=_bcast(bvt[:]))
            # bulk tail-only tensors (wo8/xq/ln): the DMA issue is DEFERRED
            # into pair 2's piece stream -- issuing them at t=0 saturates
            # HBM and starves the startup-critical xT/weight loads
            wo8_sb = singles.tile([128, NP, D], FP8)
            xq_sb = singles.tile([128, 4, D], F32)
            if apply_gb:
                g_bc = singles.tile([128, D], F32)
                b_bc = singles.tile([128, D], F32)

            def bulk_dma_piece():
                # dummy one-element pre-writes sourced from oT8[:,0] (only
                # available after pair-0's norm): gives the DMAs a real WAW
                # dependency so the scheduler cannot hoist them to t=0
                # (they are otherwise dependency-free and get reordered
                # right back into the startup HBM crunch)
                nc.vector.tensor_copy(out=wo8_sb[0:1, 0, 0:1],
                                      in_=oT8_sb[0:1, 0, 0:1])
                nc.vector.tensor_copy(out=xq_sb[0:1, 0, 0:1],
                                      in_=oT8_sb[0:1, 0, 0:1])
                nc.gpsimd.dma_start(out=wo8_sb, in_=wo8[:])
                nc.gpsimd.dma_start(out=xq_sb, in_=xq[:])
                if apply_gb:
                    nc.vector.tensor_copy(out=g_bc[0:1, 0:1],
                                          in_=oT8_sb[0:1, 0, 0:1])
                    nc.vector.tensor_copy(out=b_bc[0:1, 0:1],
                                          in_=oT8_sb[0:1, 0, 0:1])
                    nc.gpsimd.dma_start(out=g_bc, in_=_bcast(gg[:]))
                    nc.gpsimd.dma_start(out=b_bc, in_=_bcast(bb[:]))
            eps_sb = singles.tile([128, 1], F32)
            nc.vector.memset(eps_sb, LN_EPS)
            # oT is stored fp8 scaled by 64 (wo is prescaled by 8 on host;
            # the residual add divides by 512): bcast = (SCALE/64)*den, so
            # 1/bcast = 64/(SCALE*den)
            ones_sc = singles.tile([1, d], BF16)
            nc.vector.memset(ones_sc, SCALE / 64.0)
            oT8_sb = singles.tile([128, NP, SQ], FP8)

            # Touch DMA-loaded constants once on VectorE / GpSimd so later
            # consumers need no DMA waits.
            scr = singles.tile([128, 8], F32)
            touches = [msk_sb[:, 0:1]]
            if apply_qkvb:
                touches += [bqk_sb[:, 0:1], bv_bc[:, 0, 0:1]]
            for i, t in enumerate(touches):
                nc.vector.tensor_copy(out=scr[:, i:i + 1], in_=t)
            scr8 = singles.tile([128, 16], FP8)
            nc.gpsimd.tensor_copy(out=scr8, in_=negm_sb[:, 0, :])

            # ---- per-pair qkv emission pieces -------------------------
            built = {}

            def qkv_pieces(p):
                xT_t = xpool.tile([128, S], BF16, name="xT_t")
                xqT_t = qxpool.tile([128, SQ], BF16, name="xqT_t")
                kT_t = kpool.tile([128, S], BF16, name="kT_t")
                qT_t = qpool.tile([128, 2, SQ], BF16, name="qT_t")
                v_t = vpool.tile([128, TCK, 2, 80], FP8, name="v_t")
                built[p] = (kT_t, qT_t, v_t)
                head = []

                def dma_piece():
                    # xqT on the scalar ring (parallel to sync); xT split in
                    # 4 so k_piece(c) waits only on its own 512-col chunk
                    nc.scalar.dma_start(out=xqT_t, in_=xqT[ds(128 * p, 128), :])
                    for c in range(4):
                        nc.sync.dma_start(out=xT_t[:, ts(c, 512)],
                                          in_=xT[ds(128 * p, 128), ts(c, 512)])
                head.append(dma_piece)

                def k_piece(c):
                    def f():
                        ps = psD.tile([128, 512], F32, tag="qkv", name="ps")
                        nc.tensor.matmul(ps, lhsT=wk_ap(p),
                                         rhs=xT_t[:, ts(c, 512)],
                                         start=True, stop=True)
                        if apply_qkvb:
                            nc.vector.tensor_scalar(
                                out=kT_t[:, ts(c, 512)], in0=ps,
                                scalar1=bk_sb[:, p:p + 1],
                                scalar2=None, op0=OP.add)
                        else:
                            nc.vector.tensor_copy(out=kT_t[:, ts(c, 512)],
                                                  in_=ps)
                    return f
                for c in range(4):
                    head.append(k_piece(c))

                def q_piece():
                    ps = psD.tile([128, 512], F32, tag="qkv", name="ps")
                    nc.tensor.matmul(ps, lhsT=wq_ap(p), rhs=xqT_t,
                                     start=True, stop=True)
                    if apply_qkvb:
                        nc.vector.tensor_scalar(out=qT_t[:, 0, :], in0=ps,
                                                scalar1=bq_sb[:, p:p + 1],
                                                scalar2=None, op0=OP.add)
                    else:
                        nc.vector.tensor_copy(out=qT_t[:, 0, :], in_=ps)
                    # masked-query variant for key chunks >= 8: col 511
                    # scaled by msk (0 on the masked core -> score 0)
                    if apply_qkvb:
                        nc.vector.tensor_scalar(out=qT_t[:, 1, :], in0=ps,
                                                scalar1=bq_sb[:, p:p + 1],
                                                scalar2=None, op0=OP.add)
                    else:
                        nc.vector.tensor_copy(out=qT_t[:, 1, :], in_=ps)
                    nc.gpsimd.tensor_scalar(out=qT_t[:, 1, 511:512],
                                            in0=qT_t[:, 1, 511:512],
                                            scalar1=msk_sb[:, 0:1],
                                            scalar2=None, op0=OP.mult)
                head.append(q_piece)

                def ones_piece():
                    nc.gpsimd.memset(v_t[:, :, :, 64:65], 1.0)
                head.append(ones_piece)

                tail = []

                def v_piece(tc0):
                    def f():
                        ps = psD.tile([128, 512], F32, tag="qkv", name="ps")
                        for j in range(4):
                            nc.tensor.matmul(ps[:, ts(j, 128)],
                                             lhsT=xT_t[:, ds(128 * (tc0 + j), 128)],
                                             rhs=wv_ap(p),
                                             start=True, stop=True)
                        if apply_qkvb:
                            for j in range(4):
                                nc.vector.tensor_tensor(
                                    out=v_t[:, tc0 + j, :, 0:64],
                                    in0=ps[:, ts(j, 128)].rearrange(
                                        "a (h e) -> a h e", h=2),
                                    in1=bv_bc[:, p, :].rearrange(
                                        "a (h e) -> a h e", h=2),
                                    op=OP.add)
                        else:
                            nc.vector.tensor_copy(
                                out=v_t[:, tc0:tc0 + 4, :, 0:64],
                                in_=ps.rearrange("a (c h e) -> a c h e",
                                                 c=4, h=2))
                    return f
                for tc0 in range(0, TCK, 4):
                    tail.append(v_piece(tc0))
                return head, tail

            # ---- normalization tail: oT8[:, pp, :] = oTr * 64/(SCALE*den)
            dens = {}

            def emit_norm_tail(pp, bc):
                den, oTr = dens.pop(pp)
                nc.tensor.matmul(bc[0:64, :], lhsT=ones_sc[0:1, :],
                                 rhs=den[0:1, 0, :], start=True, stop=True)
                nc.tensor.matmul(bc[64:128, :], lhsT=ones_sc[0:1, :],
                                 rhs=den[0:1, 1, :], start=True, stop=True)
                scale_t = rpool.tile([128, 512], F32, tag="rs", name="scale_t")
                nc.vector.reciprocal_approx_fast(out=scale_t, in_=bc)
                # last pair's norm gates the whole projection tail: the DVE
                # is faster than GpSimd and free at that point
                eng = nc.vector if pp == NP - 1 else nc.gpsimd
                eng.tensor_tensor(out=oT8_sb[:, pp, :],
                                  in0=oTr, in1=scale_t,
                                  op=OP.mult)

            def proj_mm(t, pp, m, fc, start, stop):
                nc.tensor.matmul(t,
                                 lhsT=oT8_sb[:, 2 * pp:2 * pp + 2, ts(m, 128)],
                                 rhs=wo8_sb[:, 2 * pp:2 * pp + 2, ts(fc, 512)],
                                 start=start, stop=stop, perf_mode=DR)

            # psD pre-accumulated projection groups for m=3 (built during
            # the last pair's attention stream; pair-pairs 0..2 = pairs 0..5)
            prd = {}

            def prd_pieces():
                # t30/t31 allocation is deferred past chunk 6 so the
                # norm_tail(6) bc tile grabs a psD slot first (the t3x
                # slots are only released in the LN tail -> cycle)
                def acc_piece(pp):
                    def f():
                        if pp == 0:
                            prd[(3, 0)] = psD.tile([128, 512], F32,
                                                   tag="qkv", name="t30")
                            prd[(3, 1)] = psD.tile([128, 512], F32,
                                                   tag="qkv", name="t31")
                        for fc0 in range(2):
                            proj_mm(prd[(3, fc0)], pp, 3, fc0,
                                    start=(pp == 0), stop=False)
                    return f
                return [], [None] * 7 + [acc_piece(pp) for pp in range(3)]

            # ---- attention: score chunks -> exp (Scalar/DVE split) ->
            # DoubleRow PV per chunk-pair, lagged
            pvq = []

            def pop_pv():
                fn = pvq.pop(0)
                fn()

            head0, tail0 = qkv_pieces(0)
            for piece in head0:
                piece()

            for p in range(NP):
                if p == 0:
                    head, tail = qkv_pieces(1)
                    pieces = tail0 + head + tail
                elif p + 1 < NP:
                    head, tail = qkv_pieces(p + 1)
                    pieces = head + tail
                    if p == 1:
                        pieces = head + [bulk_dma_piece] + tail
                else:
                    head, tail = prd_pieces()
                    pieces = tail
                kT_t, qT_t, v_t = built.pop(p)
                oA = psB.tile([65, 512], F32, tag="ov", name="oA")
                oB = psB.tile([65, 512], F32, tag="ov", name="oB")

                def mk_pv(oA, oB, v_t, p, cp, ex2):
                    def f():
                        for h, o in ((0, oA), (1, oB)):
                            nc.tensor.matmul(
                                o[:, :],
                                lhsT=v_t[:, 2 * cp:2 * cp + 2, h, 0:65],
                                rhs=ex2[:, h, :, :],
                                start=(cp == 0), stop=False,
                                perf_mode=DR)
                        if cp >= NCP // 2:
                            # subtract the spurious exp=1 contribution of
                            # the zeroed masked-query column (all-zero
                            # rhs on unmasked cores)
                            for h, o in ((0, oA), (1, oB)):
                                nc.tensor.matmul(
                                    o[:, 511:512],
                                    lhsT=v_t[:, 2 * cp:2 * cp + 2, h, 0:65],
                                    rhs=negm_sb[:, :, 0:1],
                                    start=False, stop=(cp == NCP - 1),
                                    perf_mode=DR)
                        if cp == NCP - 1:
                            # Drain oA/oB (raw bf16); denominators from row 64.
                            oTr = orpool.tile([128, 512], BF16, tag="or",
                                              name="oTr")
                            nc.vector.tensor_copy(out=oTr[0:64, :],
                                                  in_=oA[0:64, :])
                            nc.vector.tensor_copy(out=oTr[64:128, :],
                                                  in_=oB[0:64, :])
                            den = rpool.tile([1, 2, 512], BF16, tag="den",
                                             name="den")
                            deng = nc.scalar if p == NP - 1 else nc.vector
                            if p == NP - 1:
                                deng.copy(out=den[0:1, 0, :], in_=oA[64:65, :])
                                deng.copy(out=den[0:1, 1, :], in_=oB[64:65, :])
                            else:
                                deng.tensor_copy(out=den[0:1, 0, :],
                                                 in_=oA[64:65, :])
                                deng.tensor_copy(out=den[0:1, 1, :],
                                                 in_=oB[64:65, :])
                            dens[p] = (den, oTr)
                    return f

                ex2 = None
                for c in range(TCK):
                    cp, ci = divmod(c, 2)
                    if ci == 0:
                        ex2 = epool.tile([128, 2, 2, 512], FP8, name="ex2")
                    qv = 1 if c >= TCK // 2 else 0
                    if c == 0:
                        # chunk 0 scores go to psD banks (free since this
                        # pair's v-casts) so they can be computed during the
                        # PREVIOUS pair's tail; the first exp of this pair
                        # then starts the moment the last exp of the
                        # previous pair retires (no psA-bank seam stall)
                        sc0h = [psD.tile([128, 512], F32, tag="qkv",
                                         name="sc0h") for _ in range(2)]
                        with tc.high_priority():
                            for h in range(2):
                                nc.tensor.matmul(
                                    sc0h[h],
                                    lhsT=kT_t[ds(64 * h, 64), 0:128],
                                    rhs=qT_t[ds(64 * h, 64), qv, :],
                                    start=True, stop=True)
                            for h in range(2):
                                nc.scalar.activation(out=ex2[:, h, 0, :],
                                                     in_=sc0h[h],
                                                     func=AF.Exp)
                        if pieces:
                            piece = pieces.pop(0)
                            if piece:
                                piece()
                        continue
                    sc = psA.tile([128, 2, 512], F32, tag="sc", name="sc")
                    with tc.high_priority():
                        nc.tensor.matmul(sc[:, 0, :],
                                         lhsT=kT_t[0:64, ds(128 * c, 128)],
                                         rhs=qT_t[0:64, qv, :],
                                         start=True, stop=True)
                        nc.tensor.matmul(sc[:, 1, :],
                                         lhsT=kT_t[64:128, ds(128 * c, 128)],
                                         rhs=qT_t[64:128, qv, :],
                                         start=True, stop=True)
                    if c in DVE_CHUNKS:
                        with tc.high_priority():
                            nc.vector.tensor_scalar(
                                out=ex2[:, :, ci, :].bitcast(I8), in0=sc,
                                scalar1=EXPA, scalar2=EXPB,
                                op0=OP.mult, op1=OP.add)
                    else:
                        with tc.high_priority():
                            nc.scalar.activation(out=ex2[:, :, ci, :], in_=sc,
                                                 func=AF.Exp)
                    if ci == 1:
                        pvq.append(mk_pv(oA, oB, v_t, p, cp, ex2))
                    if len(pvq) >= 2 and (ci != 1 or cp != NCP - 1
                                          or p == NP - 1):
                        # defer the last chunk-pair's pop across the pair
                        # seam (except the final pair, whose drain gates
                        # the tail)
                        pop_pv()
                    if c == 6 and p > 0:
                        bc = psD.tile([128, 512], F32, tag="qkv", name="bc")
                        emit_norm_tail(p - 1, bc)
                    npiece = 2 if (p == 0 and c < 8) else 1
                    for _ in range(npiece):
                        if pieces:
                            piece = pieces.pop(0)
                            if piece:
                                piece()

            while pvq:
                pop_pv()

            # sqrt table preload: fills ScalarE's idle window right after
            # the last exp so the LN sqrts don't pay the table switch.
            # Reads the last sc tile so the scheduler cannot hoist it early
            # (which would evict the exp table set before the exps run).
            dum = stpool.tile([128, 1], F32, tag="dum", name="dum")
            nc.scalar.activation(out=dum, in_=sc[:, 0, 0:1], func=AF.Sqrt)

            # last pair's normalization first (bc takes the psA slot freed
            # at the last exp)
            prs = {}
            bc7 = psA.tile([128, 2, 512], F32, tag="sc", name="bc7")
            with tc.high_priority():
                emit_norm_tail(NP - 1, bc7[:, 0, :])

            # (0,*) groups: pre-accumulate pair-pairs 0..2 in the other psA
            # slot
            pr2a = psA.tile([128, 2, 512], F32, tag="sc", name="pr2a")
            for gi in range(2):
                for pp in range(3):
                    proj_mm(pr2a[:, gi, :], pp, 0, gi,
                            start=(pp == 0), stop=False)
                prs[(0, gi)] = pr2a[:, gi, :]

            # (1,*) groups: pre-accumulate in the psB slots freed by the
            # pair-7 drains
            for fc0 in range(2):
                prb = psB.tile([128, 512], F32, tag="ov", name="prb")
                for pp in range(3):
                    proj_mm(prb, pp, 1, fc0, start=(pp == 0), stop=False)
                prs[(1, fc0)] = prb
            # (2,0): the unused half of the bc7 tile is a free psum bank
            for pp in range(3):
                proj_mm(bc7[:, 1, :], pp, 2, 0, start=(pp == 0), stop=False)
            prs[(2, 0)] = bc7[:, 1, :]
            # (2,1): the broadcast half of bc7 frees once the reciprocal
            # has read it; start=True reclaims the bank
            for pp in range(3):
                proj_mm(bc7[:, 0, :], pp, 2, 1, start=(pp == 0), stop=False)
            prs[(2, 1)] = bc7[:, 0, :]
            prs.update(prd)
            prd.clear()

            if apply_gb:
                for i, t in enumerate([g_bc[:, 0:1], b_bc[:, 0:1]]):
                    nc.vector.tensor_copy(out=scr[:, 6 + i:7 + i], in_=t)

            # ---- finish projections + residual + fused-stats LayerNorm
            out_queues = [nc.sync, nc.scalar]
            ys = {}
            for m in range(4):
                y_t = ypool.tile([128, D], F32, tag="y", name="y_t")
                sums = stpool.tile([128, 3], F32, tag="sums", name="sums")
                for fc in range(2):
                    pr = prs.pop((m, fc))
                    with tc.high_priority():
                        proj_mm(pr, 3, m, fc, start=False, stop=True)
                    nc.vector.scalar_tensor_tensor(
                        out=y_t[:, ts(fc, 512)], in0=pr, scalar=1.0 / 512.0,
                        in1=xq_sb[:, m, ts(fc, 512)],
                        op0=OP.mult, op1=OP.add,
                        accum_out=sums[:, fc:fc + 1])
                ysq = ypool.tile([128, D], BF16, tag="ysq", name="ysq")
                nc.scalar.activation(out=ysq, in_=y_t, func=AF.Square,
                                     accum_out=sums[:, 2:3])
                ys[m] = (y_t, sums)

            # phase 2: stats combine, normalize, store
            for m in range(4):
                y_t, sums = ys.pop(m)
                mv = stpool.tile([128, 2], F32, tag="mv", name="mv")
                nc.vector.scalar_tensor_tensor(
                    out=mv[:, 0:1], in0=sums[:, 0:1], scalar=1.0,
                    in1=sums[:, 1:2], op0=OP.mult, op1=OP.add)
                nc.vector.tensor_scalar(out=mv[:, 0:1], in0=mv[:, 0:1],
                                        scalar1=1.0 / D, scalar2=None,
                                        op0=OP.mult)
                nc.vector.tensor_tensor(out=mv[:, 1:2], in0=mv[:, 0:1],
                                        in1=mv[:, 0:1], op=OP.mult)
                var = stpool.tile([128, 1], F32, tag="var", name="var")
                nc.vector.scalar_tensor_tensor(
                    out=var, in0=sums[:, 2:3], scalar=1.0 / D,
                    in1=mv[:, 1:2], op0=OP.mult, op1=OP.subtract)
                sd = stpool.tile([128, 1], F32, tag="sd", name="sd")
                nc.scalar.activation(out=sd, in_=var, func=AF.Sqrt,
                                     bias=eps_sb[:, 0:1], scale=1.0)
                rstd = stpool.tile([128, 1], F32, tag="rsd", name="rstd")
                nc.vector.reciprocal(out=rstd, in_=sd)
                yn = ypool.tile([128, D], F32, tag="yn", name="yn")
                nc.vector.tensor_scalar(out=yn, in0=y_t, scalar1=mv[:, 0:1],
                                        scalar2=rstd, op0=OP.subtract,
                                        op1=OP.mult)
                if apply_gb:
                    ot = ypool.tile([128, D], F32, tag="ot", name="ot")
                    nc.vector.tensor_tensor(out=ot[:, 0:512], in0=yn[:, 0:512],
                                            in1=g_bc[:, 0:512], op=OP.mult)
                    nc.vector.tensor_tensor(out=ot[:, 512:1024],
                                            in0=yn[:, 512:1024],
                                            in1=g_bc[:, 512:1024], op=OP.mult)
                    nc.vector.tensor_tensor(out=ot[:, 0:512], in0=ot[:, 0:512],
                                            in1=b_bc[:, 0:512], op=OP.add)
                    nc.vector.tensor_tensor(out=ot[:, 512:1024],
                                            in0=ot[:, 512:1024],
                                            in1=b_bc[:, 512:1024], op=OP.add)
                    for fc in range(2):
                        out_queues[fc].dma_start(
                            out=out[ds(128 * m, 128), ts(fc, 512)],
                            in_=ot[:, ts(fc, 512)])
                else:
                    for fc in range(2):
                        out_queues[fc].dma_start(
                            out=out[ds(128 * m, 128), ts(fc, 512)],
                            in_=yn[:, ts(fc, 512)])
    nc.compile()
    return nc


def prep_inputs(x, Wq, bq, Wk, bk, Wv, bv, Wo, bo, ln_g, ln_b):
    """Host-side sharding/layout prep -> list of 8 per-core input maps."""
    bf = ml_dtypes.bfloat16
    x = np.asarray(x, np.float32)
    Wq, Wk, Wv = (np.asarray(w, np.float32) for w in (Wq, Wk, Wv))
    Wo = np.asarray(Wo, np.float32)
    bq, bk, bv, bo = (np.asarray(v_, np.float32) for v_ in (bq, bk, bv, bo))
    ln_g, ln_b = np.asarray(ln_g, np.float32), np.asarray(ln_b, np.float32)

    def pairs(W):  # [H,d,d] -> [128,NP,128]: block-diag per pair, part-major
        out = np.zeros((NP, 128, 128), np.float32)
        for p in range(NP):
            out[p, :d, :d] = W[2 * p]
            out[p, d:, d:] = W[2 * p + 1]
        return np.ascontiguousarray(out.transpose(1, 0, 2)).astype(bf)

    wq_b, wk_b, wv_b = pairs(Wq), pairs(Wk), pairs(Wv)
    wfirst = np.ascontiguousarray(np.stack(
        [wk_b[:, 0], wk_b[:, 1], wq_b[:, 0], wv_b[:, 0]], axis=1))
    wrest = np.ascontiguousarray(np.concatenate(
        [wk_b[:, 2:NP], wq_b[:, 1:NP], wv_b[:, 1:NP]], axis=1))
    bqk = np.concatenate([bq.reshape(NP, 128).T, bk.reshape(NP, 128).T],
                         1).copy()             # [128, 2*NP]
    bvt = bv.reshape(NP, 128).copy()            # [NP, 128]
    e4 = ml_dtypes.float8_e4m3fn
    wo8_b = np.ascontiguousarray(
        (Wo * 8.0).reshape(NP, 128, D).transpose(1, 0, 2)).astype(e4)
    xT_all = [np.ascontiguousarray(x[b_].T).astype(bf) for b_ in range(B)]

    in_maps = []
    for c in range(N_CORES):
        b_, j = divmod(c, 4)
        rows = slice(j * SQ, (j + 1) * SQ)
        xq_pre = np.ascontiguousarray(
            (x[b_, rows] + bo).reshape(4, 128, D).transpose(1, 0, 2)
        ).astype(np.float32)                    # [128, 4, D]
        masked = (j == 3)
        negm = np.zeros((128, 2, 16), e4)
        if masked:
            negm[:, :, 0] = -1.0
        in_maps.append({
            "xT": xT_all[b_],
            "xqT": np.ascontiguousarray(xT_all[b_][:, rows]),
            "xq": xq_pre,
            "wfirst": wfirst, "wrest": wrest,
            "bqk": bqk, "bvt": bvt,
            "wo8": wo8_b.view(np.uint8),
            "gg": ln_g, "bb": ln_b,
            "msk": np.array([[0.0 if masked else 1.0]], np.float32),
            "negm": negm.view(np.uint8),
        })
    return in_maps


_NC = {}


def _get_nc(apply_gb, apply_qkvb):
    key = (apply_gb, apply_qkvb)
    if key not in _NC:
        _NC[key] = build_nc(apply_gb=apply_gb, apply_qkvb=apply_qkvb)
    return _NC[key]


def _gather(results):
    y = np.empty((B, S, D), np.float32)
    for c, r in enumerate(results):
        b_, j = divmod(c, 4)
        y[b_, j * SQ:(j + 1) * SQ] = r["out"]
    return y


def _needs_gb(ln_g, ln_b):
    return not (np.all(np.asarray(ln_g) == 1.0)
                and np.all(np.asarray(ln_b) == 0.0))


def _needs_qkvb(bq, bk, bv):
    return not all(np.all(np.asarray(b) == 0.0) for b in (bq, bk, bv))


def kernel(**inputs):
    apply_gb = _needs_gb(inputs["ln_g"], inputs["ln_b"])
    apply_qkvb = _needs_qkvb(inputs["bq"], inputs["bk"], inputs["bv"])
    nc = _get_nc(apply_gb, apply_qkvb)
    in_maps = prep_inputs(**inputs)
    res = run_bass_kernel_spmd(nc, in_maps, core_ids=list(range(N_CORES)))
    return _gather(res.results)


def kernel_timed(**inputs):
    """Returns (output, exec_time_ns or None). Used by test.py."""
    apply_gb = _needs_gb(inputs["ln_g"], inputs["ln_b"])
    apply_qkvb = _needs_qkvb(inputs["bq"], inputs["bk"], inputs["bv"])
    nc = _get_nc(apply_gb, apply_qkvb)
    in_maps = prep_inputs(**inputs)
    res = run_bass_kernel_spmd(nc, in_maps, core_ids=list(range(N_CORES)),
                               trace=True)
    return _gather(res.results), res.exec_time_ns


# revision 35
# speedup vs baseline: 1.0794x; 1.0047x over previous
"""Trainium2 Bass kernel for a fused multi-head attention layer.

Math (per batch b):
    xh = x.reshape(S, H, d); q/k/v = xh @ W{q,k,v}[h] + b
    scores = q @ k^T  (per head);  scores[-1, -1024:] = -inf
    attn = softmax(scores, -1) / sqrt(D)
    o = concat_h(attn @ v);  proj = o @ Wo + bo
    out = LayerNorm(x + proj) * g + beta

Sharding: 8 cores = 2 batches x 4 query-blocks of 512 rows. Each core
computes K/V for its full batch (duplicated across the 4 cores of a
batch) and Q/attention/projection/LN for its own 512 query rows. No
collectives.

v2 design notes (vs the all-bf16 v1):
  * The exp stream is split between ScalarE (activation Exp -> fp8e4)
    and the DVE (Schraudolph fast-exp: round(a*s + 56) to int8 IS the
    fp8e4 bit pattern of exp(s); verified round-to-nearest+saturate on
    HW).  Both engines also share the PSUM->SBUF cast pool.
  * V and the attention weights are fp8e4; the PV matmuls run in
    DoubleRow perf mode contracting two 128-key chunks at once
    (lhsT [128,2,65] incl the ones-column, rhs [128,2,256]); the
    ones-column still yields the softmax denominator for free.
  * The seq-mask costs no per-chunk work: score chunks >= 8 use a
    second qT whose column 511 is zeroed on the masked core (GpSimd),
    making the masked scores 0 -> exp = 1 exactly; per-pair DoubleRow
    fixup matmuls with rhs = -mask subtract the spurious sum_v/count
    from the PV output and denominator.
  * v-casts are batched 4 chunks per DVE op; oT normalization mult
    runs on GpSimd.
"""

import numpy as np
import ml_dtypes

import concourse.bass as bass
import concourse.mybir as mybir
import concourse.tile as tile
from concourse import bacc
from concourse.bass import ds, ts
from concourse.bass_utils import run_bass_kernel_spmd

BF16 = mybir.dt.bfloat16
F32 = mybir.dt.float32
FP8 = mybir.dt.float8e4
I8 = mybir.dt.int8
AF = mybir.ActivationFunctionType
OP = mybir.AluOpType
DR = mybir.MatmulPerfMode.DoubleRow

B, S, D, H = 2, 2048, 1024, 16
d = 64            # head dim
NP = H // 2       # 8 head pairs
SQ = S // 4       # 512 query rows per core
TCK = S // 128    # 16 key chunks of 128
NCP = TCK // 2    # 8 chunk-pairs
SEQ_LEN = 1024
SCALE = float(np.sqrt(D))
LN_EPS = 1e-5
N_CORES = 8
EXPA = 8.0 / float(np.log(2.0))   # Schraudolph slope for e4m3 bits
EXPB = 56.0                        # 8 * bias(7)
# chunks handled by the DVE fast-exp (rest on ScalarE); 15 lets the
# ScalarE run ahead into the next pair's psD chunk-0 exps at the seam
DVE_CHUNKS = (5, 9, 13, 15)


def _bcast(ap, p=128):
    """AP replicating `ap` across p partitions (partition step 0)."""
    return bass.AP(tensor=ap.tensor, offset=ap.offset, ap=[[0, p]] + list(ap.ap))


def build_nc(apply_gb=True, apply_qkvb=True):
    nc = bacc.Bacc("TRN2")

    xT = nc.dram_tensor("xT", [D, S], BF16, kind="ExternalInput")       # x[b].T
    xqT = nc.dram_tensor("xqT", [D, SQ], BF16, kind="ExternalInput")    # x[b,rows].T
    xq = nc.dram_tensor("xq", [128, 4, D], F32, kind="ExternalInput")   # x[b,rows]+bo
    # combined weight loads: one DMA descriptor each (descriptor gen on the
    # gpsimd ring is ~640ns apiece and serializes startup)
    wfirst = nc.dram_tensor("wfirst", [128, 4, 128], BF16, kind="ExternalInput")
    wrest = nc.dram_tensor("wrest", [128, 20, 128], BF16, kind="ExternalInput")
    bqk = nc.dram_tensor("bqk", [128, 2 * NP], F32, kind="ExternalInput")
    bvt = nc.dram_tensor("bvt", [NP, 128], F32, kind="ExternalInput")
    wo8 = nc.dram_tensor("wo8", [128, NP, D], FP8, kind="ExternalInput")
    gg = nc.dram_tensor("gg", [D], F32, kind="ExternalInput")
    bb = nc.dram_tensor("bb", [D], F32, kind="ExternalInput")
    msk = nc.dram_tensor("msk", [1, 1], F32, kind="ExternalInput")      # 0 if masked
    negm = nc.dram_tensor("negm", [128, 2, 16], FP8, kind="ExternalInput")
    out = nc.dram_tensor("out", [SQ, D], F32, kind="ExternalOutput")

    with tile.TileContext(nc) as tc:
        with (
            tc.tile_pool(name="singles", bufs=1) as singles,
            tc.tile_pool(name="xpool", bufs=2) as xpool,
            tc.tile_pool(name="kpool", bufs=2) as kpool,
            tc.tile_pool(name="qpool", bufs=2) as qpool,
            tc.tile_pool(name="qxpool", bufs=2) as qxpool,
            tc.tile_pool(name="vpool", bufs=2) as vpool,
            tc.tile_pool(name="epool", bufs=4) as epool,
            tc.tile_pool(name="rpool", bufs=2) as rpool,
            tc.tile_pool(name="orpool", bufs=2) as orpool,
            tc.tile_pool(name="ypool", bufs=8) as ypool,
            tc.tile_pool(name="stpool", bufs=8) as stpool,
            tc.tile_pool(name="psA", bufs=2, space="PSUM") as psA,
            tc.tile_pool(name="psB", bufs=2, space="PSUM") as psB,
            tc.tile_pool(name="psD", bufs=2, space="PSUM") as psD,
        ):
            # ---- warm-up: bridge the PE HAM clock gate until real MMs
            wu = singles.tile([128, 512], BF16)
            nc.vector.memset(wu, 0.0)
            for _ in range(8):
                wps = psD.tile([128, 512], F32, tag="qkv", name="wps")
                nc.tensor.matmul(wps, lhsT=wu[:, 0:128], rhs=wu,
                                 start=True, stop=True)

            # ---- constants / weights (contiguous host-prearranged DMAs).
            # wfirst = [wk p0, wk p1, wq p0, wv p0]; wrest = wk p2..7 +
            # wq p1..7 + wv p1..7 (one descriptor each on the gpsimd ring)
            wf_sb = singles.tile([128, 4, 128], BF16)
            wr_sb = singles.tile([128, 20, 128], BF16)
            msk_sb = singles.tile([128, 1], F32)
            negm_sb = singles.tile([128, 2, 16], FP8)
            nc.gpsimd.dma_start(out=wf_sb, in_=wfirst[:])
            nc.scalar.dma_start(out=msk_sb, in_=_bcast(msk[:].rearrange("a b -> (a b)")))
            nc.scalar.dma_start(out=negm_sb, in_=negm[:])
            nc.gpsimd.dma_start(out=wr_sb, in_=wrest[:])

            def wk_ap(p):
                return wf_sb[:, p, :] if p < 2 else wr_sb[:, p - 2, :]

            def wq_ap(p):
                return wf_sb[:, 2, :] if p < 1 else wr_sb[:, 5 + p, :]

            def wv_ap(p):
                return wf_sb[:, 3, :] if p < 1 else wr_sb[:, 12 + p, :]

            if apply_qkvb:
                bqk_sb = singles.tile([128, 2 * NP], F32)
                nc.gpsimd.dma_start(out=bqk_sb, in_=bqk[:])
                bq_sb = bqk_sb[:, 0:NP]
                bk_sb = bqk_sb[:, NP:2 * NP]
                bv_bc = singles.tile([128, NP, 128], F32)
                nc.gpsimd.dma_start(out=bv_bc, in_=_bcast(bvt[:]))
            # bulk tail-only tensors (wo8/xq/ln): the DMA issue is DEFERRED
            # into pair 2's piece stream -- issuing them at t=0 saturates
            # HBM and starves the startup-critical xT/weight loads
            wo8_sb = singles.tile([128, NP, D], FP8)
            xq_sb = singles.tile([128, 4, D], F32)
            if apply_gb:
                g_bc = singles.tile([128, D], F32)
                b_bc = singles.tile([128, D], F32)

            def bulk_dma_piece():
                # dummy one-element pre-writes sourced from oT8[:,0] (only
                # available after pair-0's norm): gives the DMAs a real WAW
                # dependency so the scheduler cannot hoist them to t=0
                # (they are otherwise dependency-free and get reordered
                # right back into the startup HBM crunch)
                nc.vector.tensor_copy(out=wo8_sb[0:1, 0, 0:1],
                                      in_=oT8_sb[0:1, 0, 0:1])
                nc.vector.tensor_copy(out=xq_sb[0:1, 0, 0:1],
                                      in_=oT8_sb[0:1, 0, 0:1])
                nc.gpsimd.dma_start(out=wo8_sb, in_=wo8[:])
                nc.gpsimd.dma_start(out=xq_sb, in_=xq[:])
                if apply_gb:
                    nc.vector.tensor_copy(out=g_bc[0:1, 0:1],
                                          in_=oT8_sb[0:1, 0, 0:1])
                    nc.vector.tensor_copy(out=b_bc[0:1, 0:1],
                                          in_=oT8_sb[0:1, 0, 0:1])
                    nc.gpsimd.dma_start(out=g_bc, in_=_bcast(gg[:]))
                    nc.gpsimd.dma_start(out=b_bc, in_=_bcast(bb[:]))
            eps_sb = singles.tile([128, 1], F32)
            nc.vector.memset(eps_sb, LN_EPS)
            # oT is stored fp8 scaled by 64 (wo is prescaled by 8 on host;
            # the residual add divides by 512): bcast = (SCALE/64)*den, so
            # 1/bcast = 64/(SCALE*den)
            ones_sc = singles.tile([1, d], BF16)
            nc.vector.memset(ones_sc, SCALE / 64.0)
            oT8_sb = singles.tile([128, NP, SQ], FP8)

            # Touch DMA-loaded constants once on VectorE / GpSimd so later
            # consumers need no DMA waits.
            scr = singles.tile([128, 8], F32)
            touches = [msk_sb[:, 0:1]]
            if apply_qkvb:
                touches += [bqk_sb[:, 0:1], bv_bc[:, 0, 0:1]]
            for i, t in enumerate(touches):
                nc.vector.tensor_copy(out=scr[:, i:i + 1], in_=t)
            scr8 = singles.tile([128, 16], FP8)
            nc.gpsimd.tensor_copy(out=scr8, in_=negm_sb[:, 0, :])

            # ---- per-pair qkv emission pieces -------------------------
            built = {}

            def qkv_pieces(p):
                xT_t = xpool.tile([128, S], BF16, name="xT_t")
                xqT_t = qxpool.tile([128, SQ], BF16, name="xqT_t")
                kT_t = kpool.tile([128, S], BF16, name="kT_t")
                qT_t = qpool.tile([128, 2, SQ], BF16, name="qT_t")
                v_t = vpool.tile([128, TCK, 2, 80], FP8, name="v_t")
                built[p] = (kT_t, qT_t, v_t)
                head = []

                def dma_piece():
                    # xqT on the scalar ring (parallel to sync); xT split in
                    # 4 so k_piece(c) waits only on its own 512-col chunk
                    nc.scalar.dma_start(out=xqT_t, in_=xqT[ds(128 * p, 128), :])
                    for c in range(4):
                        nc.sync.dma_start(out=xT_t[:, ts(c, 512)],
                                          in_=xT[ds(128 * p, 128), ts(c, 512)])
                head.append(dma_piece)

                def k_piece(c):
                    def f():
                        ps = psD.tile([128, 512], F32, tag="qkv", name="ps")
                        nc.tensor.matmul(ps, lhsT=wk_ap(p),
                                         rhs=xT_t[:, ts(c, 512)],
                                         start=True, stop=True)
                        if apply_qkvb:
                            nc.vector.tensor_scalar(
                                out=kT_t[:, ts(c, 512)], in0=ps,
                                scalar1=bk_sb[:, p:p + 1],
                                scalar2=None, op0=OP.add)
                        else:
                            nc.vector.tensor_copy(out=kT_t[:, ts(c, 512)],
                                                  in_=ps)
                    return f
                for c in range(4):
                    head.append(k_piece(c))

                def q_piece():
                    ps = psD.tile([128, 512], F32, tag="qkv", name="ps")
                    nc.tensor.matmul(ps, lhsT=wq_ap(p), rhs=xqT_t,
                                     start=True, stop=True)
                    if apply_qkvb:
                        nc.vector.tensor_scalar(out=qT_t[:, 0, :], in0=ps,
                                                scalar1=bq_sb[:, p:p + 1],
                                                scalar2=None, op0=OP.add)
                    else:
                        nc.vector.tensor_copy(out=qT_t[:, 0, :], in_=ps)
                    # masked-query variant for key chunks >= 8: col 511
                    # scaled by msk (0 on the masked core -> score 0)
                    if apply_qkvb:
                        nc.vector.tensor_scalar(out=qT_t[:, 1, :], in0=ps,
                                                scalar1=bq_sb[:, p:p + 1],
                                                scalar2=None, op0=OP.add)
                    else:
                        nc.vector.tensor_copy(out=qT_t[:, 1, :], in_=ps)
                    nc.gpsimd.tensor_scalar(out=qT_t[:, 1, 511:512],
                                            in0=qT_t[:, 1, 511:512],
                                            scalar1=msk_sb[:, 0:1],
                                            scalar2=None, op0=OP.mult)
                head.append(q_piece)

                def ones_piece():
                    nc.gpsimd.memset(v_t[:, :, :, 64:65], 1.0)
                head.append(ones_piece)

                tail = []

                def v_piece(tc0):
                    def f():
                        ps = psD.tile([128, 512], F32, tag="qkv", name="ps")
                        for j in range(4):
                            nc.tensor.matmul(ps[:, ts(j, 128)],
                                             lhsT=xT_t[:, ds(128 * (tc0 + j), 128)],
                                             rhs=wv_ap(p),
                                             start=True, stop=True)
                        if apply_qkvb:
                            for j in range(4):
                                nc.vector.tensor_tensor(
                                    out=v_t[:, tc0 + j, :, 0:64],
                                    in0=ps[:, ts(j, 128)].rearrange(
                                        "a (h e) -> a h e", h=2),
                                    in1=bv_bc[:, p, :].rearrange(
                                        "a (h e) -> a h e", h=2),
                                    op=OP.add)
                        else:
                            nc.vector.tensor_copy(
                                out=v_t[:, tc0:tc0 + 4, :, 0:64],
                                in_=ps.rearrange("a (c h e) -> a c h e",
                                                 c=4, h=2))
                    return f
                for tc0 in range(0, TCK, 4):
                    tail.append(v_piece(tc0))
                return head, tail

            # ---- normalization tail: oT8[:, pp, :] = oTr * 64/(SCALE*den)
            dens = {}

            def emit_norm_tail(pp, bc):
                den, oTr = dens.pop(pp)
                nc.tensor.matmul(bc[0:64, :], lhsT=ones_sc[0:1, :],
                                 rhs=den[0:1, 0, :], start=True, stop=True)
                nc.tensor.matmul(bc[64:128, :], lhsT=ones_sc[0:1, :],
                                 rhs=den[0:1, 1, :], start=True, stop=True)
                scale_t = rpool.tile([128, 512], F32, tag="rs", name="scale_t")
                nc.vector.reciprocal_approx_fast(out=scale_t, in_=bc)
                # last pair's norm gates the whole projection tail: the DVE
                # is faster than GpSimd and free at that point
                eng = nc.vector if pp == NP - 1 else nc.gpsimd
                eng.tensor_tensor(out=oT8_sb[:, pp, :],
                                  in0=oTr, in1=scale_t,
                                  op=OP.mult)

            def proj_mm(t, pp, m, fc, start, stop):
                nc.tensor.matmul(t,
                                 lhsT=oT8_sb[:, 2 * pp:2 * pp + 2, ts(m, 128)],
                                 rhs=wo8_sb[:, 2 * pp:2 * pp + 2, ts(fc, 512)],
                                 start=start, stop=stop, perf_mode=DR)

            # psD pre-accumulated projection groups for m=3 (built during
            # the last pair's attention stream; pair-pairs 0..2 = pairs 0..5)
            prd = {}

            def prd_pieces():
                # t30/t31 allocation is deferred past chunk 6 so the
                # norm_tail(6) bc tile grabs a psD slot first (the t3x
                # slots are only released in the LN tail -> cycle)
                def acc_piece(pp):
                    def f():
                        if pp == 0:
                            prd[(3, 0)] = psD.tile([128, 512], F32,
                                                   tag="qkv", name="t30")
                            prd[(3, 1)] = psD.tile([128, 512], F32,
                                                   tag="qkv", name="t31")
                        for fc0 in range(2):
                            proj_mm(prd[(3, fc0)], pp, 3, fc0,
                                    start=(pp == 0), stop=False)
                    return f
                return [], [None] * 7 + [acc_piece(pp) for pp in range(3)]

            # ---- attention: score chunks -> exp (Scalar/DVE split) ->
            # DoubleRow PV per chunk-pair, lagged
            pvq = []

            def pop_pv():
                fn = pvq.pop(0)
                fn()

            head0, tail0 = qkv_pieces(0)
            for piece in head0:
                piece()

            for p in range(NP):
                if p == 0:
                    head, tail = qkv_pieces(1)
                    pieces = tail0 + head + tail
                elif p + 1 < NP:
                    head, tail = qkv_pieces(p + 1)
                    pieces = head + tail
                    if p == 1:
                        pieces = head + [bulk_dma_piece] + tail
                else:
                    head, tail = prd_pieces()
                    pieces = tail
                kT_t, qT_t, v_t = built.pop(p)
                oA = psB.tile([65, 512], F32, tag="ov", name="oA")
                oB = psB.tile([65, 512], F32, tag="ov", name="oB")

                def mk_pv(oA, oB, v_t, p, cp, ex2):
                    def f():
                        for h, o in ((0, oA), (1, oB)):
                            nc.tensor.matmul(
                                o[:, :],
                                lhsT=v_t[:, 2 * cp:2 * cp + 2, h, 0:65],
                                rhs=ex2[:, h, :, :],
                                start=(cp == 0), stop=False,
                                perf_mode=DR)
                        if cp >= NCP // 2:
                            # subtract the spurious exp=1 contribution of
                            # the zeroed masked-query column (all-zero
                            # rhs on unmasked cores)
                            for h, o in ((0, oA), (1, oB)):
                                nc.tensor.matmul(
                                    o[:, 511:512],
                                    lhsT=v_t[:, 2 * cp:2 * cp + 2, h, 0:65],
                                    rhs=negm_sb[:, :, 0:1],
                                    start=False, stop=(cp == NCP - 1),
                                    perf_mode=DR)
                        if cp == NCP - 1:
                            # Drain oA/oB (raw bf16); denominators from row 64.
                            oTr = orpool.tile([128, 512], BF16, tag="or",
                                              name="oTr")
                            nc.vector.tensor_copy(out=oTr[0:64, :],
                                                  in_=oA[0:64, :])
                            nc.vector.tensor_copy(out=oTr[64:128, :],
                                                  in_=oB[0:64, :])
                            den = rpool.tile([1, 2, 512], BF16, tag="den",
                                             name="den")
                            if p == NP - 1:
                                # split across engines: this chain gates
                                # the whole projection/LN tail
                                nc.scalar.copy(out=den[0:1, 0, :],
                                               in_=oA[64:65, :])
                                nc.vector.tensor_copy(out=den[0:1, 1, :],
                                                      in_=oB[64:65, :])
                            else:
                                nc.vector.tensor_copy(out=den[0:1, 0, :],
                                                      in_=oA[64:65, :])
                                nc.vector.tensor_copy(out=den[0:1, 1, :],
                                                      in_=oB[64:65, :])
                            dens[p] = (den, oTr)
                    return f

                ex2 = None
                for c in range(TCK):
                    cp, ci = divmod(c, 2)
                    if ci == 0:
                        ex2 = epool.tile([128, 2, 2, 512], FP8, name="ex2")
                    qv = 1 if c >= TCK // 2 else 0
                    if c == 0:
                        # chunk 0 scores go to psD banks (free since this
                        # pair's v-casts) so they can be computed during the
                        # PREVIOUS pair's tail; the first exp of this pair
                        # then starts the moment the last exp of the
                        # previous pair retires (no psA-bank seam stall)
                        sc0h = [psD.tile([128, 512], F32, tag="qkv",
                                         name="sc0h") for _ in range(2)]
                        with tc.high_priority():
                            for h in range(2):
                                nc.tensor.matmul(
                                    sc0h[h],
                                    lhsT=kT_t[ds(64 * h, 64), 0:128],
                                    rhs=qT_t[ds(64 * h, 64), qv, :],
                                    start=True, stop=True)
                            for h in range(2):
                                nc.scalar.activation(out=ex2[:, h, 0, :],
                                                     in_=sc0h[h],
                                                     func=AF.Exp)
                        if pieces:
                            piece = pieces.pop(0)
                            if piece:
                                piece()
                        continue
                    sc = psA.tile([128, 2, 512], F32, tag="sc", name="sc")
                    with tc.high_priority():
                        nc.tensor.matmul(sc[:, 0, :],
                                         lhsT=kT_t[0:64, ds(128 * c, 128)],
                                         rhs=qT_t[0:64, qv, :],
                                         start=True, stop=True)
                        nc.tensor.matmul(sc[:, 1, :],
                                         lhsT=kT_t[64:128, ds(128 * c, 128)],
                                         rhs=qT_t[64:128, qv, :],
                                         start=True, stop=True)
                    if c in DVE_CHUNKS:
                        with tc.high_priority():
                            nc.vector.tensor_scalar(
                                out=ex2[:, :, ci, :].bitcast(I8), in0=sc,
                                scalar1=EXPA, scalar2=EXPB,
                                op0=OP.mult, op1=OP.add)
                    else:
                        with tc.high_priority():
                            nc.scalar.activation(out=ex2[:, :, ci, :], in_=sc,
                                                 func=AF.Exp)
                    if ci == 1:
                        pvq.append(mk_pv(oA, oB, v_t, p, cp, ex2))
                    if len(pvq) >= 2 and (ci != 1 or cp != NCP - 1
                                          or p == NP - 1):
                        # defer the last chunk-pair's pop across the pair
                        # seam (except the final pair, whose drain gates
                        # the tail)
                        pop_pv()
                    if c == 6 and p > 0:
                        bc = psD.tile([128, 512], F32, tag="qkv", name="bc")
                        emit_norm_tail(p - 1, bc)
                    npiece = 2 if (p == 0 and c < 8) else 1
                    for _ in range(npiece):
                        if pieces:
                            piece = pieces.pop(0)
                            if piece:
                                piece()

            while pvq:
                pop_pv()

            # sqrt table preload: fills ScalarE's idle window right after
            # the last exp so the LN sqrts don't pay the table switch.
            # Reads the last sc tile so the scheduler cannot hoist it early
            # (which would evict the exp table set before the exps run).
            dum = stpool.tile([128, 1], F32, tag="dum", name="dum")
            nc.scalar.activation(out=dum, in_=sc[:, 0, 0:1], func=AF.Sqrt)

            # last pair's normalization first (bc takes the psA slot freed
            # at the last exp)
            prs = {}
            bc7 = psA.tile([128, 2, 512], F32, tag="sc", name="bc7")
            with tc.high_priority():
                emit_norm_tail(NP - 1, bc7[:, 0, :])

            if apply_gb:
                for i, t in enumerate([g_bc[:, 0:1], b_bc[:, 0:1]]):
                    nc.vector.tensor_copy(out=scr[:, 6 + i:7 + i], in_=t)

            # ---- finish projections + residual + fused-stats LayerNorm.
            # Per-group (preaccs + finishing stop + residual add) so the
            # first LN chains start while later groups still project.
            # m=3 was pre-accumulated in psD during the pair-7 stream; m=0
            # uses the psA slot freed by the last exps; m=1 the psB slots
            # freed by the pair-7 drains; m=2 (last) the two bc7 halves.
            def group_tiles(m):
                if m == 3:
                    return [prd[(3, 0)], prd[(3, 1)]], False
                if m == 0:
                    pr2a = psA.tile([128, 2, 512], F32, tag="sc", name="pr2a")
                    return [pr2a[:, 0, :], pr2a[:, 1, :]], True
                if m == 1:
                    return [psB.tile([128, 512], F32, tag="ov", name="prb")
                            for _ in range(2)], True
                return [bc7[:, 1, :], bc7[:, 0, :]], True

            out_queues = [nc.sync, nc.scalar]
            ys = {}
            for m in (3, 0, 1, 2):
                tiles, need_acc = group_tiles(m)
                y_t = ypool.tile([128, D], F32, tag="y", name="y_t")
                sums = stpool.tile([128, 3], F32, tag="sums", name="sums")
                for fc in range(2):
                    pr = tiles[fc]
                    if need_acc:
                        for pp in range(3):
                            proj_mm(pr, pp, m, fc, start=(pp == 0),
                                    stop=False)
                    with tc.high_priority():
                        proj_mm(pr, 3, m, fc, start=False, stop=True)
                    nc.vector.scalar_tensor_tensor(
                        out=y_t[:, ts(fc, 512)], in0=pr, scalar=1.0 / 512.0,
                        in1=xq_sb[:, m, ts(fc, 512)],
                        op0=OP.mult, op1=OP.add,
                        accum_out=sums[:, fc:fc + 1])
                ysq = ypool.tile([128, D], BF16, tag="ysq", name="ysq")
                nc.scalar.activation(out=ysq, in_=y_t, func=AF.Square,
                                     accum_out=sums[:, 2:3])
                ys[m] = (y_t, sums)

            # phase 2: stats combine, normalize, store
            for m in (3, 0, 1, 2):
                y_t, sums = ys.pop(m)
                mv = stpool.tile([128, 2], F32, tag="mv", name="mv")
                nc.vector.scalar_tensor_tensor(
                    out=mv[:, 0:1], in0=sums[:, 0:1], scalar=1.0,
                    in1=sums[:, 1:2], op0=OP.mult, op1=OP.add)
                nc.vector.tensor_scalar(out=mv[:, 0:1], in0=mv[:, 0:1],
                                        scalar1=1.0 / D, scalar2=None,
                                        op0=OP.mult)
                nc.vector.tensor_tensor(out=mv[:, 1:2], in0=mv[:, 0:1],
                                        in1=mv[:, 0:1], op=OP.mult)
                var = stpool.tile([128, 1], F32, tag="var", name="var")
                nc.vector.scalar_tensor_tensor(
                    out=var, in0=sums[:, 2:3], scalar=1.0 / D,
                    in1=mv[:, 1:2], op0=OP.mult, op1=OP.subtract)
                sd = stpool.tile([128, 1], F32, tag="sd", name="sd")
                nc.scalar.activation(out=sd, in_=var, func=AF.Sqrt,
                                     bias=eps_sb[:, 0:1], scale=1.0)
                rstd = stpool.tile([128, 1], F32, tag="rsd", name="rstd")
                nc.vector.reciprocal(out=rstd, in_=sd)
                yn = ypool.tile([128, D], F32, tag="yn", name="yn")
                nc.vector.tensor_scalar(out=yn, in0=y_t, scalar1=mv[:, 0:1],
                                        scalar2=rstd, op0=OP.subtract,
                                        op1=OP.mult)
                if apply_gb:
                    ot = ypool.tile([128, D], F32, tag="ot", name="ot")
                    nc.vector.tensor_tensor(out=ot[:, 0:512], in0=yn[:, 0:512],
                                            in1=g_bc[:, 0:512], op=OP.mult)
                    nc.vector.tensor_tensor(out=ot[:, 512:1024],
                                            in0=yn[:, 512:1024],
                                            in1=g_bc[:, 512:1024], op=OP.mult)
                    nc.vector.tensor_tensor(out=ot[:, 0:512], in0=ot[:, 0:512],
                                            in1=b_bc[:, 0:512], op=OP.add)
                    nc.vector.tensor_tensor(out=ot[:, 512:1024],
                                            in0=ot[:, 512:1024],
                                            in1=b_bc[:, 512:1024], op=OP.add)
                    for fc in range(2):
                        out_queues[fc].dma_start(
                            out=out[ds(128 * m, 128), ts(fc, 512)],
                            in_=ot[:, ts(fc, 512)])
                else:
                    for fc in range(2):
                        out_queues[fc].dma_start(
                            out=out[ds(128 * m, 128), ts(fc, 512)],
                            in_=yn[:, ts(fc, 512)])
    nc.compile()
    return nc


def prep_inputs(x, Wq, bq, Wk, bk, Wv, bv, Wo, bo, ln_g, ln_b):
    """Host-side sharding/layout prep -> list of 8 per-core input maps."""
    bf = ml_dtypes.bfloat16
    x = np.asarray(x, np.float32)
    Wq, Wk, Wv = (np.asarray(w, np.float32) for w in (Wq, Wk, Wv))
    Wo = np.asarray(Wo, np.float32)
    bq, bk, bv, bo = (np.asarray(v_, np.float32) for v_ in (bq, bk, bv, bo))
    ln_g, ln_b = np.asarray(ln_g, np.float32), np.asarray(ln_b, np.float32)

    def pairs(W):  # [H,d,d] -> [128,NP,128]: block-diag per pair, part-major
        out = np.zeros((NP, 128, 128), np.float32)
        for p in range(NP):
            out[p, :d, :d] = W[2 * p]
            out[p, d:, d:] = W[2 * p + 1]
        return np.ascontiguousarray(out.transpose(1, 0, 2)).astype(bf)

    wq_b, wk_b, wv_b = pairs(Wq), pairs(Wk), pairs(Wv)
    wfirst = np.ascontiguousarray(np.stack(
        [wk_b[:, 0], wk_b[:, 1], wq_b[:, 0], wv_b[:, 0]], axis=1))
    wrest = np.ascontiguousarray(np.concatenate(
        [wk_b[:, 2:NP], wq_b[:, 1:NP], wv_b[:, 1:NP]], axis=1))
    bqk = np.concatenate([bq.reshape(NP, 128).T, bk.reshape(NP, 128).T],
                         1).copy()             # [128, 2*NP]
    bvt = bv.reshape(NP, 128).copy()            # [NP, 128]
    e4 = ml_dtypes.float8_e4m3fn
    wo8_b = np.ascontiguousarray(
        (Wo * 8.0).reshape(NP, 128, D).transpose(1, 0, 2)).astype(e4)
    xT_all = [np.ascontiguousarray(x[b_].T).astype(bf) for b_ in range(B)]

    in_maps = []
    for c in range(N_CORES):
        b_, j = divmod(c, 4)
        rows = slice(j * SQ, (j + 1) * SQ)
        xq_pre = np.ascontiguousarray(
            (x[b_, rows] + bo).reshape(4, 128, D).transpose(1, 0, 2)
        ).astype(np.float32)                    # [128, 4, D]
        masked = (j == 3)
        negm = np.zeros((128, 2, 16), e4)
        if masked:
            negm[:, :, 0] = -1.0
        in_maps.append({
            "xT": xT_all[b_],
            "xqT": np.ascontiguousarray(xT_all[b_][:, rows]),
            "xq": xq_pre,
            "wfirst": wfirst, "wrest": wrest,
            "bqk": bqk, "bvt": bvt,
            "wo8": wo8_b.view(np.uint8),
            "gg": ln_g, "bb": ln_b,
            "msk": np.array([[0.0 if masked else 1.0]], np.float32),
            "negm": negm.view(np.uint8),
        })
    return in_maps


_NC = {}


def _get_nc(apply_gb, apply_qkvb):
    key = (apply_gb, apply_qkvb)
    if key not in _NC:
        _NC[key] = build_nc(apply_gb=apply_gb, apply_qkvb=apply_qkvb)
    return _NC[key]


def _gather(results):
    y = np.empty((B, S, D), np.float32)
    for c, r in enumerate(results):
        b_, j = divmod(c, 4)
        y[b_, j * SQ:(j + 1) * SQ] = r["out"]
    return y


def _needs_gb(ln_g, ln_b):
    return not (np.all(np.asarray(ln_g) == 1.0)
                and np.all(np.asarray(ln_b) == 0.0))


def _needs_qkvb(bq, bk, bv):
    return not all(np.all(np.asarray(b) == 0.0) for b in (bq, bk, bv))


def kernel(**inputs):
    apply_gb = _needs_gb(inputs["ln_g"], inputs["ln_b"])
    apply_qkvb = _needs_qkvb(inputs["bq"], inputs["bk"], inputs["bv"])
    nc = _get_nc(apply_gb, apply_qkvb)
    in_maps = prep_inputs(**inputs)
    res = run_bass_kernel_spmd(nc, in_maps, core_ids=list(range(N_CORES)))
    return _gather(res.results)


def kernel_timed(**inputs):
    """Returns (output, exec_time_ns or None). Used by test.py."""
    apply_gb = _needs_gb(inputs["ln_g"], inputs["ln_b"])
    apply_qkvb = _needs_qkvb(inputs["bq"], inputs["bk"], inputs["bv"])
    nc = _get_nc(apply_gb, apply_qkvb)
    in_maps = prep_inputs(**inputs)
    res = run_bass_kernel_spmd(nc, in_maps, core_ids=list(range(N_CORES)),
                               trace=True)
    return _gather(res.results), res.exec_time_ns


# revision 36
# speedup vs baseline: 1.1182x; 1.0359x over previous
"""Trainium2 Bass kernel for a fused multi-head attention layer.

Math (per batch b):
    xh = x.reshape(S, H, d); q/k/v = xh @ W{q,k,v}[h] + b
    scores = q @ k^T  (per head);  scores[-1, -1024:] = -inf
    attn = softmax(scores, -1) / sqrt(D)
    o = concat_h(attn @ v);  proj = o @ Wo + bo
    out = LayerNorm(x + proj) * g + beta

Sharding: 8 cores = 2 batches x 4 query-blocks of 512 rows. Each core
computes K/V for its full batch (duplicated across the 4 cores of a
batch) and Q/attention/projection/LN for its own 512 query rows. No
collectives.

v2 design notes (vs the all-bf16 v1):
  * The exp stream is split between ScalarE (activation Exp -> fp8e4)
    and the DVE (Schraudolph fast-exp: round(a*s + 56) to int8 IS the
    fp8e4 bit pattern of exp(s); verified round-to-nearest+saturate on
    HW).  Both engines also share the PSUM->SBUF cast pool.
  * V and the attention weights are fp8e4; the PV matmuls run in
    DoubleRow perf mode contracting two 128-key chunks at once
    (lhsT [128,2,65] incl the ones-column for the free softmax
    denominator, rhs [128,2,512]).
  * Chunk 0's scores go to psD banks (free mid-pair) instead of psA,
    so they compute during the previous pair's tail and a pair's first
    exp starts the moment the previous pair's last exp retires; DVE
    chunk 15 likewise lets ScalarE run ahead across the seam.
  * The seq-mask costs no per-chunk work: score chunks >= 8 use a
    second qT whose column 511 is zeroed on the masked core, making
    masked scores 0 -> exp = 1 exactly; per-pair DoubleRow fixup
    matmuls with rhs = -mask subtract the spurious sum_v/count from
    the PV output and denominator (all-zero rhs on unmasked cores:
    one SPMD program for all 8 cores).
  * The output projection is fp8e4 DoubleRow over head-pair-pairs (oT
    stored fp8 scaled x64 via the reciprocal constant, Wo prescaled x8
    on host, residual add multiplies by 1/512), halving projection
    matmul count; m=3's pair-pairs 0..2 accumulate in psD during the
    last pair's stream, and the tail emits each (m,fc) group's preaccs
    + finisher + residual-add together so the LN chains overlap later
    groups' matmuls.
  * Bulk tail-only DMAs (wo8/xq/ln) carry a dummy WAW dependency on
    pair-0's normalized output so the scheduler cannot hoist them into
    the startup HBM crunch; first-pair weights are combined into two
    DMA descriptors; xT splits into 4 chunks so each k-cast waits only
    its own 512 columns.
  * v-casts are batched 4 chunks per DVE op; oT normalization mult
    runs on GpSimd (fp8 out) except the tail-critical last pair's
    (DVE).
"""

import numpy as np
import ml_dtypes

import concourse.bass as bass
import concourse.mybir as mybir
import concourse.tile as tile
from concourse import bacc
from concourse.bass import ds, ts
from concourse.bass_utils import run_bass_kernel_spmd

BF16 = mybir.dt.bfloat16
F32 = mybir.dt.float32
FP8 = mybir.dt.float8e4
I8 = mybir.dt.int8
AF = mybir.ActivationFunctionType
OP = mybir.AluOpType
DR = mybir.MatmulPerfMode.DoubleRow

B, S, D, H = 2, 2048, 1024, 16
d = 64            # head dim
NP = H // 2       # 8 head pairs
SQ = S // 4       # 512 query rows per core
TCK = S // 128    # 16 key chunks of 128
NCP = TCK // 2    # 8 chunk-pairs
SEQ_LEN = 1024
SCALE = float(np.sqrt(D))
LN_EPS = 1e-5
N_CORES = 8
EXPA = 8.0 / float(np.log(2.0))   # Schraudolph slope for e4m3 bits
EXPB = 56.0                        # 8 * bias(7)
# chunks handled by the DVE fast-exp (rest on ScalarE); 15 lets the
# ScalarE run ahead into the next pair's psD chunk-0 exps at the seam
DVE_CHUNKS = (5, 9, 13, 15)


def _bcast(ap, p=128):
    """AP replicating `ap` across p partitions (partition step 0)."""
    return bass.AP(tensor=ap.tensor, offset=ap.offset, ap=[[0, p]] + list(ap.ap))


def build_nc(apply_gb=True, apply_qkvb=True):
    nc = bacc.Bacc("TRN2")

    xT = nc.dram_tensor("xT", [D, S], BF16, kind="ExternalInput")       # x[b].T
    xqT = nc.dram_tensor("xqT", [D, SQ], BF16, kind="ExternalInput")    # x[b,rows].T
    xq = nc.dram_tensor("xq", [128, 4, D], F32, kind="ExternalInput")   # x[b,rows]+bo
    # combined weight loads: one DMA descriptor each (descriptor gen on the
    # gpsimd ring is ~640ns apiece and serializes startup)
    wfirst = nc.dram_tensor("wfirst", [128, 4, 128], BF16, kind="ExternalInput")
    wrest = nc.dram_tensor("wrest", [128, 20, 128], BF16, kind="ExternalInput")
    bqk = nc.dram_tensor("bqk", [128, 2 * NP], F32, kind="ExternalInput")
    bvt = nc.dram_tensor("bvt", [NP, 128], F32, kind="ExternalInput")
    wo8 = nc.dram_tensor("wo8", [128, NP, D], FP8, kind="ExternalInput")
    gg = nc.dram_tensor("gg", [D], F32, kind="ExternalInput")
    bb = nc.dram_tensor("bb", [D], F32, kind="ExternalInput")
    msk = nc.dram_tensor("msk", [1, 1], F32, kind="ExternalInput")      # 0 if masked
    negm = nc.dram_tensor("negm", [128, 2, 16], FP8, kind="ExternalInput")
    out = nc.dram_tensor("out", [SQ, D], F32, kind="ExternalOutput")

    with tile.TileContext(nc) as tc:
        with (
            tc.tile_pool(name="singles", bufs=1) as singles,
            tc.tile_pool(name="xpool", bufs=2) as xpool,
            tc.tile_pool(name="kpool", bufs=2) as kpool,
            tc.tile_pool(name="qpool", bufs=2) as qpool,
            tc.tile_pool(name="qxpool", bufs=2) as qxpool,
            tc.tile_pool(name="vpool", bufs=2) as vpool,
            tc.tile_pool(name="epool", bufs=4) as epool,
            tc.tile_pool(name="rpool", bufs=2) as rpool,
            tc.tile_pool(name="orpool", bufs=2) as orpool,
            tc.tile_pool(name="ypool", bufs=8) as ypool,
            tc.tile_pool(name="stpool", bufs=8) as stpool,
            tc.tile_pool(name="psA", bufs=2, space="PSUM") as psA,
            tc.tile_pool(name="psB", bufs=2, space="PSUM") as psB,
            tc.tile_pool(name="psD", bufs=2, space="PSUM") as psD,
        ):
            # ---- warm-up: bridge the PE HAM clock gate until real MMs
            wu = singles.tile([128, 512], BF16)
            nc.vector.memset(wu, 0.0)
            for _ in range(8):
                wps = psD.tile([128, 512], F32, tag="qkv", name="wps")
                nc.tensor.matmul(wps, lhsT=wu[:, 0:128], rhs=wu,
                                 start=True, stop=True)

            # ---- constants / weights (contiguous host-prearranged DMAs).
            # wfirst = [wk p0, wk p1, wq p0, wv p0]; wrest = wk p2..7 +
            # wq p1..7 + wv p1..7 (one descriptor each on the gpsimd ring)
            wf_sb = singles.tile([128, 4, 128], BF16)
            wr_sb = singles.tile([128, 20, 128], BF16)
            msk_sb = singles.tile([128, 1], F32)
            negm_sb = singles.tile([128, 2, 16], FP8)
            nc.gpsimd.dma_start(out=wf_sb, in_=wfirst[:])
            nc.scalar.dma_start(out=msk_sb, in_=_bcast(msk[:].rearrange("a b -> (a b)")))
            nc.scalar.dma_start(out=negm_sb, in_=negm[:])
            nc.gpsimd.dma_start(out=wr_sb, in_=wrest[:])

            def wk_ap(p):
                return wf_sb[:, p, :] if p < 2 else wr_sb[:, p - 2, :]

            def wq_ap(p):
                return wf_sb[:, 2, :] if p < 1 else wr_sb[:, 5 + p, :]

            def wv_ap(p):
                return wf_sb[:, 3, :] if p < 1 else wr_sb[:, 12 + p, :]

            if apply_qkvb:
                bqk_sb = singles.tile([128, 2 * NP], F32)
                nc.gpsimd.dma_start(out=bqk_sb, in_=bqk[:])
                bq_sb = bqk_sb[:, 0:NP]
                bk_sb = bqk_sb[:, NP:2 * NP]
                bv_bc = singles.tile([128, NP, 128], F32)
                nc.gpsimd.dma_start(out=bv_bc, in_=_bcast(bvt[:]))
            # bulk tail-only tensors (wo8/xq/ln): the DMA issue is DEFERRED
            # into pair 2's piece stream -- issuing them at t=0 saturates
            # HBM and starves the startup-critical xT/weight loads
            wo8_sb = singles.tile([128, NP, D], FP8)
            xq_sb = singles.tile([128, 4, D], F32)
            if apply_gb:
                g_bc = singles.tile([128, D], F32)
                b_bc = singles.tile([128, D], F32)

            def bulk_dma_piece():
                # dummy one-element pre-writes sourced from oT8[:,0] (only
                # available after pair-0's norm): gives the DMAs a real WAW
                # dependency so the scheduler cannot hoist them to t=0
                # (they are otherwise dependency-free and get reordered
                # right back into the startup HBM crunch)
                nc.vector.tensor_copy(out=wo8_sb[0:1, 0, 0:1],
                                      in_=oT8_sb[0:1, 0, 0:1])
                nc.vector.tensor_copy(out=xq_sb[0:1, 0, 0:1],
                                      in_=oT8_sb[0:1, 0, 0:1])
                nc.gpsimd.dma_start(out=wo8_sb, in_=wo8[:])
                nc.gpsimd.dma_start(out=xq_sb, in_=xq[:])
                if apply_gb:
                    nc.vector.tensor_copy(out=g_bc[0:1, 0:1],
                                          in_=oT8_sb[0:1, 0, 0:1])
                    nc.vector.tensor_copy(out=b_bc[0:1, 0:1],
                                          in_=oT8_sb[0:1, 0, 0:1])
                    nc.gpsimd.dma_start(out=g_bc, in_=_bcast(gg[:]))
                    nc.gpsimd.dma_start(out=b_bc, in_=_bcast(bb[:]))
            eps_sb = singles.tile([128, 1], F32)
            nc.vector.memset(eps_sb, LN_EPS)
            # oT is stored fp8 scaled by 64 (wo is prescaled by 8 on host;
            # the residual add divides by 512): bcast = (SCALE/64)*den, so
            # 1/bcast = 64/(SCALE*den)
            ones_sc = singles.tile([1, d], BF16)
            nc.vector.memset(ones_sc, SCALE / 64.0)
            oT8_sb = singles.tile([128, NP, SQ], FP8)

            # Touch DMA-loaded constants once on VectorE / GpSimd so later
            # consumers need no DMA waits.
            scr = singles.tile([128, 8], F32)
            touches = [msk_sb[:, 0:1]]
            if apply_qkvb:
                touches += [bqk_sb[:, 0:1], bv_bc[:, 0, 0:1]]
            for i, t in enumerate(touches):
                nc.vector.tensor_copy(out=scr[:, i:i + 1], in_=t)
            scr8 = singles.tile([128, 16], FP8)
            nc.gpsimd.tensor_copy(out=scr8, in_=negm_sb[:, 0, :])

            # ---- per-pair qkv emission pieces -------------------------
            built = {}

            def qkv_pieces(p):
                xT_t = xpool.tile([128, S], BF16, name="xT_t")
                xqT_t = qxpool.tile([128, SQ], BF16, name="xqT_t")
                kT_t = kpool.tile([128, S], BF16, name="kT_t")
                qT_t = qpool.tile([128, 2, SQ], BF16, name="qT_t")
                v_t = vpool.tile([128, TCK, 2, 80], FP8, name="v_t")
                built[p] = (kT_t, qT_t, v_t)
                head = []

                def dma_piece():
                    # xqT on the scalar ring (parallel to sync); xT split in
                    # 4 so k_piece(c) waits only on its own 512-col chunk
                    nc.scalar.dma_start(out=xqT_t, in_=xqT[ds(128 * p, 128), :])
                    for c in range(4):
                        nc.sync.dma_start(out=xT_t[:, ts(c, 512)],
                                          in_=xT[ds(128 * p, 128), ts(c, 512)])
                head.append(dma_piece)

                def k_piece(c):
                    def f():
                        ps = psD.tile([128, 512], F32, tag="qkv", name="ps")
                        nc.tensor.matmul(ps, lhsT=wk_ap(p),
                                         rhs=xT_t[:, ts(c, 512)],
                                         start=True, stop=True)
                        if apply_qkvb:
                            nc.vector.tensor_scalar(
                                out=kT_t[:, ts(c, 512)], in0=ps,
                                scalar1=bk_sb[:, p:p + 1],
                                scalar2=None, op0=OP.add)
                        else:
                            nc.vector.tensor_copy(out=kT_t[:, ts(c, 512)],
                                                  in_=ps)
                    return f
                for c in range(4):
                    head.append(k_piece(c))

                def q_piece():
                    ps = psD.tile([128, 512], F32, tag="qkv", name="ps")
                    nc.tensor.matmul(ps, lhsT=wq_ap(p), rhs=xqT_t,
                                     start=True, stop=True)
                    if apply_qkvb:
                        nc.vector.tensor_scalar(out=qT_t[:, 0, :], in0=ps,
                                                scalar1=bq_sb[:, p:p + 1],
                                                scalar2=None, op0=OP.add)
                    else:
                        nc.vector.tensor_copy(out=qT_t[:, 0, :], in_=ps)
                    # masked-query variant for key chunks >= 8: col 511
                    # scaled by msk (0 on the masked core -> score 0)
                    if apply_qkvb:
                        nc.vector.tensor_scalar(out=qT_t[:, 1, :], in0=ps,
                                                scalar1=bq_sb[:, p:p + 1],
                                                scalar2=None, op0=OP.add)
                    else:
                        nc.vector.tensor_copy(out=qT_t[:, 1, :], in_=ps)
                    nc.gpsimd.tensor_scalar(out=qT_t[:, 1, 511:512],
                                            in0=qT_t[:, 1, 511:512],
                                            scalar1=msk_sb[:, 0:1],
                                            scalar2=None, op0=OP.mult)
                head.append(q_piece)

                def ones_piece():
                    nc.gpsimd.memset(v_t[:, :, :, 64:65], 1.0)
                head.append(ones_piece)

                tail = []

                def v_piece(tc0):
                    def f():
                        ps = psD.tile([128, 512], F32, tag="qkv", name="ps")
                        for j in range(4):
                            nc.tensor.matmul(ps[:, ts(j, 128)],
                                             lhsT=xT_t[:, ds(128 * (tc0 + j), 128)],
                                             rhs=wv_ap(p),
                                             start=True, stop=True)
                        if apply_qkvb:
                            for j in range(4):
                                nc.vector.tensor_tensor(
                                    out=v_t[:, tc0 + j, :, 0:64],
                                    in0=ps[:, ts(j, 128)].rearrange(
                                        "a (h e) -> a h e", h=2),
                                    in1=bv_bc[:, p, :].rearrange(
                                        "a (h e) -> a h e", h=2),
                                    op=OP.add)
                        else:
                            nc.vector.tensor_copy(
                                out=v_t[:, tc0:tc0 + 4, :, 0:64],
                                in_=ps.rearrange("a (c h e) -> a c h e",
                                                 c=4, h=2))
                    return f
                for tc0 in range(0, TCK, 4):
                    tail.append(v_piece(tc0))
                return head, tail

            # ---- normalization tail: oT8[:, pp, :] = oTr * 64/(SCALE*den)
            dens = {}

            def emit_norm_tail(pp, bc):
                den, oTr = dens.pop(pp)
                nc.tensor.matmul(bc[0:64, :], lhsT=ones_sc[0:1, :],
                                 rhs=den[0:1, 0, :], start=True, stop=True)
                nc.tensor.matmul(bc[64:128, :], lhsT=ones_sc[0:1, :],
                                 rhs=den[0:1, 1, :], start=True, stop=True)
                scale_t = rpool.tile([128, 512], F32, tag="rs", name="scale_t")
                nc.vector.reciprocal_approx_fast(out=scale_t, in_=bc)
                # last pair's norm gates the whole projection tail: the DVE
                # is faster than GpSimd and free at that point
                eng = nc.vector if pp == NP - 1 else nc.gpsimd
                eng.tensor_tensor(out=oT8_sb[:, pp, :],
                                  in0=oTr, in1=scale_t,
                                  op=OP.mult)

            def proj_mm(t, pp, m, fc, start, stop):
                nc.tensor.matmul(t,
                                 lhsT=oT8_sb[:, 2 * pp:2 * pp + 2, ts(m, 128)],
                                 rhs=wo8_sb[:, 2 * pp:2 * pp + 2, ts(fc, 512)],
                                 start=start, stop=stop, perf_mode=DR)

            # psD pre-accumulated projection groups for m=3 (built during
            # the last pair's attention stream; pair-pairs 0..2 = pairs 0..5)
            prd = {}

            def prd_pieces():
                # t30/t31 allocation is deferred past chunk 6 so the
                # norm_tail(6) bc tile grabs a psD slot first (the t3x
                # slots are only released in the LN tail -> cycle)
                def acc_piece(pp):
                    def f():
                        if pp == 0:
                            prd[(3, 0)] = psD.tile([128, 512], F32,
                                                   tag="qkv", name="t30")
                            prd[(3, 1)] = psD.tile([128, 512], F32,
                                                   tag="qkv", name="t31")
                        for fc0 in range(2):
                            proj_mm(prd[(3, fc0)], pp, 3, fc0,
                                    start=(pp == 0), stop=False)
                    return f
                return [], [None] * 7 + [acc_piece(pp) for pp in range(3)]

            # ---- attention: score chunks -> exp (Scalar/DVE split) ->
            # DoubleRow PV per chunk-pair, lagged
            pvq = []

            def pop_pv():
                fn = pvq.pop(0)
                fn()

            head0, tail0 = qkv_pieces(0)
            for piece in head0:
                piece()

            for p in range(NP):
                if p == 0:
                    head, tail = qkv_pieces(1)
                    pieces = tail0 + head + tail
                elif p + 1 < NP:
                    head, tail = qkv_pieces(p + 1)
                    pieces = head + tail
                    if p == 1:
                        pieces = head + [bulk_dma_piece] + tail
                else:
                    head, tail = prd_pieces()
                    pieces = tail
                kT_t, qT_t, v_t = built.pop(p)
                oA = psB.tile([65, 512], F32, tag="ov", name="oA")
                oB = psB.tile([65, 512], F32, tag="ov", name="oB")

                def mk_pv(oA, oB, v_t, p, cp, ex2):
                    def f():
                        for h, o in ((0, oA), (1, oB)):
                            nc.tensor.matmul(
                                o[:, :],
                                lhsT=v_t[:, 2 * cp:2 * cp + 2, h, 0:65],
                                rhs=ex2[:, h, :, :],
                                start=(cp == 0), stop=False,
                                perf_mode=DR)
                        if cp >= NCP // 2:
                            # subtract the spurious exp=1 contribution of
                            # the zeroed masked-query column (all-zero
                            # rhs on unmasked cores)
                            for h, o in ((0, oA), (1, oB)):
                                nc.tensor.matmul(
                                    o[:, 511:512],
                                    lhsT=v_t[:, 2 * cp:2 * cp + 2, h, 0:65],
                                    rhs=negm_sb[:, :, 0:1],
                                    start=False, stop=(cp == NCP - 1),
                                    perf_mode=DR)
                        if cp == NCP - 1:
                            # Drain oA/oB (raw bf16); denominators from row 64.
                            oTr = orpool.tile([128, 512], BF16, tag="or",
                                              name="oTr")
                            nc.vector.tensor_copy(out=oTr[0:64, :],
                                                  in_=oA[0:64, :])
                            nc.vector.tensor_copy(out=oTr[64:128, :],
                                                  in_=oB[0:64, :])
                            den = rpool.tile([1, 2, 512], BF16, tag="den",
                                             name="den")
                            if p == NP - 1:
                                # split across engines: this chain gates
                                # the whole projection/LN tail
                                nc.scalar.copy(out=den[0:1, 0, :],
                                               in_=oA[64:65, :])
                                nc.vector.tensor_copy(out=den[0:1, 1, :],
                                                      in_=oB[64:65, :])
                            else:
                                nc.vector.tensor_copy(out=den[0:1, 0, :],
                                                      in_=oA[64:65, :])
                                nc.vector.tensor_copy(out=den[0:1, 1, :],
                                                      in_=oB[64:65, :])
                            dens[p] = (den, oTr)
                    return f

                ex2 = None
                for c in range(TCK):
                    cp, ci = divmod(c, 2)
                    if ci == 0:
                        ex2 = epool.tile([128, 2, 2, 512], FP8, name="ex2")
                    qv = 1 if c >= TCK // 2 else 0
                    if c == 0:
                        # chunk 0 scores go to psD banks (free since this
                        # pair's v-casts) so they can be computed during the
                        # PREVIOUS pair's tail; the first exp of this pair
                        # then starts the moment the last exp of the
                        # previous pair retires (no psA-bank seam stall)
                        sc0h = [psD.tile([128, 512], F32, tag="qkv",
                                         name="sc0h") for _ in range(2)]
                        with tc.high_priority():
                            for h in range(2):
                                nc.tensor.matmul(
                                    sc0h[h],
                                    lhsT=kT_t[ds(64 * h, 64), 0:128],
                                    rhs=qT_t[ds(64 * h, 64), qv, :],
                                    start=True, stop=True)
                            for h in range(2):
                                nc.scalar.activation(out=ex2[:, h, 0, :],
                                                     in_=sc0h[h],
                                                     func=AF.Exp)
                        if pieces:
                            piece = pieces.pop(0)
                            if piece:
                                piece()
                        continue
                    sc = psA.tile([128, 2, 512], F32, tag="sc", name="sc")
                    with tc.high_priority():
                        nc.tensor.matmul(sc[:, 0, :],
                                         lhsT=kT_t[0:64, ds(128 * c, 128)],
                                         rhs=qT_t[0:64, qv, :],
                                         start=True, stop=True)
                        nc.tensor.matmul(sc[:, 1, :],
                                         lhsT=kT_t[64:128, ds(128 * c, 128)],
                                         rhs=qT_t[64:128, qv, :],
                                         start=True, stop=True)
                    if c in DVE_CHUNKS:
                        with tc.high_priority():
                            nc.vector.tensor_scalar(
                                out=ex2[:, :, ci, :].bitcast(I8), in0=sc,
                                scalar1=EXPA, scalar2=EXPB,
                                op0=OP.mult, op1=OP.add)
                    else:
                        with tc.high_priority():
                            nc.scalar.activation(out=ex2[:, :, ci, :], in_=sc,
                                                 func=AF.Exp)
                    if ci == 1:
                        pvq.append(mk_pv(oA, oB, v_t, p, cp, ex2))
                    if len(pvq) >= 2 and (ci != 1 or cp != NCP - 1
                                          or p == NP - 1):
                        # defer the last chunk-pair's pop across the pair
                        # seam (except the final pair, whose drain gates
                        # the tail)
                        pop_pv()
                    if c == 6 and p > 0:
                        bc = psD.tile([128, 512], F32, tag="qkv", name="bc")
                        emit_norm_tail(p - 1, bc)
                    npiece = 2 if (p == 0 and c < 8) else 1
                    for _ in range(npiece):
                        if pieces:
                            piece = pieces.pop(0)
                            if piece:
                                piece()

            while pvq:
                pop_pv()

            # sqrt table preload: fills ScalarE's idle window right after
            # the last exp so the LN sqrts don't pay the table switch.
            # Reads the last sc tile so the scheduler cannot hoist it early
            # (which would evict the exp table set before the exps run).
            dum = stpool.tile([128, 1], F32, tag="dum", name="dum")
            nc.scalar.activation(out=dum, in_=sc[:, 0, 0:1], func=AF.Sqrt)

            # last pair's normalization first (bc takes the psA slot freed
            # at the last exp)
            prs = {}
            bc7 = psA.tile([128, 2, 512], F32, tag="sc", name="bc7")
            with tc.high_priority():
                emit_norm_tail(NP - 1, bc7[:, 0, :])

            if apply_gb:
                for i, t in enumerate([g_bc[:, 0:1], b_bc[:, 0:1]]):
                    nc.vector.tensor_copy(out=scr[:, 6 + i:7 + i], in_=t)

            # ---- finish projections + residual + fused-stats LayerNorm.
            # Per-group (preaccs + finishing stop + residual add) so the
            # first LN chains start while later groups still project.
            # m=3 was pre-accumulated in psD during the pair-7 stream; m=0
            # uses the psA slot freed by the last exps; m=1 the psB slots
            # freed by the pair-7 drains; m=2 (last) the two bc7 halves.
            def group_tiles(m):
                if m == 3:
                    return [prd[(3, 0)], prd[(3, 1)]], False
                if m == 0:
                    pr2a = psA.tile([128, 2, 512], F32, tag="sc", name="pr2a")
                    return [pr2a[:, 0, :], pr2a[:, 1, :]], True
                if m == 1:
                    return [psB.tile([128, 512], F32, tag="ov", name="prb")
                            for _ in range(2)], True
                return [bc7[:, 1, :], bc7[:, 0, :]], True

            out_queues = [nc.sync, nc.scalar]
            ys = {}
            for m in (3, 0, 1, 2):
                tiles, need_acc = group_tiles(m)
                y_t = ypool.tile([128, D], F32, tag="y", name="y_t")
                sums = stpool.tile([128, 3], F32, tag="sums", name="sums")
                for fc in range(2):
                    pr = tiles[fc]
                    if need_acc:
                        for pp in range(3):
                            proj_mm(pr, pp, m, fc, start=(pp == 0),
                                    stop=False)
                    with tc.high_priority():
                        proj_mm(pr, 3, m, fc, start=False, stop=True)
                    nc.vector.scalar_tensor_tensor(
                        out=y_t[:, ts(fc, 512)], in0=pr, scalar=1.0 / 512.0,
                        in1=xq_sb[:, m, ts(fc, 512)],
                        op0=OP.mult, op1=OP.add,
                        accum_out=sums[:, fc:fc + 1])
                ysq = ypool.tile([128, D], BF16, tag="ysq", name="ysq")
                nc.scalar.activation(out=ysq, in_=y_t, func=AF.Square,
                                     accum_out=sums[:, 2:3])
                ys[m] = (y_t, sums)

            # phase 2: stats combine, normalize, store
            for m in (3, 0, 1, 2):
                y_t, sums = ys.pop(m)
                mv = stpool.tile([128, 2], F32, tag="mv", name="mv")
                nc.vector.scalar_tensor_tensor(
                    out=mv[:, 0:1], in0=sums[:, 0:1], scalar=1.0,
                    in1=sums[:, 1:2], op0=OP.mult, op1=OP.add)
                nc.vector.tensor_scalar(out=mv[:, 0:1], in0=mv[:, 0:1],
                                        scalar1=1.0 / D, scalar2=None,
                                        op0=OP.mult)
                nc.vector.tensor_tensor(out=mv[:, 1:2], in0=mv[:, 0:1],
                                        in1=mv[:, 0:1], op=OP.mult)
                var = stpool.tile([128, 1], F32, tag="var", name="var")
                nc.vector.scalar_tensor_tensor(
                    out=var, in0=sums[:, 2:3], scalar=1.0 / D,
                    in1=mv[:, 1:2], op0=OP.mult, op1=OP.subtract)
                sd = stpool.tile([128, 1], F32, tag="sd", name="sd")
                nc.scalar.activation(out=sd, in_=var, func=AF.Sqrt,
                                     bias=eps_sb[:, 0:1], scale=1.0)
                rstd = stpool.tile([128, 1], F32, tag="rsd", name="rstd")
                nc.vector.reciprocal(out=rstd, in_=sd)
                yn = ypool.tile([128, D], F32, tag="yn", name="yn")
                nc.vector.tensor_scalar(out=yn, in0=y_t, scalar1=mv[:, 0:1],
                                        scalar2=rstd, op0=OP.subtract,
                                        op1=OP.mult)
                if apply_gb:
                    ot = ypool.tile([128, D], F32, tag="ot", name="ot")
                    nc.vector.tensor_tensor(out=ot[:, 0:512], in0=yn[:, 0:512],
                                            in1=g_bc[:, 0:512], op=OP.mult)
                    nc.vector.tensor_tensor(out=ot[:, 512:1024],
                                            in0=yn[:, 512:1024],
                                            in1=g_bc[:, 512:1024], op=OP.mult)
                    nc.vector.tensor_tensor(out=ot[:, 0:512], in0=ot[:, 0:512],
                                            in1=b_bc[:, 0:512], op=OP.add)
                    nc.vector.tensor_tensor(out=ot[:, 512:1024],
                                            in0=ot[:, 512:1024],
                                            in1=b_bc[:, 512:1024], op=OP.add)
                    for fc in range(2):
                        out_queues[fc].dma_start(
                            out=out[ds(128 * m, 128), ts(fc, 512)],
                            in_=ot[:, ts(fc, 512)])
                else:
                    for fc in range(2):
                        out_queues[fc].dma_start(
                            out=out[ds(128 * m, 128), ts(fc, 512)],
                            in_=yn[:, ts(fc, 512)])
    nc.compile()
    return nc


def prep_inputs(x, Wq, bq, Wk, bk, Wv, bv, Wo, bo, ln_g, ln_b):
    """Host-side sharding/layout prep -> list of 8 per-core input maps."""
    bf = ml_dtypes.bfloat16
    x = np.asarray(x, np.float32)
    Wq, Wk, Wv = (np.asarray(w, np.float32) for w in (Wq, Wk, Wv))
    Wo = np.asarray(Wo, np.float32)
    bq, bk, bv, bo = (np.asarray(v_, np.float32) for v_ in (bq, bk, bv, bo))
    ln_g, ln_b = np.asarray(ln_g, np.float32), np.asarray(ln_b, np.float32)

    def pairs(W):  # [H,d,d] -> [128,NP,128]: block-diag per pair, part-major
        out = np.zeros((NP, 128, 128), np.float32)
        for p in range(NP):
            out[p, :d, :d] = W[2 * p]
            out[p, d:, d:] = W[2 * p + 1]
        return np.ascontiguousarray(out.transpose(1, 0, 2)).astype(bf)

    wq_b, wk_b, wv_b = pairs(Wq), pairs(Wk), pairs(Wv)
    wfirst = np.ascontiguousarray(np.stack(
        [wk_b[:, 0], wk_b[:, 1], wq_b[:, 0], wv_b[:, 0]], axis=1))
    wrest = np.ascontiguousarray(np.concatenate(
        [wk_b[:, 2:NP], wq_b[:, 1:NP], wv_b[:, 1:NP]], axis=1))
    bqk = np.concatenate([bq.reshape(NP, 128).T, bk.reshape(NP, 128).T],
                         1).copy()             # [128, 2*NP]
    bvt = bv.reshape(NP, 128).copy()            # [NP, 128]
    e4 = ml_dtypes.float8_e4m3fn
    wo8_b = np.ascontiguousarray(
        (Wo * 8.0).reshape(NP, 128, D).transpose(1, 0, 2)).astype(e4)
    xT_all = [np.ascontiguousarray(x[b_].T).astype(bf) for b_ in range(B)]

    in_maps = []
    for c in range(N_CORES):
        b_, j = divmod(c, 4)
        rows = slice(j * SQ, (j + 1) * SQ)
        xq_pre = np.ascontiguousarray(
            (x[b_, rows] + bo).reshape(4, 128, D).transpose(1, 0, 2)
        ).astype(np.float32)                    # [128, 4, D]
        masked = (j == 3)
        negm = np.zeros((128, 2, 16), e4)
        if masked:
            negm[:, :, 0] = -1.0
        in_maps.append({
            "xT": xT_all[b_],
            "xqT": np.ascontiguousarray(xT_all[b_][:, rows]),
            "xq": xq_pre,
            "wfirst": wfirst, "wrest": wrest,
            "bqk": bqk, "bvt": bvt,
            "wo8": wo8_b.view(np.uint8),
            "gg": ln_g, "bb": ln_b,
            "msk": np.array([[0.0 if masked else 1.0]], np.float32),
            "negm": negm.view(np.uint8),
        })
    return in_maps


_NC = {}


def _get_nc(apply_gb, apply_qkvb):
    key = (apply_gb, apply_qkvb)
    if key not in _NC:
        _NC[key] = build_nc(apply_gb=apply_gb, apply_qkvb=apply_qkvb)
    return _NC[key]


def _gather(results):
    y = np.empty((B, S, D), np.float32)
    for c, r in enumerate(results):
        b_, j = divmod(c, 4)
        y[b_, j * SQ:(j + 1) * SQ] = r["out"]
    return y


def _needs_gb(ln_g, ln_b):
    return not (np.all(np.asarray(ln_g) == 1.0)
                and np.all(np.asarray(ln_b) == 0.0))


def _needs_qkvb(bq, bk, bv):
    return not all(np.all(np.asarray(b) == 0.0) for b in (bq, bk, bv))


def kernel(**inputs):
    apply_gb = _needs_gb(inputs["ln_g"], inputs["ln_b"])
    apply_qkvb = _needs_qkvb(inputs["bq"], inputs["bk"], inputs["bv"])
    nc = _get_nc(apply_gb, apply_qkvb)
    in_maps = prep_inputs(**inputs)
    res = run_bass_kernel_spmd(nc, in_maps, core_ids=list(range(N_CORES)))
    return _gather(res.results)


def kernel_timed(**inputs):
    """Returns (output, exec_time_ns or None). Used by test.py."""
    apply_gb = _needs_gb(inputs["ln_g"], inputs["ln_b"])
    apply_qkvb = _needs_qkvb(inputs["bq"], inputs["bk"], inputs["bv"])
    nc = _get_nc(apply_gb, apply_qkvb)
    in_maps = prep_inputs(**inputs)
    res = run_bass_kernel_spmd(nc, in_maps, core_ids=list(range(N_CORES)),
                               trace=True)
    return _gather(res.results), res.exec_time_ns
